# revision 1
# baseline (speedup 1.0000x reference)
"""Causal self-attention (GQA + RoPE + QK-norm) Trainium2 Bass kernel.

Sharding: 8 cores = 4 batches x 2 head-groups.  Core c -> batch c//2,
q heads (c%2)*8..+8, kv heads (c%2)*2..+2.  wproj is row-sharded, so each
core emits a partial (T, C) output; the host sums the two partials per batch.

Device-side layout strategy (per core):
  - x is fed pre-transposed (xT, [C, T]) and bf16-cast by the host.
  - QKV projections produce Q,K token-major ([tok, cols]); RoPE + rms-norm
    run token-major (free-axis per-head reductions), then 128x128 PE
    transposes produce qT/kT feature-major for the attention matmuls.
    V is produced token-major, which is exactly the p@v stationary layout.
  - scores are computed transposed (scoresT[tk, tq]) so that after exp the
    p tiles are already the moving operand for the p@v matmul; the softmax
    denominator comes from a ones-column matmul accumulated in PSUM.
  - exp has no max-subtraction: qk-norm bounds |s| <= sqrt(128) ~ 11.32.
  - output projection accumulates over the 8 local heads; partial written
    fp32 to DRAM.
"""

import numpy as np
import ml_dtypes
from contextlib import ExitStack

import concourse.bass as bass
import concourse.mybir as mybir
import concourse.tile as tile
from concourse import bacc
from concourse.bass_utils import run_bass_kernel_spmd
from concourse.masks import make_identity

BF16 = mybir.dt.bfloat16
F32 = mybir.dt.float32
F32R = mybir.dt.float32r
AF = mybir.ActivationFunctionType

B, T, C = 4, 2048, 2048
H, KV, D = 16, 4, 128
HG, KVG = H // 2, KV // 2          # per-core q heads (8), kv heads (2)
QC, KC = HG * D, KVG * D           # 1024, 256
P = 128
TOKCH = T // P                     # 16 token chunks
NREP = H // KV                     # 4
EPS = 1e-5
NEG = -1.0e5                       # additive causal mask (exp -> 0)


DEBUG_DUMP = False
PHASES = ("A", "B", "C")


def _build():
    nc = bacc.Bacc("TRN2", target_bir_lowering=False, debug=False, num_devices=8)
    xt = nc.dram_tensor("xt", [C, T], BF16, kind="ExternalInput")
    wq = nc.dram_tensor("wq", [C, QC], BF16, kind="ExternalInput")
    wkv = nc.dram_tensor("wkv", [C, 2 * KC], BF16, kind="ExternalInput")
    wp = nc.dram_tensor("wp", [QC, C], BF16, kind="ExternalInput")
    cosd = nc.dram_tensor("cosd", [T, D // 2], F32, kind="ExternalInput")
    sind = nc.dram_tensor("sind", [T, D // 2], F32, kind="ExternalInput")
    out = nc.dram_tensor("out", [T, C], F32, kind="ExternalOutput")
    if DEBUG_DUMP:
        d_qt = nc.dram_tensor("d_qt", [P, HG, T], F32, kind="ExternalOutput")
        d_kt = nc.dram_tensor("d_kt", [P, KVG, T], F32, kind="ExternalOutput")
        d_v = nc.dram_tensor("d_v", [P, TOKCH, KC], F32, kind="ExternalOutput")
        d_yt = nc.dram_tensor("d_yt", [P, HG, T], F32, kind="ExternalOutput")

    with tile.TileContext(nc) as tc, ExitStack() as ctx:
        singles = ctx.enter_context(tc.tile_pool(name="singles", bufs=1))

        # ---- resident tensors ----
        wq_sb = singles.tile([P, C // P, QC], BF16)
        wkv_sb = singles.tile([P, C // P, 2 * KC], BF16)
        wqr = wq.rearrange("(co p) q -> p co q", p=P)
        wkvr = wkv.rearrange("(co p) q -> p co q", p=P)
        for co in range(C // P):
            nc.sync.dma_start(wq_sb[:, co, :], wqr[:, co, :])
            nc.sync.dma_start(wkv_sb[:, co, :], wkvr[:, co, :])
        cos_sb = singles.tile([P, TOKCH, D // 2], F32)
        nc.sync.dma_start(cos_sb, cosd.rearrange("(tc p) d -> p tc d", p=P))
        sin_sb = singles.tile([P, TOKCH, D // 2], F32)
        nc.sync.dma_start(sin_sb, sind.rearrange("(tc p) d -> p tc d", p=P))

        ident = singles.tile([P, P], BF16)
        make_identity(nc, ident)
        ones_col = singles.tile([P, 1], BF16)
        nc.vector.memset(ones_col, 1.0)
        ones_row = singles.tile([1, P], F32)
        nc.vector.memset(ones_row, 1.0)
        zero_col = singles.tile([P, 1], F32)
        nc.vector.memset(zero_col, 0.0)
        eps_col = singles.tile([P, 1], F32)
        nc.vector.memset(eps_col, EPS)
        nc.const_aps.aps[(F32, 0.0)] = zero_col[:]
        nc.const_aps.aps[(F32, EPS)] = eps_col[:]

        # 4 diagonal-block masks: variant o (offset o*128): keep where
        # i >= j + o*128  (j = tk partition, i = tq free)
        mask_sb = singles.tile([P, 4, 512], F32)
        nc.vector.memset(mask_sb, 0.0)
        for o in range(4):
            nc.gpsimd.affine_select(
                out=mask_sb[:, o, :], in_=mask_sb[:, o, :],
                compare_op=mybir.AluOpType.is_ge, fill=NEG,
                base=-o * P, pattern=[[1, 512]], channel_multiplier=-1,
            )

        qT = singles.tile([P, HG, T], BF16)      # [d, h, tok]
        kT = singles.tile([P, KVG, T], BF16)
        v_sb = singles.tile([P, TOKCH, KC], BF16)  # [tok%128, chunk, vcol]
        yT = singles.tile([P, HG, T], BF16)

        # ================= phase A: QKV proj + RoPE + qk-norm =============
        if "A" not in PHASES:
            pass
        else:
         with tc.tile_pool(name="xa", bufs=3) as xpool, \
             tc.tile_pool(name="pa", bufs=2, space="PSUM") as pps, \
             tc.tile_pool(name="sa", bufs=3) as spool:
            for t in range(TOKCH):
                xtile = xpool.tile([P, C // P, P], BF16, tag="xt")
                nc.sync.dma_start(xtile, xt.rearrange("(co p) t -> p co t", p=P)[:, :, t * P:(t + 1) * P])
                ps_q0 = pps.tile([P, 512], F32, tag="q0")
                ps_q1 = pps.tile([P, 512], F32, tag="q1")
                ps_kv = pps.tile([P, 512], F32, tag="kv")
                ps_k = ps_kv[:, 0:KC]
                ps_v = ps_kv[:, KC:2 * KC]
                nco = C // P
                for co in range(nco):
                    lhsT = xtile[:, co, :]
                    st = dict(start=(co == 0), stop=(co == nco - 1))
                    nc.tensor.matmul(ps_q0, lhsT, wq_sb[:, co, 0:512], **st)
                    nc.tensor.matmul(ps_q1, lhsT, wq_sb[:, co, 512:1024], **st)
                    nc.tensor.matmul(ps_kv, lhsT, wkv_sb[:, co, :], **st)

                # V: cast straight to resident token-major buffer
                nc.vector.tensor_copy(v_sb[:, t, :], ps_v)

                # Q/K: fused multi-head rope + rms-norm + cast + transpose
                def rope_norm(ps, nh, dstT, h0, qscale):
                    h2 = D // 2
                    v4 = ps.rearrange("p (h a d) -> p h a d", h=nh, a=2)
                    q1, q2 = v4[:, :, 0, :], v4[:, :, 1, :]
                    r = spool.tile([P, nh, 2, h2], F32, tag=f"rope{nh}")
                    r1, r2 = r[:, :, 0, :], r[:, :, 1, :]
                    s2 = spool.tile([P, nh, h2], F32, tag=f"scr{nh}")
                    cs = cos_sb[:, t, None, :].to_broadcast([P, nh, h2])
                    sn = sin_sb[:, t, None, :].to_broadcast([P, nh, h2])
                    nc.vector.tensor_mul(r1, q1, cs)
                    nc.vector.tensor_mul(s2, q2, sn)
                    nc.vector.tensor_sub(r1, r1, s2)
                    nc.vector.tensor_mul(r2, q1, sn)
                    nc.vector.tensor_mul(s2, q2, cs)
                    nc.vector.tensor_add(r2, r2, s2)
                    rf = r.rearrange("p h a d -> p h (a d)")
                    sq = spool.tile([P, nh, D], F32, tag=f"sq{nh}")
                    nc.scalar.activation(sq, rf, AF.Square)
                    ss = spool.tile([P, nh], F32, tag=f"ss{nh}")
                    nc.vector.tensor_reduce(ss, sq, axis=mybir.AxisListType.X,
                                            op=mybir.AluOpType.add)
                    rt = spool.tile([P, nh], F32, tag=f"rt{nh}")
                    nc.scalar.activation(rt, ss, AF.Sqrt, scale=1.0 / D, bias=EPS)
                    rq = spool.tile([P, nh], F32, tag=f"rq{nh}")
                    nc.vector.reciprocal(rq, rt)
                    if qscale != 1.0:
                        nc.vector.tensor_scalar_mul(rq, rq, qscale)
                    qbf = spool.tile([P, nh, D], BF16, tag=f"qbf{nh}")
                    nc.vector.tensor_mul(qbf, rf, rq[:, :, None].to_broadcast([P, nh, D]))
                    pst = pps.tile([P, 4, P], BF16, tag="tr")
                    for i in range(nh):
                        nc.tensor.transpose(pst[:, i, :], qbf[:, i, :], ident)
                    # one strided copy: psum [128, nh*128] -> nh head slices of dstT
                    nc.vector.tensor_copy(
                        dstT[:, h0:h0 + nh, t * P:(t + 1) * P], pst[:, 0:nh, :])

                qsc = 1.0 / float(np.sqrt(D))
                rope_norm(ps_q0, 4, qT, 0, qsc)
                rope_norm(ps_q1, 4, qT, 4, qsc)
                rope_norm(ps_k, KVG, kT, 0, 1.0)

        # ================= phase B: attention ============================
        if "B" not in PHASES:
            pass
        else:
         with tc.tile_pool(name="psc", bufs=4, space="PSUM") as psc, \
             tc.tile_pool(name="psy", bufs=2, space="PSUM") as psy, \
             tc.tile_pool(name="pss", bufs=2, space="PSUM") as pss, \
             tc.tile_pool(name="pb", bufs=4) as ppool, \
             tc.tile_pool(name="sb", bufs=3) as bpool:
            NT = T // 512  # 4 tq tiles
            for t in range(NT):
                for h in range(HG):
                    g = h // NREP
                    nch = 4 * (t + 1)
                    ps_y = psy.tile([P, 512], F32, tag="y")
                    ps_sden = pss.tile([P, 512], F32, tag="sden")
                    ps_s = ps_sden[0:1, :]
                    for c in range(nch):
                        o = c * P - t * 512
                        col0 = max(o, 0)
                        ps_sc = psc.tile([P, 512], F32, tag="sc")
                        nc.tensor.matmul(
                            ps_sc[:, col0:512], kT[:, g, c * P:(c + 1) * P],
                            qT[:, h, t * 512 + col0:(t + 1) * 512],
                            start=True, stop=True)
                        if o >= 0:
                            # after the col0 shift the partial block is always
                            # the i' >= j triangle
                            nc.vector.tensor_add(ps_sc[:, col0:col0 + P],
                                                 ps_sc[:, col0:col0 + P],
                                                 mask_sb[:, 0, 0:P])
                        pt = ppool.tile([P, 512], BF16, tag="pt")
                        nc.scalar.activation(pt[:, col0:512], ps_sc[:, col0:512], AF.Exp)
                        st = dict(start=(c == 0), stop=(c == nch - 1))
                        nc.tensor.matmul(ps_y[:, col0:512],
                                         v_sb[:, c, g * P:(g + 1) * P],
                                         pt[:, col0:512], **st)
                        nc.tensor.matmul(ps_s[:, col0:512], ones_col,
                                         pt[:, col0:512], **st)
                    rc = bpool.tile([1, 512], F32, tag="rc")
                    nc.vector.reciprocal(rc, ps_s)
                    nc.tensor.matmul(ps_sden, ones_row, rc, start=True, stop=True)
                    rb = bpool.tile([P, 512], F32, tag="rb")
                    nc.vector.tensor_copy(rb, ps_sden)
                    nc.vector.tensor_mul(yT[:, h, t * 512:(t + 1) * 512], ps_y, rb)

        if DEBUG_DUMP:
            with tc.tile_pool(name="dbg", bufs=2) as dpool:
                for h in range(HG):
                    dt_ = dpool.tile([P, T], F32, tag="d")
                    nc.vector.tensor_copy(dt_, qT[:, h, :])
                    nc.sync.dma_start(d_qt[:, h, :], dt_)
                    dt_ = dpool.tile([P, T], F32, tag="d")
                    nc.vector.tensor_copy(dt_, yT[:, h, :])
                    nc.sync.dma_start(d_yt[:, h, :], dt_)
                for g in range(KVG):
                    dt_ = dpool.tile([P, T], F32, tag="d")
                    nc.vector.tensor_copy(dt_, kT[:, g, :])
                    nc.sync.dma_start(d_kt[:, g, :], dt_)
                dt_ = dpool.tile([P, TOKCH * KC], F32, tag="d")
                nc.vector.tensor_copy(dt_.rearrange("p (a b) -> p a b", a=TOKCH), v_sb[:, :, :])
                nc.sync.dma_start(d_v[:, :, :], dt_.rearrange("p (a b) -> p a b", a=TOKCH))

        # ================= phase C: output projection =====================
        if "C" not in PHASES:
            pass
        else:
         with tc.tile_pool(name="wp", bufs=1) as wpool, \
             tc.tile_pool(name="po", bufs=2, space="PSUM") as pso, \
             tc.tile_pool(name="so", bufs=3) as opool:
            wpr = wp.rearrange("(hc p) c -> p hc c", p=P)
            wp_ts = []
            for ct in range(C // 512):
                wp_t = wpool.tile([P, HG, 512], BF16, tag=f"wpt{ct}")
                nc.sync.dma_start(wp_t, wpr[:, :, ct * 512:(ct + 1) * 512])
                wp_ts.append(wp_t)
            for t in range(TOKCH):
                for ct in range(C // 512):
                    ps_o = pso.tile([P, 512], F32, tag="o")
                    for hc in range(HG):
                        nc.tensor.matmul(
                            ps_o, yT[:, hc, t * P:(t + 1) * P], wp_ts[ct][:, hc, :],
                            start=(hc == 0), stop=(hc == HG - 1))
                    ob = opool.tile([P, 512], F32, tag="ob")
                    nc.vector.tensor_copy(ob, ps_o)
                    nc.sync.dma_start(out[t * P:(t + 1) * P, ct * 512:(ct + 1) * 512], ob)
    nc.compile()
    return nc


_NC_CACHE = []


def _get_prog():
    if not _NC_CACHE:
        _NC_CACHE.append(_build())
    return _NC_CACHE[0]


def _make_in_maps(inputs):
    x, cos, sin = inputs["x"], inputs["cos"], inputs["sin"]
    wq, wk, wv, wproj = inputs["wq"], inputs["wk"], inputs["wv"], inputs["wproj"]
    bf = ml_dtypes.bfloat16
    cos2 = np.ascontiguousarray(cos.reshape(T, D // 2), dtype=np.float32)
    sin2 = np.ascontiguousarray(sin.reshape(T, D // 2), dtype=np.float32)
    in_maps = []
    for core in range(8):
        b, g = core // 2, core % 2
        qs = slice(g * QC, (g + 1) * QC)
        ks = slice(g * KC, (g + 1) * KC)
        in_maps.append({
            "xt": np.ascontiguousarray(x[b].T).astype(bf),
            "wq": np.ascontiguousarray(wq[:, qs]).astype(bf),
            "wkv": np.ascontiguousarray(np.hstack([wk[:, ks], wv[:, ks]])).astype(bf),
            "wp": np.ascontiguousarray(wproj[qs, :]).astype(bf),
            "cosd": cos2,
            "sind": sin2,
        })
    return in_maps


def kernel(x, cos, sin, wq, wk, wv, wproj):
    nc = _get_prog()
    in_maps = _make_in_maps(dict(x=x, cos=cos, sin=sin, wq=wq, wk=wk, wv=wv, wproj=wproj))
    res = run_bass_kernel_spmd(nc, in_maps, core_ids=list(range(8))).results
    outp = np.empty((B, T, C), np.float32)
    for b in range(B):
        outp[b] = res[2 * b]["out"] + res[2 * b + 1]["out"]
    return outp



# revision 30
# speedup vs baseline: 1.1488x; 1.1488x over previous
"""Causal self-attention (GQA + RoPE + QK-norm) Trainium2 Bass kernel.

Sharding: 8 cores = 4 batches x 2 head-groups.  Core c -> batch c//2,
q heads (c%2)*8..+8, kv heads (c%2)*2..+2.  wproj is row-sharded, so each
core emits a partial (T, C) output; the host sums the two partials per batch.

Device-side layout strategy (per core):
  - x is fed pre-transposed (xT, [C, T]) and bf16-cast by the host.
  - QKV projections produce Q,K token-major ([tok, cols]); RoPE + rms-norm
    run token-major (free-axis per-head reductions), then 128x128 PE
    transposes produce qT/kT feature-major for the attention matmuls.
    V is produced token-major, which is exactly the p@v stationary layout.
  - scores are computed transposed (scoresT[tk, tq]) so that after exp the
    p tiles are already the moving operand for the p@v matmul; the softmax
    denominator comes from a ones-column matmul accumulated in PSUM.
  - exp has no max-subtraction: qk-norm bounds |s| <= sqrt(128) ~ 11.32.
  - output projection accumulates over the 8 local heads; partial written
    fp32 to DRAM.
"""

import numpy as np
import ml_dtypes
from contextlib import ExitStack

import concourse.bass as bass
import concourse.mybir as mybir
import concourse.tile as tile
from concourse import bacc
from concourse.bass_utils import run_bass_kernel_spmd
from concourse.masks import make_identity

BF16 = mybir.dt.bfloat16
F32 = mybir.dt.float32
F32R = mybir.dt.float32r
AF = mybir.ActivationFunctionType

B, T, C = 4, 2048, 2048
H, KV, D = 16, 4, 128
HG, KVG = H // 2, KV // 2          # per-core q heads (8), kv heads (2)
QC, KC = HG * D, KVG * D           # 1024, 256
P = 128
TOKCH = T // P                     # 16 token chunks
NREP = H // KV                     # 4
EPS = 1e-5
NEG = -1.0e5                       # additive causal mask (exp -> 0)


DEBUG_DUMP = False
PHASES = ("A", "B", "C")


def _build():
    nc = bacc.Bacc("TRN2", target_bir_lowering=False, debug=False, num_devices=8)
    # x pre-tiled by the host as [tokch, p, co, tk] so every DMA partition row
    # is 4KB contiguous (co*tk*2B) instead of 256B strided
    xt = nc.dram_tensor("xt", [TOKCH, P, C // P, P], BF16, kind="ExternalInput")
    wq = nc.dram_tensor("wq", [C, QC], BF16, kind="ExternalInput")
    wkv = nc.dram_tensor("wkv", [C, 2 * KC], BF16, kind="ExternalInput")
    wp = nc.dram_tensor("wp", [QC, C], BF16, kind="ExternalInput")
    # cos/sin pre-tiled by host as [p, tc, d] (contiguous 4KB rows)
    cosd = nc.dram_tensor("cosd", [P, TOKCH, D // 2], F32, kind="ExternalInput")
    sind = nc.dram_tensor("sind", [P, TOKCH, D // 2], F32, kind="ExternalInput")
    out = nc.dram_tensor("out", [T, C], F32, kind="ExternalOutput")
    if DEBUG_DUMP:
        d_qt = nc.dram_tensor("d_qt", [P, HG, T], F32, kind="ExternalOutput")
        d_kt = nc.dram_tensor("d_kt", [P, KVG, T], F32, kind="ExternalOutput")
        d_v = nc.dram_tensor("d_v", [P, TOKCH, KC], F32, kind="ExternalOutput")
        d_yt = nc.dram_tensor("d_yt", [P, HG, T], F32, kind="ExternalOutput")

    with tile.TileContext(nc) as tc, ExitStack() as ctx:
        singles = ctx.enter_context(tc.tile_pool(name="singles", bufs=1))
        xpool = ctx.enter_context(tc.tile_pool(name="xa", bufs=3))

        # ---- prefetch the first x tile before the weight bulk so the PE
        # can start within a few us ----
        xtile0 = xpool.tile([P, C // P, P], BF16, tag="xt")
        for g4 in range(4):
            nc.sync.dma_start(xtile0[:, 4 * g4:4 * (g4 + 1), :],
                              xt[0, :, 4 * g4:4 * (g4 + 1), :])

        # ---- resident tensors ----
        # weight DMAs issued per-co round-robin over three queues so early
        # co chunks land in consumption order and issue rate isn't limited
        # by one sequencer (~600ns per dma_start)
        wq_sb = singles.tile([P, C // P, QC], BF16)
        wkv_sb = singles.tile([P, C // P, 2 * KC], BF16)
        wqr = wq.rearrange("(co p) q -> p co q", p=P)
        wkvr = wkv.rearrange("(co p) q -> p co q", p=P)
        cos_sb = singles.tile([P, TOKCH, D // 2], F32)
        sin_sb = singles.tile([P, TOKCH, D // 2], F32)
        nc.scalar.dma_start(cos_sb, cosd[:])
        nc.scalar.dma_start(sin_sb, sind[:])
        qs = [nc.sync, nc.scalar]
        for co in range(C // P):
            eng = qs[co % 2]
            eng.dma_start(wq_sb[:, co, :], wqr[:, co, :])
            eng.dma_start(wkv_sb[:, co, :], wkvr[:, co, :])

        ident = singles.tile([P, P], BF16)
        make_identity(nc, ident)
        ones_col = singles.tile([P, 1], BF16)
        nc.vector.memset(ones_col, 1.0)
        zero_col = singles.tile([P, 1], F32)
        nc.vector.memset(zero_col, 0.0)
        eps_col = singles.tile([P, 1], F32)
        nc.vector.memset(eps_col, EPS)
        nc.const_aps.aps[(F32, 0.0)] = zero_col[:]
        nc.const_aps.aps[(F32, EPS)] = eps_col[:]
        # scratch for the dummy exp that prewarms the exp act-table at the
        # A->B phase boundary (overlaps the 1.28us table load)
        warm = singles.tile([1, 1], F32)

        # diagonal-block mask: keep where i >= j (j = tk partition, i = tq free)
        mask_sb = singles.tile([P, P], F32)
        nc.vector.memset(mask_sb, 0.0)
        nc.gpsimd.affine_select(
            out=mask_sb, in_=mask_sb,
            compare_op=mybir.AluOpType.is_ge, fill=NEG,
            base=0, pattern=[[1, P]], channel_multiplier=-1,
        )

        qT = singles.tile([P, HG, T], BF16)      # [d, h, tok]
        kT = singles.tile([P, KVG, T], BF16)
        v_sb = singles.tile([P, TOKCH, KC], BF16)  # [tok%128, chunk, vcol]
        yT = singles.tile([P, HG, T], BF16)

        # ================= phase A: QKV proj + RoPE + qk-norm =============
        if "A" not in PHASES:
            pass
        else:
         with tc.tile_pool(name="pa", bufs=2, space="PSUM") as pps, \
             tc.tile_pool(name="sa", bufs=3) as spool:
            for t in range(TOKCH):
                if t == 0:
                    xtile = xtile0
                else:
                    xtile = xpool.tile([P, C // P, P], BF16, tag="xt")
                    nc.sync.dma_start(xtile, xt[t])
                ps_q0 = pps.tile([P, 512], F32, tag="q0")
                ps_q1 = pps.tile([P, 512], F32, tag="q1")
                ps_kv = pps.tile([P, 512], F32, tag="kv")
                ps_k = ps_kv[:, 0:KC]
                ps_v = ps_kv[:, KC:2 * KC]
                nco = C // P
                for co in range(nco):
                    lhsT = xtile[:, co, :]
                    st = dict(start=(co == 0), stop=(co == nco - 1))
                    nc.tensor.matmul(ps_q0, lhsT, wq_sb[:, co, 0:512], **st)
                    nc.tensor.matmul(ps_q1, lhsT, wq_sb[:, co, 512:1024], **st)
                    nc.tensor.matmul(ps_kv, lhsT, wkv_sb[:, co, :], **st)

                # V: cast straight to resident token-major buffer (Act engine;
                # DVE is the critical engine in this phase)
                nc.scalar.copy(v_sb[:, t, :], ps_v)

                # Q/K: fused multi-head rope + rms-norm + cast + transpose
                def rope_norm(ps, nh, dstT, h0, qscale):
                    h2 = D // 2
                    v4 = ps.rearrange("p (h a d) -> p h a d", h=nh, a=2)
                    q1, q2 = v4[:, :, 0, :], v4[:, :, 1, :]
                    r = spool.tile([P, nh, 2, h2], F32, tag=f"rope{nh}")
                    r1, r2 = r[:, :, 0, :], r[:, :, 1, :]
                    s2 = spool.tile([P, nh, h2], F32, tag=f"scr{nh}")
                    cs = cos_sb[:, t, None, :].to_broadcast([P, nh, h2])
                    sn = sin_sb[:, t, None, :].to_broadcast([P, nh, h2])
                    nc.vector.tensor_mul(r1, q1, cs)
                    nc.vector.tensor_mul(s2, q2, sn)
                    nc.vector.tensor_sub(r1, r1, s2)
                    nc.vector.tensor_mul(r2, q1, sn)
                    nc.vector.tensor_mul(s2, q2, cs)
                    nc.vector.tensor_add(r2, r2, s2)
                    rf = r.rearrange("p h a d -> p h (a d)")
                    sq = spool.tile([P, nh, D], F32, tag=f"sq{nh}")
                    nc.scalar.activation(sq, rf, AF.Square)
                    ss = spool.tile([P, nh], F32, tag=f"ss{nh}")
                    nc.vector.tensor_reduce(ss, sq, axis=mybir.AxisListType.X,
                                            op=mybir.AluOpType.add)
                    rt = spool.tile([P, nh], F32, tag=f"rt{nh}")
                    nc.scalar.activation(rt, ss, AF.Sqrt, scale=1.0 / D, bias=EPS)
                    rq = spool.tile([P, nh], F32, tag=f"rq{nh}")
                    nc.vector.reciprocal(rq, rt)
                    if qscale != 1.0:
                        nc.vector.tensor_scalar_mul(rq, rq, qscale)
                    qbf = spool.tile([P, nh, D], BF16, tag=f"qbf{nh}")
                    nc.vector.tensor_mul(qbf, rf, rq[:, :, None].to_broadcast([P, nh, D]))
                    pst = pps.tile([P, 4, P], BF16, tag="tr")
                    for i in range(nh):
                        nc.tensor.transpose(pst[:, i, :], qbf[:, i, :], ident)
                    # one strided copy: psum [128, nh*128] -> nh head slices of
                    # dstT (Act engine; DVE is the critical engine here)
                    nc.scalar.copy(
                        dstT[:, h0:h0 + nh, t * P:(t + 1) * P], pst[:, 0:nh, :])

                qsc = 1.0 / float(np.sqrt(D))
                rope_norm(ps_q0, 4, qT, 0, qsc)
                rope_norm(ps_q1, 4, qT, 4, qsc)
                rope_norm(ps_k, KVG, kT, 0, 1.0)
            # prewarm the exp table while the phase-A tail drains
            nc.scalar.activation(warm, zero_col[0:1, :], AF.Exp)

        # ================= phase B: attention ============================
        # wp prefetch: issue at phase-B start so the tiles are resident long
        # before phase C begins (phase-A pools have closed, SBUF is free)
        wpool = ctx.enter_context(tc.tile_pool(name="wp", bufs=1))
        wpr = wp.rearrange("(hc p) c -> p hc c", p=P)
        wp_ts = []
        for ct in range(C // 512):
            wp_t = wpool.tile([P, HG, 512], BF16, tag=f"wpt{ct}")
            nc.sync.dma_start(wp_t, wpr[:, :, ct * 512:(ct + 1) * 512])
            wp_ts.append(wp_t)

        if "B" not in PHASES:
            pass
        else:
         with tc.tile_pool(name="psc", bufs=5, space="PSUM") as psc, \
             tc.tile_pool(name="psy", bufs=2, space="PSUM") as psy, \
             tc.tile_pool(name="pss", bufs=1, space="PSUM") as pss, \
             tc.tile_pool(name="pb", bufs=6) as ppool, \
             tc.tile_pool(name="sb", bufs=3) as bpool:
            NT = T // 512  # 4 tq tiles
            # software pipeline: the PE queue is in-order, so scores for
            # chunk idx+DEPTH are emitted before pv/ones of chunk idx; the
            # scores->mask->exp chain (~1.6us) hides behind DEPTH chunks of
            # PE work.  The (h, c) stream is flattened so the pipeline also
            # covers head boundaries.
            DEPTH = 4
            for t in range(NT):
                nch = 4 * (t + 1)
                items = [(h, c) for h in range(HG) for c in range(nch)]
                live = {}

                def front(idx):
                    h, c = items[idx]
                    g = h // NREP
                    o = c * P - t * 512
                    col0 = max(o, 0)
                    ps_sc = psc.tile([P, 512], F32, tag="sc")
                    nc.tensor.matmul(
                        ps_sc[:, col0:512], kT[:, g, c * P:(c + 1) * P],
                        qT[:, h, t * 512 + col0:(t + 1) * 512],
                        start=True, stop=True)
                    if o >= 0:
                        # after the col0 shift the partial block is always
                        # the i' >= j triangle
                        nc.vector.tensor_add(ps_sc[:, col0:col0 + P],
                                             ps_sc[:, col0:col0 + P], mask_sb)
                    pt = ppool.tile([P, 512], BF16, tag="pt")
                    nc.scalar.activation(pt[:, col0:512], ps_sc[:, col0:512],
                                         AF.Exp)
                    live[idx] = (pt, col0)

                for i in range(min(DEPTH, len(items))):
                    front(i)
                ys = {}
                for idx, (h, c) in enumerate(items):
                    if idx + DEPTH < len(items):
                        front(idx + DEPTH)
                    g = h // NREP
                    if c == 0:
                        ps_y = psy.tile([P, 512], F32, tag="y")
                        ps_sden = pss.tile([P, 512], F32, tag="sden")
                        ys[h] = (ps_y, ps_sden)
                    ps_y, ps_sden = ys[h]
                    ps_s = ps_sden[0:1, :]
                    pt, col0 = live.pop(idx)
                    st = dict(start=(c == 0), stop=(c == nch - 1))
                    nc.tensor.matmul(ps_y[:, col0:512],
                                     v_sb[:, c, g * P:(g + 1) * P],
                                     pt[:, col0:512], **st)
                    nc.tensor.matmul(ps_s[:, col0:512], ones_col,
                                     pt[:, col0:512], **st)
                    if c == nch - 1:
                        # recip first (frees the single pss buffer fastest),
                        # then copy (frees ps_y); normalize the bf16 slice in
                        # place on the Pool engine once the broadcast lands
                        yslice = yT[:, h, t * 512:(t + 1) * 512]
                        rc = bpool.tile([1, 512], F32, tag="rc")
                        nc.vector.reciprocal(rc, ps_s)
                        nc.vector.tensor_copy(yslice, ps_y)
                        rb = bpool.tile([P, 512], F32, tag="rb")
                        nc.gpsimd.partition_broadcast(rb, rc, channels=P)
                        nc.vector.tensor_mul(yslice, yslice, rb)

        if DEBUG_DUMP:
            with tc.tile_pool(name="dbg", bufs=2) as dpool:
                for h in range(HG):
                    dt_ = dpool.tile([P, T], F32, tag="d")
                    nc.vector.tensor_copy(dt_, qT[:, h, :])
                    nc.sync.dma_start(d_qt[:, h, :], dt_)
                    dt_ = dpool.tile([P, T], F32, tag="d")
                    nc.vector.tensor_copy(dt_, yT[:, h, :])
                    nc.sync.dma_start(d_yt[:, h, :], dt_)
                for g in range(KVG):
                    dt_ = dpool.tile([P, T], F32, tag="d")
                    nc.vector.tensor_copy(dt_, kT[:, g, :])
                    nc.sync.dma_start(d_kt[:, g, :], dt_)
                dt_ = dpool.tile([P, TOKCH * KC], F32, tag="d")
                nc.vector.tensor_copy(dt_.rearrange("p (a b) -> p a b", a=TOKCH), v_sb[:, :, :])
                nc.sync.dma_start(d_v[:, :, :], dt_.rearrange("p (a b) -> p a b", a=TOKCH))

        # ================= phase C: output projection =====================
        if "C" not in PHASES:
            pass
        else:
         with tc.tile_pool(name="po", bufs=2, space="PSUM") as pso, \
             tc.tile_pool(name="so", bufs=3) as opool:
            for t in range(TOKCH):
                for ct in range(C // 512):
                    ps_o = pso.tile([P, 512], F32, tag="o")
                    for hc in range(HG):
                        nc.tensor.matmul(
                            ps_o, yT[:, hc, t * P:(t + 1) * P], wp_ts[ct][:, hc, :],
                            start=(hc == 0), stop=(hc == HG - 1))
                    ob = opool.tile([P, 512], F32, tag="ob")
                    nc.vector.tensor_copy(ob, ps_o)
                    nc.sync.dma_start(out[t * P:(t + 1) * P, ct * 512:(ct + 1) * 512], ob)
    nc.compile()
    return nc


_NC_CACHE = []


def _get_prog():
    if not _NC_CACHE:
        _NC_CACHE.append(_build())
    return _NC_CACHE[0]


def _make_in_maps(inputs):
    x, cos, sin = inputs["x"], inputs["cos"], inputs["sin"]
    wq, wk, wv, wproj = inputs["wq"], inputs["wk"], inputs["wv"], inputs["wproj"]
    bf = ml_dtypes.bfloat16
    # [p, tc, d] tiling (contiguous 4KB DMA rows)
    cos2 = np.ascontiguousarray(
        cos.reshape(TOKCH, P, D // 2).transpose(1, 0, 2), dtype=np.float32)
    sin2 = np.ascontiguousarray(
        sin.reshape(TOKCH, P, D // 2).transpose(1, 0, 2), dtype=np.float32)
    in_maps = []
    for core in range(8):
        b, g = core // 2, core % 2
        qs = slice(g * QC, (g + 1) * QC)
        ks = slice(g * KC, (g + 1) * KC)
        # x[b].T is [C, T]; tile to [tokch, p(C-chunk), co, tk]
        xtb = (x[b].T.astype(bf)
               .reshape(C // P, P, TOKCH, P)     # [co, p, tc, tk]
               .transpose(2, 1, 0, 3))           # [tc, p, co, tk]
        in_maps.append({
            "xt": np.ascontiguousarray(xtb),
            "wq": np.ascontiguousarray(wq[:, qs]).astype(bf),
            "wkv": np.ascontiguousarray(np.hstack([wk[:, ks], wv[:, ks]])).astype(bf),
            "wp": np.ascontiguousarray(wproj[qs, :]).astype(bf),
            "cosd": cos2,
            "sind": sin2,
        })
    return in_maps


def kernel(x, cos, sin, wq, wk, wv, wproj):
    nc = _get_prog()
    in_maps = _make_in_maps(dict(x=x, cos=cos, sin=sin, wq=wq, wk=wk, wv=wv, wproj=wproj))
    res = run_bass_kernel_spmd(nc, in_maps, core_ids=list(range(8))).results
    outp = np.empty((B, T, C), np.float32)
    for b in range(B):
        outp[b] = res[2 * b]["out"] + res[2 * b + 1]["out"]
    return outp



# revision 59
# speedup vs baseline: 1.1845x; 1.0311x over previous
"""Causal self-attention (GQA + RoPE + QK-norm) Trainium2 Bass kernel.

Sharding: 8 cores = 4 batches x 2 head-groups.  Core c -> batch c//2,
q heads (c%2)*8..+8, kv heads (c%2)*2..+2.  wproj is row-sharded, so each
core emits a partial (T, C) output; the host sums the two partials per batch.

Device-side layout strategy (per core):
  - x is fed pre-transposed (xT, [C, T]) and bf16-cast by the host.
  - QKV projections produce Q,K token-major ([tok, cols]); RoPE + rms-norm
    run token-major (free-axis per-head reductions), then 128x128 PE
    transposes produce qT/kT feature-major for the attention matmuls.
    V is produced token-major, which is exactly the p@v stationary layout.
  - scores are computed transposed (scoresT[tk, tq]) so that after exp the
    p tiles are already the moving operand for the p@v matmul; the softmax
    denominator comes from a ones-column matmul accumulated in PSUM.
  - exp has no max-subtraction: qk-norm bounds |s| <= sqrt(128) ~ 11.32.
  - output projection accumulates over the 8 local heads; partial written
    fp32 to DRAM.
"""

import numpy as np
import ml_dtypes
from contextlib import ExitStack

import concourse.bass as bass
import concourse.mybir as mybir
import concourse.tile as tile
from concourse import bacc
from concourse.bass_utils import run_bass_kernel_spmd
from concourse.masks import make_identity

BF16 = mybir.dt.bfloat16
F32 = mybir.dt.float32
F32R = mybir.dt.float32r
AF = mybir.ActivationFunctionType

B, T, C = 4, 2048, 2048
H, KV, D = 16, 4, 128
HG, KVG = H // 2, KV // 2          # per-core q heads (8), kv heads (2)
QC, KC = HG * D, KVG * D           # 1024, 256
P = 128
TOKCH = T // P                     # 16 token chunks
NREP = H // KV                     # 4
EPS = 1e-5
NEG = -1.0e5                       # additive causal mask (exp -> 0)


DEBUG_DUMP = False
PHASES = ("A", "B", "C")


def _build():
    nc = bacc.Bacc("TRN2", target_bir_lowering=False, debug=False, num_devices=8)
    # x pre-tiled by the host as [tokch, p, co, tk] so every DMA partition row
    # is 4KB contiguous (co*tk*2B) instead of 256B strided
    xt = nc.dram_tensor("xt", [TOKCH, P, C // P, P], BF16, kind="ExternalInput")
    wq = nc.dram_tensor("wq", [C, QC], BF16, kind="ExternalInput")
    wkv = nc.dram_tensor("wkv", [C, 2 * KC], BF16, kind="ExternalInput")
    wp = nc.dram_tensor("wp", [QC, C], BF16, kind="ExternalInput")
    # cos/sin pre-tiled by host as [p, tc, d] (contiguous 4KB rows)
    cosd = nc.dram_tensor("cosd", [P, TOKCH, D // 2], F32, kind="ExternalInput")
    sind = nc.dram_tensor("sind", [P, TOKCH, D // 2], F32, kind="ExternalInput")
    out = nc.dram_tensor("out", [T, C], F32, kind="ExternalOutput")
    if DEBUG_DUMP:
        d_qt = nc.dram_tensor("d_qt", [P, HG, T], F32, kind="ExternalOutput")
        d_kt = nc.dram_tensor("d_kt", [P, KVG, T], F32, kind="ExternalOutput")
        d_v = nc.dram_tensor("d_v", [P, TOKCH, KC], F32, kind="ExternalOutput")
        d_yt = nc.dram_tensor("d_yt", [P, HG, T], F32, kind="ExternalOutput")

    with tile.TileContext(nc) as tc, ExitStack() as ctx:
        singles = ctx.enter_context(tc.tile_pool(name="singles", bufs=1))
        # bufs must cover the V-lag window (xtile(t) is re-read by the lagged
        # V projection at iteration t+VLAG); the pool closes with phase A
        phase_a_pools = ExitStack()
        xpool = phase_a_pools.enter_context(tc.tile_pool(name="xa", bufs=8))

        # ---- prefetch the first x tile before the weight bulk so the PE
        # can start within a few us ----
        # ---- resident tensors ----
        # weight DMAs issued per-co round-robin over both HWDGE queues so
        # early co chunks land in consumption order and issue rate isn't
        # limited by one sequencer (~600ns per dma_start).  The first x
        # chunk + first co weights go out first so the PE starts ASAP.
        wq_sb = singles.tile([P, C // P, QC], BF16)
        wkv_sb = singles.tile([P, C // P, 2 * KC], BF16)
        wqr = wq.rearrange("(co p) q -> p co q", p=P)
        wkvr = wkv.rearrange("(co p) q -> p co q", p=P)
        cos_sb = singles.tile([P, TOKCH, D // 2], F32)
        sin_sb = singles.tile([P, TOKCH, D // 2], F32)
        xtile0 = xpool.tile([P, C // P, P], BF16, tag="xt")
        nc.sync.dma_start(xtile0[:, 0:4, :], xt[0, :, 0:4, :])
        nc.scalar.dma_start(wq_sb[:, 0, :], wqr[:, 0, :])
        nc.sync.dma_start(wkv_sb[:, 0, :], wkvr[:, 0, :])
        nc.scalar.dma_start(wq_sb[:, 1, :], wqr[:, 1, :])
        for g4 in range(1, 4):
            nc.sync.dma_start(xtile0[:, 4 * g4:4 * (g4 + 1), :],
                              xt[0, :, 4 * g4:4 * (g4 + 1), :])
        nc.gpsimd.dma_start(wkv_sb[:, 1, :], wkvr[:, 1, :])
        nc.scalar.dma_start(cos_sb, cosd[:])
        nc.sync.dma_start(sin_sb, sind[:])
        qs = [nc.sync, nc.scalar]
        for co in range(2, C // P):
            eng = qs[co % 2]
            eng.dma_start(wq_sb[:, co, :], wqr[:, co, :])
            nc.gpsimd.dma_start(wkv_sb[:, co, :], wkvr[:, co, :])

        ident = singles.tile([P, P], BF16)
        make_identity(nc, ident)
        ones_col = singles.tile([P, 1], BF16)
        nc.vector.memset(ones_col, 1.0)
        zero_col = singles.tile([P, 1], F32)
        nc.vector.memset(zero_col, 0.0)
        eps_col = singles.tile([P, 1], F32)
        nc.vector.memset(eps_col, EPS)
        nc.const_aps.aps[(F32, 0.0)] = zero_col[:]
        nc.const_aps.aps[(F32, EPS)] = eps_col[:]
        # scratch for the dummy exp that prewarms the exp act-table at the
        # A->B phase boundary (overlaps the 1.28us table load)
        warm = singles.tile([1, 1], F32)

        # diagonal-block mask: keep where i >= j (j = tk partition, i = tq
        # free).  bf16 so it can be ADDED into the scores psum by a 128-col
        # matmul (ident.T @ mask) instead of a DVE op in the exp chain.
        mask_sb = singles.tile([P, P], BF16)
        nc.vector.memset(mask_sb, 0.0)
        nc.gpsimd.affine_select(
            out=mask_sb, in_=mask_sb,
            compare_op=mybir.AluOpType.is_ge, fill=NEG,
            base=0, pattern=[[1, P]], channel_multiplier=-1,
        )

        qT = singles.tile([P, HG, T], BF16)      # [d, h, tok]
        kT = singles.tile([P, KVG, T], BF16)
        v_sb = singles.tile([P, TOKCH, KC], BF16)  # [tok%128, chunk, vcol]
        yT = singles.tile([P, HG, T], BF16)

        # ================= phase A: QKV proj + RoPE + qk-norm =============
        if "A" not in PHASES:
            pass
        else:
         with phase_a_pools, \
             tc.tile_pool(name="pa", bufs=2, space="PSUM") as pps, \
             tc.tile_pool(name="pkv", bufs=1, space="PSUM") as pkv, \
             tc.tile_pool(name="sa", bufs=3) as spool:
            # The V projection is split out of the QK pass and lagged by VLAG
            # chunks: the final VLAG V-chunks are pure PE work that runs while
            # the last rope chains (DVE) drain, so phase B starts without
            # waiting on the phase-A tail.
            VLAG = 6
            nco = C // P
            xtiles = {}

            def v_chunk(tv):
                xv = xtiles.pop(tv)
                ps_v = pkv.tile([P, KC], F32, tag="v")
                for co in range(nco):
                    nc.tensor.matmul(ps_v, xv[:, co, :],
                                     wkv_sb[:, co, KC:2 * KC],
                                     start=(co == 0), stop=(co == nco - 1))
                # cast straight to resident token-major buffer (Act engine;
                # DVE is the critical engine in this phase)
                nc.scalar.copy(v_sb[:, tv, :], ps_v)

            for t in range(TOKCH):
                if t == 0:
                    xtile = xtile0
                else:
                    xtile = xpool.tile([P, C // P, P], BF16, tag="xt")
                    nc.sync.dma_start(xtile, xt[t])
                xtiles[t] = xtile
                ps_q0 = pps.tile([P, 512], F32, tag="q0")
                ps_q1 = pps.tile([P, 512], F32, tag="q1")
                ps_k = pkv.tile([P, KC], F32, tag="k")
                for co in range(nco):
                    lhsT = xtile[:, co, :]
                    st = dict(start=(co == 0), stop=(co == nco - 1))
                    nc.tensor.matmul(ps_q0, lhsT, wq_sb[:, co, 0:512], **st)
                    nc.tensor.matmul(ps_q1, lhsT, wq_sb[:, co, 512:1024], **st)
                    nc.tensor.matmul(ps_k, lhsT, wkv_sb[:, co, 0:KC], **st)
                if t >= VLAG:
                    v_chunk(t - VLAG)

                # Q/K: fused multi-head rope + rms-norm + cast + transpose
                def rope_norm(ps, nh, dstT, h0, qscale):
                    h2 = D // 2
                    v4 = ps.rearrange("p (h a d) -> p h a d", h=nh, a=2)
                    q1, q2 = v4[:, :, 0, :], v4[:, :, 1, :]
                    r = spool.tile([P, nh, 2, h2], F32, tag=f"rope{nh}")
                    r1, r2 = r[:, :, 0, :], r[:, :, 1, :]
                    s2 = spool.tile([P, nh, h2], F32, tag=f"scr{nh}")
                    cs = cos_sb[:, t, None, :].to_broadcast([P, nh, h2])
                    sn = sin_sb[:, t, None, :].to_broadcast([P, nh, h2])
                    nc.vector.tensor_mul(r1, q1, cs)
                    nc.vector.tensor_mul(s2, q2, sn)
                    nc.vector.tensor_sub(r1, r1, s2)
                    nc.vector.tensor_mul(r2, q1, sn)
                    nc.vector.tensor_mul(s2, q2, cs)
                    nc.vector.tensor_add(r2, r2, s2)
                    rf = r.rearrange("p h a d -> p h (a d)")
                    sq = spool.tile([P, nh, D], F32, tag=f"sq{nh}")
                    nc.scalar.activation(sq, rf, AF.Square)
                    ss = spool.tile([P, nh], F32, tag=f"ss{nh}")
                    nc.vector.tensor_reduce(ss, sq, axis=mybir.AxisListType.X,
                                            op=mybir.AluOpType.add)
                    rq = spool.tile([P, nh], F32, tag=f"rq{nh}")
                    rt = spool.tile([P, nh], F32, tag=f"rt{nh}")
                    nc.scalar.activation(rt, ss, AF.Sqrt, scale=1.0 / D,
                                         bias=EPS)
                    nc.vector.reciprocal(rq, rt)
                    if qscale != 1.0:
                        nc.vector.tensor_scalar_mul(rq, rq, qscale)
                    qbf = spool.tile([P, nh, D], BF16, tag=f"qbf{nh}")
                    nc.vector.tensor_mul(qbf, rf, rq[:, :, None].to_broadcast([P, nh, D]))
                    pst = pps.tile([P, 4, P], BF16, tag="tr")
                    for i in range(nh):
                        nc.tensor.transpose(pst[:, i, :], qbf[:, i, :], ident)
                    # one strided copy: psum [128, nh*128] -> nh head slices of
                    # dstT (Act engine; DVE is the critical engine here)
                    nc.scalar.copy(
                        dstT[:, h0:h0 + nh, t * P:(t + 1) * P], pst[:, 0:nh, :])

                qsc = 1.0 / float(np.sqrt(D))
                rope_norm(ps_q0, 4, qT, 0, qsc)
                rope_norm(ps_q1, 4, qT, 4, qsc)
                rope_norm(ps_k, KVG, kT, 0, 1.0)

            # prewarm the exp act-table (the 1.28us load runs behind the
            # trailing V chunks), then the lagged V tail: pure PE work that
            # covers the final rope chains
            nc.scalar.activation(warm, zero_col[0:1, :], AF.Exp)
            for tv in range(TOKCH - VLAG, TOKCH):
                v_chunk(tv)

        # ================= phase B: attention ============================
        # wp prefetch: issue at phase-B start so the tiles are resident long
        # before phase C begins (phase-A pools have closed, SBUF is free)
        wpool = ctx.enter_context(tc.tile_pool(name="wp", bufs=1))
        wpr = wp.rearrange("(hc p) c -> p hc c", p=P)
        wp_ts = []
        for ct in range(C // 512):
            wp_t = wpool.tile([P, HG, 512], BF16, tag=f"wpt{ct}")
            nc.sync.dma_start(wp_t, wpr[:, :, ct * 512:(ct + 1) * 512])
            wp_ts.append(wp_t)

        if "B" not in PHASES:
            pass
        else:
         with tc.tile_pool(name="psc", bufs=4, space="PSUM") as psc, \
             tc.tile_pool(name="psy", bufs=2, space="PSUM") as psy, \
             tc.tile_pool(name="pss", bufs=2, space="PSUM") as pss, \
             tc.tile_pool(name="pb", bufs=6) as ppool, \
             tc.tile_pool(name="sb", bufs=3) as bpool:
            NT = T // 512  # 4 tq tiles
            # software pipeline: the PE queue is in-order, so scores for
            # chunk idx+DEPTH are emitted before pv/ones of chunk idx; the
            # scores->mask->exp chain (~1.6us) hides behind DEPTH chunks of
            # PE work.  The (h, c) stream is flattened so the pipeline also
            # covers head boundaries.
            DEPTH = 4
            for t in range(NT):
                nch = 4 * (t + 1)
                items = [(h, c) for h in range(HG) for c in range(nch)]
                live = {}

                def front(idx):
                    h, c = items[idx]
                    g = h // NREP
                    o = c * P - t * 512
                    col0 = max(o, 0)
                    ps_sc = psc.tile([P, 512], F32, tag="sc")
                    nc.tensor.matmul(
                        ps_sc[:, col0:512], kT[:, g, c * P:(c + 1) * P],
                        qT[:, h, t * 512 + col0:(t + 1) * 512],
                        start=True, stop=(o < 0))
                    if o >= 0:
                        # after the col0 shift the partial block is always the
                        # i' >= j triangle; accumulate the additive mask with
                        # a 128-col matmul (53ns) right behind the scores
                        nc.tensor.matmul(ps_sc[:, col0:col0 + P], ident,
                                         mask_sb, start=False, stop=True)
                    pt = ppool.tile([P, 512], BF16, tag="pt")
                    nc.scalar.activation(pt[:, col0:512], ps_sc[:, col0:512],
                                         AF.Exp)
                    live[idx] = (pt, col0)

                for i in range(min(DEPTH, len(items))):
                    front(i)
                ys = {}
                for idx, (h, c) in enumerate(items):
                    if idx + DEPTH < len(items):
                        front(idx + DEPTH)
                    g = h // NREP
                    if c == 0:
                        ps_y = psy.tile([P, 512], F32, tag="y")
                        ps_sden = pss.tile([P, 512], F32, tag="sden")
                        ys[h] = (ps_y, ps_sden)
                    ps_y, ps_sden = ys[h]
                    ps_s = ps_sden[0:1, :]
                    pt, col0 = live.pop(idx)
                    st = dict(start=(c == 0), stop=(c == nch - 1))
                    nc.tensor.matmul(ps_y[:, col0:512],
                                     v_sb[:, c, g * P:(g + 1) * P],
                                     pt[:, col0:512], **st)
                    nc.tensor.matmul(ps_s[:, col0:512], ones_col,
                                     pt[:, col0:512], **st)
                    if c == nch - 1:
                        # recip first (frees the single pss buffer fastest),
                        # then copy (frees ps_y); normalize the bf16 slice in
                        # place on the Pool engine once the broadcast lands
                        yslice = yT[:, h, t * 512:(t + 1) * 512]
                        rc = bpool.tile([1, 512], F32, tag="rc")
                        nc.vector.reciprocal(rc, ps_s)
                        nc.vector.tensor_copy(yslice, ps_y)
                        rb = bpool.tile([P, 512], F32, tag="rb")
                        nc.gpsimd.partition_broadcast(rb, rc, channels=P)
                        nc.vector.tensor_mul(yslice, yslice, rb)

        if DEBUG_DUMP:
            with tc.tile_pool(name="dbg", bufs=2) as dpool:
                for h in range(HG):
                    dt_ = dpool.tile([P, T], F32, tag="d")
                    nc.vector.tensor_copy(dt_, qT[:, h, :])
                    nc.sync.dma_start(d_qt[:, h, :], dt_)
                    dt_ = dpool.tile([P, T], F32, tag="d")
                    nc.vector.tensor_copy(dt_, yT[:, h, :])
                    nc.sync.dma_start(d_yt[:, h, :], dt_)
                for g in range(KVG):
                    dt_ = dpool.tile([P, T], F32, tag="d")
                    nc.vector.tensor_copy(dt_, kT[:, g, :])
                    nc.sync.dma_start(d_kt[:, g, :], dt_)
                dt_ = dpool.tile([P, TOKCH * KC], F32, tag="d")
                nc.vector.tensor_copy(dt_.rearrange("p (a b) -> p a b", a=TOKCH), v_sb[:, :, :])
                nc.sync.dma_start(d_v[:, :, :], dt_.rearrange("p (a b) -> p a b", a=TOKCH))

        # ================= phase C: output projection =====================
        if "C" not in PHASES:
            pass
        else:
         with tc.tile_pool(name="po", bufs=2, space="PSUM") as pso, \
             tc.tile_pool(name="so", bufs=3) as opool:
            for t in range(TOKCH):
                for ct in range(C // 512):
                    ps_o = pso.tile([P, 512], F32, tag="o")
                    for hc in range(HG):
                        nc.tensor.matmul(
                            ps_o, yT[:, hc, t * P:(t + 1) * P], wp_ts[ct][:, hc, :],
                            start=(hc == 0), stop=(hc == HG - 1))
                    ob = opool.tile([P, 512], F32, tag="ob")
                    nc.vector.tensor_copy(ob, ps_o)
                    nc.sync.dma_start(
                        out[t * P:(t + 1) * P, ct * 512:(ct + 1) * 512], ob)
    nc.compile()
    return nc


_NC_CACHE = []


def _get_prog():
    if not _NC_CACHE:
        _NC_CACHE.append(_build())
    return _NC_CACHE[0]


def _make_in_maps(inputs):
    x, cos, sin = inputs["x"], inputs["cos"], inputs["sin"]
    wq, wk, wv, wproj = inputs["wq"], inputs["wk"], inputs["wv"], inputs["wproj"]
    bf = ml_dtypes.bfloat16
    # [p, tc, d] tiling (contiguous 4KB DMA rows)
    cos2 = np.ascontiguousarray(
        cos.reshape(TOKCH, P, D // 2).transpose(1, 0, 2), dtype=np.float32)
    sin2 = np.ascontiguousarray(
        sin.reshape(TOKCH, P, D // 2).transpose(1, 0, 2), dtype=np.float32)
    in_maps = []
    for core in range(8):
        b, g = core // 2, core % 2
        qs = slice(g * QC, (g + 1) * QC)
        ks = slice(g * KC, (g + 1) * KC)
        # x[b].T is [C, T]; tile to [tokch, p(C-chunk), co, tk]
        xtb = (x[b].T.astype(bf)
               .reshape(C // P, P, TOKCH, P)     # [co, p, tc, tk]
               .transpose(2, 1, 0, 3))           # [tc, p, co, tk]
        in_maps.append({
            "xt": np.ascontiguousarray(xtb),
            "wq": np.ascontiguousarray(wq[:, qs]).astype(bf),
            "wkv": np.ascontiguousarray(np.hstack([wk[:, ks], wv[:, ks]])).astype(bf),
            "wp": np.ascontiguousarray(wproj[qs, :]).astype(bf),
            "cosd": cos2,
            "sind": sin2,
        })
    return in_maps


def kernel(x, cos, sin, wq, wk, wv, wproj):
    nc = _get_prog()
    in_maps = _make_in_maps(dict(x=x, cos=cos, sin=sin, wq=wq, wk=wk, wv=wv, wproj=wproj))
    res = run_bass_kernel_spmd(nc, in_maps, core_ids=list(range(8))).results
    outp = np.empty((B, T, C), np.float32)
    for b in range(B):
        outp[b] = res[2 * b]["out"] + res[2 * b + 1]["out"]
    return outp



# revision 72
# speedup vs baseline: 1.2012x; 1.0141x over previous
"""Causal self-attention (GQA + RoPE + QK-norm) Trainium2 Bass kernel.

Sharding: 8 cores = 4 batches x 2 head-groups.  Core c -> batch c//2,
q heads (c%2)*8..+8, kv heads (c%2)*2..+2.  wproj is row-sharded, so each
core emits a partial (T, C) output; the host sums the two partials per batch.

Device-side layout strategy (per core):
  - x is fed pre-transposed (xT, [C, T]) and bf16-cast by the host.
  - QKV projections produce Q,K token-major ([tok, cols]); RoPE + rms-norm
    run token-major (free-axis per-head reductions), then 128x128 PE
    transposes produce qT/kT feature-major for the attention matmuls.
    V is produced token-major, which is exactly the p@v stationary layout.
  - scores are computed transposed (scoresT[tk, tq]) so that after exp the
    p tiles are already the moving operand for the p@v matmul; the softmax
    denominator comes from a ones-column matmul accumulated in PSUM.
  - exp has no max-subtraction: qk-norm bounds |s| <= sqrt(128) ~ 11.32.
  - output projection accumulates over the 8 local heads; partial written
    fp32 to DRAM.
"""

import numpy as np
import ml_dtypes
from contextlib import ExitStack

import concourse.bass as bass
import concourse.mybir as mybir
import concourse.tile as tile
from concourse import bacc
from concourse.bass_utils import run_bass_kernel_spmd
from concourse.masks import make_identity

BF16 = mybir.dt.bfloat16
F32 = mybir.dt.float32
F32R = mybir.dt.float32r
AF = mybir.ActivationFunctionType

B, T, C = 4, 2048, 2048
H, KV, D = 16, 4, 128
HG, KVG = H // 2, KV // 2          # per-core q heads (8), kv heads (2)
QC, KC = HG * D, KVG * D           # 1024, 256
P = 128
TOKCH = T // P                     # 16 token chunks
NREP = H // KV                     # 4
EPS = 1e-5
NEG = -1.0e5                       # additive causal mask (exp -> 0)


DEBUG_DUMP = False
PHASES = ("A", "B", "C")


def _build():
    nc = bacc.Bacc("TRN2", target_bir_lowering=False, debug=False, num_devices=8)
    # x pre-tiled by the host as [tokch, p, co, tk] so every DMA partition row
    # is 4KB contiguous (co*tk*2B) instead of 256B strided
    xt = nc.dram_tensor("xt", [TOKCH, P, C // P, P], BF16, kind="ExternalInput")
    wq = nc.dram_tensor("wq", [C, QC], BF16, kind="ExternalInput")
    wkv = nc.dram_tensor("wkv", [C, 2 * KC], BF16, kind="ExternalInput")
    wp = nc.dram_tensor("wp", [QC, C], BF16, kind="ExternalInput")
    # cos/sin pre-tiled by host as [p, tc, d] (contiguous 4KB rows)
    cosd = nc.dram_tensor("cosd", [P, TOKCH, D // 2], F32, kind="ExternalInput")
    sind = nc.dram_tensor("sind", [P, TOKCH, D // 2], F32, kind="ExternalInput")
    out = nc.dram_tensor("out", [T, C], F32, kind="ExternalOutput")
    if DEBUG_DUMP:
        d_qt = nc.dram_tensor("d_qt", [P, HG, T], F32, kind="ExternalOutput")
        d_kt = nc.dram_tensor("d_kt", [P, KVG, T], F32, kind="ExternalOutput")
        d_v = nc.dram_tensor("d_v", [P, TOKCH, KC], F32, kind="ExternalOutput")
        d_yt = nc.dram_tensor("d_yt", [P, HG, T], F32, kind="ExternalOutput")

    with tile.TileContext(nc) as tc, ExitStack() as ctx:
        singles = ctx.enter_context(tc.tile_pool(name="singles", bufs=1))
        # bufs must cover the V-lag window (xtile(t) is re-read by the lagged
        # V projection at iteration t+VLAG); the pool closes with phase A
        phase_a_pools = ExitStack()
        xpool = phase_a_pools.enter_context(tc.tile_pool(name="xa", bufs=8))

        # ---- prefetch the first x tile before the weight bulk so the PE
        # can start within a few us ----
        # ---- resident tensors ----
        # weight DMAs issued per-co round-robin over both HWDGE queues so
        # early co chunks land in consumption order and issue rate isn't
        # limited by one sequencer (~600ns per dma_start).  The first x
        # chunk + first co weights go out first so the PE starts ASAP.
        wq_sb = singles.tile([P, C // P, QC], BF16)
        wkv_sb = singles.tile([P, C // P, 2 * KC], BF16)
        wqr = wq.rearrange("(co p) q -> p co q", p=P)
        wkvr = wkv.rearrange("(co p) q -> p co q", p=P)
        cos_sb = singles.tile([P, TOKCH, D // 2], F32)
        sin_sb = singles.tile([P, TOKCH, D // 2], F32)
        xtile0 = xpool.tile([P, C // P, P], BF16, tag="xt")
        nc.sync.dma_start(xtile0[:, 0:4, :], xt[0, :, 0:4, :])
        nc.scalar.dma_start(wq_sb[:, 0, :], wqr[:, 0, :])
        nc.sync.dma_start(wkv_sb[:, 0, :], wkvr[:, 0, :])
        nc.scalar.dma_start(wq_sb[:, 1, :], wqr[:, 1, :])
        for g4 in range(1, 4):
            nc.sync.dma_start(xtile0[:, 4 * g4:4 * (g4 + 1), :],
                              xt[0, :, 4 * g4:4 * (g4 + 1), :])
        nc.gpsimd.dma_start(wkv_sb[:, 1, :], wkvr[:, 1, :])
        nc.scalar.dma_start(cos_sb, cosd[:])
        nc.sync.dma_start(sin_sb, sind[:])
        qs = [nc.sync, nc.scalar]
        for co in range(2, C // P):
            eng = qs[co % 2]
            eng.dma_start(wq_sb[:, co, :], wqr[:, co, :])
            nc.gpsimd.dma_start(wkv_sb[:, co, :], wkvr[:, co, :])

        ident = singles.tile([P, P], BF16)
        make_identity(nc, ident)
        ones_col = singles.tile([P, 1], BF16)
        nc.vector.memset(ones_col, 1.0)
        zero_col = singles.tile([P, 1], F32)
        nc.vector.memset(zero_col, 0.0)
        eps_col = singles.tile([P, 1], F32)
        nc.vector.memset(eps_col, EPS)
        nc.const_aps.aps[(F32, 0.0)] = zero_col[:]
        nc.const_aps.aps[(F32, EPS)] = eps_col[:]
        # scratch for the dummy exp that prewarms the exp act-table at the
        # A->B phase boundary (overlaps the 1.28us table load)
        warm = singles.tile([1, 1], F32)

        # diagonal-block mask: keep where i >= j (j = tk partition, i = tq
        # free).  bf16 so it can be ADDED into the scores psum by a 128-col
        # matmul (ident.T @ mask) instead of a DVE op in the exp chain.
        mask_sb = singles.tile([P, P], BF16)
        nc.vector.memset(mask_sb, 0.0)
        nc.gpsimd.affine_select(
            out=mask_sb, in_=mask_sb,
            compare_op=mybir.AluOpType.is_ge, fill=NEG,
            base=0, pattern=[[1, P]], channel_multiplier=-1,
        )

        qT = singles.tile([P, HG, T], BF16)      # [d, h, tok]
        kT = singles.tile([P, KVG, T], BF16)
        v_sb = singles.tile([P, TOKCH, KC], BF16)  # [tok%128, chunk, vcol]
        yT = singles.tile([P, HG, T], BF16)

        # ================= phase A: QKV proj + RoPE + qk-norm =============
        if "A" not in PHASES:
            pass
        else:
         with phase_a_pools, \
             tc.tile_pool(name="pa", bufs=2, space="PSUM") as pps, \
             tc.tile_pool(name="pkv", bufs=1, space="PSUM") as pkv, \
             tc.tile_pool(name="sa", bufs=3) as spool:
            # The V projection is split out of the QK pass and lagged by VLAG
            # chunks: the final VLAG V-chunks are pure PE work that runs while
            # the last rope chains (DVE) drain, so phase B starts without
            # waiting on the phase-A tail.
            VLAG = 6
            nco = C // P
            xtiles = {}
            # transposes lag one iteration behind their rope chain so they
            # never sit dep-blocked in the PE's 4-deep wait queue
            pending_tr = []

            def flush_trs():
                while pending_tr:
                    qbf, dstT, h0, nh, tt = pending_tr.pop(0)
                    pst = pkv.tile([P, 4, P], BF16, tag="tr")
                    for i in range(nh):
                        nc.tensor.transpose(pst[:, i, :], qbf[:, i, :], ident)
                    nc.scalar.copy(
                        dstT[:, h0:h0 + nh, tt * P:(tt + 1) * P], pst[:, 0:nh, :])

            def v_chunk(tv):
                xv = xtiles.pop(tv)
                ps_v = pkv.tile([P, KC], F32, tag="v")
                for co in range(nco):
                    nc.tensor.matmul(ps_v, xv[:, co, :],
                                     wkv_sb[:, co, KC:2 * KC],
                                     start=(co == 0), stop=(co == nco - 1))
                # cast straight to resident token-major buffer (Act engine;
                # DVE is the critical engine in this phase)
                nc.scalar.copy(v_sb[:, tv, :], ps_v)

            for t in range(TOKCH):
                if t == 0:
                    xtile = xtile0
                else:
                    xtile = xpool.tile([P, C // P, P], BF16, tag="xt")
                    nc.sync.dma_start(xtile, xt[t])
                xtiles[t] = xtile
                ps_q0 = pps.tile([P, 512], F32, tag="q0")
                ps_q1 = pps.tile([P, 512], F32, tag="q1")
                ps_k = pkv.tile([P, KC], F32, tag="k")
                for co in range(nco):
                    lhsT = xtile[:, co, :]
                    st = dict(start=(co == 0), stop=(co == nco - 1))
                    nc.tensor.matmul(ps_q0, lhsT, wq_sb[:, co, 0:512], **st)
                    nc.tensor.matmul(ps_q1, lhsT, wq_sb[:, co, 512:1024], **st)
                    nc.tensor.matmul(ps_k, lhsT, wkv_sb[:, co, 0:KC], **st)
                if t >= VLAG:
                    v_chunk(t - VLAG)
                # previous iteration's transposes: rope chains long done
                flush_trs()

                # Q/K: fused multi-head rope + rms-norm + cast + transpose
                def rope_norm(ps, nh, dstT, h0, qscale):
                    h2 = D // 2
                    v4 = ps.rearrange("p (h a d) -> p h a d", h=nh, a=2)
                    q1, q2 = v4[:, :, 0, :], v4[:, :, 1, :]
                    r = spool.tile([P, nh, 2, h2], F32, tag=f"rope{nh}")
                    r1, r2 = r[:, :, 0, :], r[:, :, 1, :]
                    s2 = spool.tile([P, nh, h2], F32, tag=f"scr{nh}")
                    cs = cos_sb[:, t, None, :].to_broadcast([P, nh, h2])
                    sn = sin_sb[:, t, None, :].to_broadcast([P, nh, h2])
                    nc.vector.tensor_mul(r1, q1, cs)
                    nc.vector.tensor_mul(s2, q2, sn)
                    nc.vector.tensor_sub(r1, r1, s2)
                    nc.vector.tensor_mul(r2, q1, sn)
                    nc.vector.tensor_mul(s2, q2, cs)
                    nc.vector.tensor_add(r2, r2, s2)
                    rf = r.rearrange("p h a d -> p h (a d)")
                    sq = spool.tile([P, nh, D], F32, tag=f"sq{nh}")
                    nc.scalar.activation(sq, rf, AF.Square)
                    ss = spool.tile([P, nh], F32, tag=f"ss{nh}")
                    nc.vector.tensor_reduce(ss, sq, axis=mybir.AxisListType.X,
                                            op=mybir.AluOpType.add)
                    rq = spool.tile([P, nh], F32, tag=f"rq{nh}")
                    rt = spool.tile([P, nh], F32, tag=f"rt{nh}")
                    nc.scalar.activation(rt, ss, AF.Sqrt, scale=1.0 / D,
                                         bias=EPS)
                    nc.vector.reciprocal(rq, rt)
                    if qscale != 1.0:
                        nc.vector.tensor_scalar_mul(rq, rq, qscale)
                    qbf = spool.tile([P, nh, D], BF16, tag=f"qbf{nh}")
                    nc.vector.tensor_mul(qbf, rf, rq[:, :, None].to_broadcast([P, nh, D]))
                    pending_tr.append((qbf, dstT, h0, nh, t))

                qsc = 1.0 / float(np.sqrt(D))
                rope_norm(ps_q0, 4, qT, 0, qsc)
                rope_norm(ps_q1, 4, qT, 4, qsc)
                rope_norm(ps_k, KVG, kT, 0, 1.0)

            # prewarm the exp act-table (the 1.28us load runs behind the
            # trailing V chunks), then the lagged V tail: pure PE work that
            # covers the final rope chains; the last transposes flush once
            # their rope chain has had ~7us of V cover
            nc.scalar.activation(warm, zero_col[0:1, :], AF.Exp)
            for tv in range(TOKCH - VLAG, TOKCH):
                v_chunk(tv)
                if tv == TOKCH - 2:
                    flush_trs()

        # ================= phase B: attention ============================
        # wp prefetch: issue at phase-B start so the tiles are resident long
        # before phase C begins (phase-A pools have closed, SBUF is free)
        wpool = ctx.enter_context(tc.tile_pool(name="wp", bufs=1))
        wpr = wp.rearrange("(hc p) c -> p hc c", p=P)
        wp_ts = []
        for ct in range(C // 512):
            wp_t = wpool.tile([P, HG, 512], BF16, tag=f"wpt{ct}")
            nc.sync.dma_start(wp_t, wpr[:, :, ct * 512:(ct + 1) * 512])
            wp_ts.append(wp_t)

        if "B" not in PHASES:
            pass
        else:
         with tc.tile_pool(name="psc", bufs=4, space="PSUM") as psc, \
             tc.tile_pool(name="psy", bufs=2, space="PSUM") as psy, \
             tc.tile_pool(name="pss", bufs=2, space="PSUM") as pss, \
             tc.tile_pool(name="pb", bufs=6) as ppool, \
             tc.tile_pool(name="sb", bufs=3) as bpool:
            NT = T // 512  # 4 tq tiles
            # software pipeline: the PE queue is in-order, so scores for
            # chunk idx+DEPTH are emitted before pv/ones of chunk idx; the
            # scores->mask->exp chain (~1.6us) hides behind DEPTH chunks of
            # PE work.  The (h, c) stream is flattened so the pipeline also
            # covers head boundaries.
            DEPTH = 3
            for t in range(NT):
                nch = 4 * (t + 1)
                items = [(h, c) for h in range(HG) for c in range(nch)]
                live = {}

                def front(idx):
                    h, c = items[idx]
                    g = h // NREP
                    o = c * P - t * 512
                    col0 = max(o, 0)
                    ps_sc = psc.tile([P, 512], F32, tag="sc")
                    nc.tensor.matmul(
                        ps_sc[:, col0:512], kT[:, g, c * P:(c + 1) * P],
                        qT[:, h, t * 512 + col0:(t + 1) * 512],
                        start=True, stop=(o < 0))
                    if o >= 0:
                        # after the col0 shift the partial block is always the
                        # i' >= j triangle; accumulate the additive mask with
                        # a 128-col matmul (53ns) right behind the scores
                        nc.tensor.matmul(ps_sc[:, col0:col0 + P], ident,
                                         mask_sb, start=False, stop=True)
                    pt = ppool.tile([P, 512], BF16, tag="pt")
                    nc.scalar.activation(pt[:, col0:512], ps_sc[:, col0:512],
                                         AF.Exp)
                    live[idx] = (pt, col0)

                for i in range(min(DEPTH, len(items))):
                    front(i)
                ys = {}
                for idx, (h, c) in enumerate(items):
                    if idx + DEPTH < len(items):
                        front(idx + DEPTH)
                    g = h // NREP
                    if c == 0:
                        ps_y = psy.tile([P, 512], F32, tag="y")
                        ps_sden = pss.tile([P, 512], F32, tag="sden")
                        ys[h] = (ps_y, ps_sden)
                    ps_y, ps_sden = ys[h]
                    ps_s = ps_sden[0:1, :]
                    pt, col0 = live.pop(idx)
                    st = dict(start=(c == 0), stop=(c == nch - 1))
                    nc.tensor.matmul(ps_y[:, col0:512],
                                     v_sb[:, c, g * P:(g + 1) * P],
                                     pt[:, col0:512], **st)
                    nc.tensor.matmul(ps_s[:, col0:512], ones_col,
                                     pt[:, col0:512], **st)
                    if c == nch - 1:
                        # recip first (frees the single pss buffer fastest),
                        # then copy (frees ps_y); normalize the bf16 slice in
                        # place on the Pool engine once the broadcast lands
                        yslice = yT[:, h, t * 512:(t + 1) * 512]
                        rc = bpool.tile([1, 512], F32, tag="rc")
                        nc.vector.reciprocal(rc, ps_s)
                        nc.vector.tensor_copy(yslice, ps_y)
                        rb = bpool.tile([P, 512], F32, tag="rb")
                        nc.gpsimd.partition_broadcast(rb, rc, channels=P)
                        nc.vector.tensor_mul(yslice, yslice, rb)

        if DEBUG_DUMP:
            with tc.tile_pool(name="dbg", bufs=2) as dpool:
                for h in range(HG):
                    dt_ = dpool.tile([P, T], F32, tag="d")
                    nc.vector.tensor_copy(dt_, qT[:, h, :])
                    nc.sync.dma_start(d_qt[:, h, :], dt_)
                    dt_ = dpool.tile([P, T], F32, tag="d")
                    nc.vector.tensor_copy(dt_, yT[:, h, :])
                    nc.sync.dma_start(d_yt[:, h, :], dt_)
                for g in range(KVG):
                    dt_ = dpool.tile([P, T], F32, tag="d")
                    nc.vector.tensor_copy(dt_, kT[:, g, :])
                    nc.sync.dma_start(d_kt[:, g, :], dt_)
                dt_ = dpool.tile([P, TOKCH * KC], F32, tag="d")
                nc.vector.tensor_copy(dt_.rearrange("p (a b) -> p a b", a=TOKCH), v_sb[:, :, :])
                nc.sync.dma_start(d_v[:, :, :], dt_.rearrange("p (a b) -> p a b", a=TOKCH))

        # ================= phase C: output projection =====================
        if "C" not in PHASES:
            pass
        else:
         with tc.tile_pool(name="po", bufs=2, space="PSUM") as pso, \
             tc.tile_pool(name="so", bufs=3) as opool:
            for t in range(TOKCH):
                for ct in range(C // 512):
                    ps_o = pso.tile([P, 512], F32, tag="o")
                    last = (t == TOKCH - 1 and ct == C // 512 - 1)
                    ob = opool.tile([P, 512], F32, tag="ob")
                    if not last:
                        for hc in range(HG):
                            nc.tensor.matmul(
                                ps_o, yT[:, hc, t * P:(t + 1) * P],
                                wp_ts[ct][:, hc, :],
                                start=(hc == 0), stop=(hc == HG - 1))
                        nc.vector.tensor_copy(ob, ps_o)
                        nc.sync.dma_start(
                            out[t * P:(t + 1) * P, ct * 512:(ct + 1) * 512], ob)
                    else:
                        # final tile in two pipelined halves to shorten the
                        # copy->dma drain tail
                        for q in range(2):
                            cs = slice(q * 256, (q + 1) * 256)
                            for hc in range(HG):
                                nc.tensor.matmul(
                                    ps_o[:, cs], yT[:, hc, t * P:(t + 1) * P],
                                    wp_ts[ct][:, hc, cs],
                                    start=(hc == 0), stop=(hc == HG - 1))
                            nc.vector.tensor_copy(ob[:, cs], ps_o[:, cs])
                            nc.sync.dma_start(
                                out[t * P:(t + 1) * P,
                                    ct * 512 + q * 256:ct * 512 + (q + 1) * 256],
                                ob[:, cs])
    nc.compile()
    return nc


_NC_CACHE = []


def _get_prog():
    if not _NC_CACHE:
        _NC_CACHE.append(_build())
    return _NC_CACHE[0]


def _make_in_maps(inputs):
    x, cos, sin = inputs["x"], inputs["cos"], inputs["sin"]
    wq, wk, wv, wproj = inputs["wq"], inputs["wk"], inputs["wv"], inputs["wproj"]
    bf = ml_dtypes.bfloat16
    # [p, tc, d] tiling (contiguous 4KB DMA rows)
    cos2 = np.ascontiguousarray(
        cos.reshape(TOKCH, P, D // 2).transpose(1, 0, 2), dtype=np.float32)
    sin2 = np.ascontiguousarray(
        sin.reshape(TOKCH, P, D // 2).transpose(1, 0, 2), dtype=np.float32)
    in_maps = []
    for core in range(8):
        b, g = core // 2, core % 2
        qs = slice(g * QC, (g + 1) * QC)
        ks = slice(g * KC, (g + 1) * KC)
        # x[b].T is [C, T]; tile to [tokch, p(C-chunk), co, tk]
        xtb = (x[b].T.astype(bf)
               .reshape(C // P, P, TOKCH, P)     # [co, p, tc, tk]
               .transpose(2, 1, 0, 3))           # [tc, p, co, tk]
        in_maps.append({
            "xt": np.ascontiguousarray(xtb),
            "wq": np.ascontiguousarray(wq[:, qs]).astype(bf),
            "wkv": np.ascontiguousarray(np.hstack([wk[:, ks], wv[:, ks]])).astype(bf),
            "wp": np.ascontiguousarray(wproj[qs, :]).astype(bf),
            "cosd": cos2,
            "sind": sin2,
        })
    return in_maps


def kernel(x, cos, sin, wq, wk, wv, wproj):
    nc = _get_prog()
    in_maps = _make_in_maps(dict(x=x, cos=cos, sin=sin, wq=wq, wk=wk, wv=wv, wproj=wproj))
    res = run_bass_kernel_spmd(nc, in_maps, core_ids=list(range(8))).results
    outp = np.empty((B, T, C), np.float32)
    for b in range(B):
        outp[b] = res[2 * b]["out"] + res[2 * b + 1]["out"]
    return outp



# revision 88
# speedup vs baseline: 1.2020x; 1.0007x over previous
"""Causal self-attention (GQA + RoPE + QK-norm) Trainium2 Bass kernel.

Sharding: 8 cores = 4 batches x 2 head-groups.  Core c -> batch c//2,
q heads (c%2)*8..+8, kv heads (c%2)*2..+2.  wproj is row-sharded, so each
core emits a partial (T, C) output; the host sums the two partials per batch.

Device-side layout strategy (per core):
  - x is fed pre-transposed (xT, [C, T]) and bf16-cast by the host.
  - QKV projections produce Q,K token-major ([tok, cols]); RoPE + rms-norm
    run token-major (free-axis per-head reductions), then 128x128 PE
    transposes produce qT/kT feature-major for the attention matmuls.
    V is produced token-major, which is exactly the p@v stationary layout.
  - scores are computed transposed (scoresT[tk, tq]) so that after exp the
    p tiles are already the moving operand for the p@v matmul; the softmax
    denominator comes from a ones-column matmul accumulated in PSUM.
  - exp has no max-subtraction: qk-norm bounds |s| <= sqrt(128) ~ 11.32.
  - output projection accumulates over the 8 local heads; partial written
    fp32 to DRAM.
"""

import numpy as np
import ml_dtypes
from contextlib import ExitStack

import concourse.bass as bass
import concourse.mybir as mybir
import concourse.tile as tile
from concourse import bacc
from concourse.bass_utils import run_bass_kernel_spmd
from concourse.masks import make_identity

BF16 = mybir.dt.bfloat16
F32 = mybir.dt.float32
F32R = mybir.dt.float32r
AF = mybir.ActivationFunctionType

B, T, C = 4, 2048, 2048
H, KV, D = 16, 4, 128
HG, KVG = H // 2, KV // 2          # per-core q heads (8), kv heads (2)
QC, KC = HG * D, KVG * D           # 1024, 256
P = 128
TOKCH = T // P                     # 16 token chunks
NREP = H // KV                     # 4
EPS = 1e-5
NEG = -1.0e5                       # additive causal mask (exp -> 0)


DEBUG_DUMP = False
PHASES = ("A", "B", "C")


def _build():
    nc = bacc.Bacc("TRN2", target_bir_lowering=False, debug=False, num_devices=8)
    # x pre-tiled by the host as [tokch, p, co, tk] so every DMA partition row
    # is 4KB contiguous (co*tk*2B) instead of 256B strided
    xt = nc.dram_tensor("xt", [TOKCH, P, C // P, P], BF16, kind="ExternalInput")
    wq = nc.dram_tensor("wq", [C, QC], BF16, kind="ExternalInput")
    wkv = nc.dram_tensor("wkv", [C, 2 * KC], BF16, kind="ExternalInput")
    wp = nc.dram_tensor("wp", [QC, C], BF16, kind="ExternalInput")
    # cos/sin pre-tiled by host as [p, tc, d] (contiguous 4KB rows)
    cosd = nc.dram_tensor("cosd", [P, TOKCH, D // 2], F32, kind="ExternalInput")
    sind = nc.dram_tensor("sind", [P, TOKCH, D // 2], F32, kind="ExternalInput")
    out = nc.dram_tensor("out", [T, C], F32, kind="ExternalOutput")
    if DEBUG_DUMP:
        d_qt = nc.dram_tensor("d_qt", [P, HG, T], F32, kind="ExternalOutput")
        d_kt = nc.dram_tensor("d_kt", [P, KVG, T], F32, kind="ExternalOutput")
        d_v = nc.dram_tensor("d_v", [P, TOKCH, KC], F32, kind="ExternalOutput")
        d_yt = nc.dram_tensor("d_yt", [P, HG, T], F32, kind="ExternalOutput")

    with tile.TileContext(nc) as tc, ExitStack() as ctx:
        singles = ctx.enter_context(tc.tile_pool(name="singles", bufs=1))
        # bufs must cover the V-lag window (xtile(t) is re-read by the lagged
        # V projection at iteration t+VLAG); the pool closes with phase A
        phase_a_pools = ExitStack()
        xpool = phase_a_pools.enter_context(tc.tile_pool(name="xa", bufs=8))

        # ---- prefetch the first x tile before the weight bulk so the PE
        # can start within a few us ----
        # ---- resident tensors ----
        # weight DMAs issued per-co round-robin over both HWDGE queues so
        # early co chunks land in consumption order and issue rate isn't
        # limited by one sequencer (~600ns per dma_start).  The first x
        # chunk + first co weights go out first so the PE starts ASAP.
        wq_sb = singles.tile([P, C // P, QC], BF16)
        wkv_sb = singles.tile([P, C // P, 2 * KC], BF16)
        wqr = wq.rearrange("(co p) q -> p co q", p=P)
        wkvr = wkv.rearrange("(co p) q -> p co q", p=P)
        cos_sb = singles.tile([P, TOKCH, D // 2], F32)
        sin_sb = singles.tile([P, TOKCH, D // 2], F32)
        xtile0 = xpool.tile([P, C // P, P], BF16, tag="xt")
        nc.sync.dma_start(xtile0[:, 0:4, :], xt[0, :, 0:4, :])
        nc.scalar.dma_start(wq_sb[:, 0, :], wqr[:, 0, :])
        nc.sync.dma_start(wkv_sb[:, 0, :], wkvr[:, 0, :])
        nc.scalar.dma_start(wq_sb[:, 1, :], wqr[:, 1, :])
        for g4 in range(1, 4):
            nc.sync.dma_start(xtile0[:, 4 * g4:4 * (g4 + 1), :],
                              xt[0, :, 4 * g4:4 * (g4 + 1), :])
        nc.gpsimd.dma_start(wkv_sb[:, 1, :], wkvr[:, 1, :])
        nc.scalar.dma_start(cos_sb, cosd[:])
        nc.sync.dma_start(sin_sb, sind[:])
        # remaining weights in 2-co pieces: fewer issues (the sequencers'
        # ~600ns per dma_start is the startup bottleneck), still fine-grained
        # enough to land in consumption order
        qs = [nc.sync, nc.scalar]
        for i, co in enumerate(range(2, C // P, 2)):
            eng = qs[i % 2]
            eng.dma_start(wq_sb[:, co:co + 2, :], wqr[:, co:co + 2, :])
            nc.gpsimd.dma_start(wkv_sb[:, co:co + 2, :], wkvr[:, co:co + 2, :])

        ident = singles.tile([P, P], BF16)
        make_identity(nc, ident)
        ones_col = singles.tile([P, 1], BF16)
        nc.vector.memset(ones_col, 1.0)
        zero_col = singles.tile([P, 1], F32)
        nc.vector.memset(zero_col, 0.0)
        eps_col = singles.tile([P, 1], F32)
        nc.vector.memset(eps_col, EPS)
        nc.const_aps.aps[(F32, 0.0)] = zero_col[:]
        nc.const_aps.aps[(F32, EPS)] = eps_col[:]
        # scratch for the dummy exp that prewarms the exp act-table at the
        # A->B phase boundary (overlaps the 1.28us table load)
        warm = singles.tile([1, 1], F32)

        # diagonal-block mask: keep where i >= j (j = tk partition, i = tq
        # free).  bf16 so it can be ADDED into the scores psum by a 128-col
        # matmul (ident.T @ mask) instead of a DVE op in the exp chain.
        mask_sb = singles.tile([P, P], BF16)
        nc.vector.memset(mask_sb, 0.0)
        nc.gpsimd.affine_select(
            out=mask_sb, in_=mask_sb,
            compare_op=mybir.AluOpType.is_ge, fill=NEG,
            base=0, pattern=[[1, P]], channel_multiplier=-1,
        )

        qT = singles.tile([P, HG, T], BF16)      # [d, h, tok]
        kT = singles.tile([P, KVG, T], BF16)
        v_sb = singles.tile([P, TOKCH, KC], BF16)  # [tok%128, chunk, vcol]
        yT = singles.tile([P, HG, T], BF16)

        # ================= phase A: QKV proj + RoPE + qk-norm =============
        if "A" not in PHASES:
            pass
        else:
         with phase_a_pools, \
             tc.tile_pool(name="pa", bufs=2, space="PSUM") as pps, \
             tc.tile_pool(name="pkv", bufs=1, space="PSUM") as pkv, \
             tc.tile_pool(name="sa", bufs=3) as spool:
            # The V projection is split out of the QK pass and lagged by VLAG
            # chunks: the final VLAG V-chunks are pure PE work that runs while
            # the last rope chains (DVE) drain, so phase B starts without
            # waiting on the phase-A tail.
            VLAG = 6
            nco = C // P
            xtiles = {}
            # transposes lag one iteration behind their rope chain so they
            # never sit dep-blocked in the PE's 4-deep wait queue
            pending_tr = []

            def flush_trs():
                while pending_tr:
                    qbf, dstT, h0, nh, tt = pending_tr.pop(0)
                    pst = pkv.tile([P, 4, P], BF16, tag="tr")
                    for i in range(nh):
                        nc.tensor.transpose(pst[:, i, :], qbf[:, i, :], ident)
                    nc.scalar.copy(
                        dstT[:, h0:h0 + nh, tt * P:(tt + 1) * P], pst[:, 0:nh, :])

            def v_chunk(tv):
                xv = xtiles.pop(tv)
                # alternate psum tags so consecutive V chunks don't serialize
                # on one buffer's Act-copy release
                ps_v = pkv.tile([P, KC], F32, tag=("v" if tv % 2 == 0 else "v2"))
                for co in range(nco):
                    nc.tensor.matmul(ps_v, xv[:, co, :],
                                     wkv_sb[:, co, KC:2 * KC],
                                     start=(co == 0), stop=(co == nco - 1))
                # cast straight to resident token-major buffer (Act engine;
                # DVE is the critical engine in this phase)
                nc.scalar.copy(v_sb[:, tv, :], ps_v)

            for t in range(TOKCH):
                if t == 0:
                    xtile = xtile0
                else:
                    xtile = xpool.tile([P, C // P, P], BF16, tag="xt")
                    nc.sync.dma_start(xtile, xt[t])
                xtiles[t] = xtile
                ps_q0 = pps.tile([P, 512], F32, tag="q0")
                ps_q1 = pps.tile([P, 512], F32, tag="q1")
                ps_k = pkv.tile([P, KC], F32, tag="k")
                for co in range(nco):
                    lhsT = xtile[:, co, :]
                    st = dict(start=(co == 0), stop=(co == nco - 1))
                    nc.tensor.matmul(ps_q0, lhsT, wq_sb[:, co, 0:512], **st)
                    nc.tensor.matmul(ps_q1, lhsT, wq_sb[:, co, 512:1024], **st)
                    nc.tensor.matmul(ps_k, lhsT, wkv_sb[:, co, 0:KC], **st)
                if t >= VLAG:
                    v_chunk(t - VLAG)
                # previous iteration's transposes: rope chains long done
                flush_trs()

                # Q/K: fused multi-head rope + rms-norm + cast + transpose
                def rope_norm(ps, nh, dstT, h0, qscale, rsq_dve=False):
                    h2 = D // 2
                    v4 = ps.rearrange("p (h a d) -> p h a d", h=nh, a=2)
                    q1, q2 = v4[:, :, 0, :], v4[:, :, 1, :]
                    r = spool.tile([P, nh, 2, h2], F32, tag=f"rope{nh}")
                    r1, r2 = r[:, :, 0, :], r[:, :, 1, :]
                    s2 = spool.tile([P, nh, h2], F32, tag=f"scr{nh}")
                    cs = cos_sb[:, t, None, :].to_broadcast([P, nh, h2])
                    sn = sin_sb[:, t, None, :].to_broadcast([P, nh, h2])
                    nc.vector.tensor_mul(r1, q1, cs)
                    nc.vector.tensor_mul(s2, q2, sn)
                    nc.vector.tensor_sub(r1, r1, s2)
                    nc.vector.tensor_mul(r2, q1, sn)
                    nc.vector.tensor_mul(s2, q2, cs)
                    nc.vector.tensor_add(r2, r2, s2)
                    rf = r.rearrange("p h a d -> p h (a d)")
                    sq = spool.tile([P, nh, D], F32, tag=f"sq{nh}")
                    ss = spool.tile([P, nh], F32, tag=f"ss{nh}")
                    if rsq_dve:
                        # keep the last iteration's rope entirely off the Act
                        # engine so phase B's first exps aren't queued behind it
                        nc.vector.tensor_mul(sq, rf, rf)
                    else:
                        nc.scalar.activation(sq, rf, AF.Square)
                    nc.vector.tensor_reduce(ss, sq, axis=mybir.AxisListType.X,
                                            op=mybir.AluOpType.add)
                    rq = spool.tile([P, nh], F32, tag=f"rq{nh}")
                    if rsq_dve:
                        # DVE-only fast inverse sqrt (bit trick + 2 Newton
                        # steps, qscale folded into the last).  Used for the
                        # final token chunk so the previous iteration's Sqrt
                        # is the Act engine's last sqrt-set op and the exp
                        # table load hides behind the V tail.
                        ALU = mybir.AluOpType
                        I32 = mybir.dt.int32
                        fx = spool.tile([P, nh], F32, tag=f"fx{nh}")
                        nc.vector.tensor_scalar(fx, ss, 1.0 / D, EPS,
                                                op0=ALU.mult, op1=ALU.add)
                        fj = spool.tile([P, nh], I32, tag=f"fj{nh}")
                        nc.vector.tensor_scalar(fj, fx[:].bitcast(I32), 1, None,
                                                op0=ALU.logical_shift_right)
                        nc.vector.tensor_scalar(fj, fj, -1, 0x5f3759df + 1,
                                                op0=ALU.bitwise_xor, op1=ALU.add)
                        fy = fj[:].bitcast(F32)
                        fa = spool.tile([P, nh], F32, tag=f"fa{nh}")
                        nc.vector.tensor_mul(fa, fy, fy)
                        nc.vector.tensor_mul(fa, fa, fx)
                        nc.vector.tensor_scalar(fa, fa, -0.5, 1.5,
                                                op0=ALU.mult, op1=ALU.add)
                        nc.vector.tensor_mul(rq, fy, fa)
                        nc.vector.tensor_mul(fa, rq, rq)
                        nc.vector.tensor_mul(fa, fa, fx)
                        nc.vector.tensor_scalar(fa, fa, -0.5 * qscale,
                                                1.5 * qscale,
                                                op0=ALU.mult, op1=ALU.add)
                        nc.vector.tensor_mul(rq, rq, fa)
                    else:
                        rt = spool.tile([P, nh], F32, tag=f"rt{nh}")
                        nc.scalar.activation(rt, ss, AF.Sqrt, scale=1.0 / D,
                                             bias=EPS)
                        nc.vector.reciprocal(rq, rt)
                        if qscale != 1.0:
                            nc.vector.tensor_scalar_mul(rq, rq, qscale)
                    qbf = spool.tile([P, nh, D], BF16, tag=f"qbf{nh}")
                    nc.vector.tensor_mul(qbf, rf, rq[:, :, None].to_broadcast([P, nh, D]))
                    pending_tr.append((qbf, dstT, h0, nh, t))

                qsc = 1.0 / float(np.sqrt(D))
                rope_norm(ps_q0, 4, qT, 0, qsc)
                rope_norm(ps_q1, 4, qT, 4, qsc)
                rope_norm(ps_k, KVG, kT, 0, 1.0)
                if t == TOKCH - 1:
                    # prewarm the exp act-table; the 1.28us load runs behind
                    # the V tail
                    nc.scalar.activation(warm, zero_col[0:1, :], AF.Exp)

            # lagged V tail: pure PE work that covers the final rope chains;
            # the last transposes flush once their rope chain has had V cover
            for tv in range(TOKCH - VLAG, TOKCH):
                v_chunk(tv)
                if tv == TOKCH - 2:
                    flush_trs()

        # ================= phase B: attention ============================
        # wp prefetch: issue at phase-B start so the tiles are resident long
        # before phase C begins (phase-A pools have closed, SBUF is free)
        wpool = ctx.enter_context(tc.tile_pool(name="wp", bufs=1))
        wpr = wp.rearrange("(hc p) c -> p hc c", p=P)
        wp_ts = []
        for ct in range(C // 512):
            wp_t = wpool.tile([P, HG, 512], BF16, tag=f"wpt{ct}")
            nc.sync.dma_start(wp_t, wpr[:, :, ct * 512:(ct + 1) * 512])
            wp_ts.append(wp_t)

        if "B" not in PHASES:
            pass
        else:
         with tc.tile_pool(name="psc", bufs=4, space="PSUM") as psc, \
             tc.tile_pool(name="psy", bufs=2, space="PSUM") as psy, \
             tc.tile_pool(name="pss", bufs=2, space="PSUM") as pss, \
             tc.tile_pool(name="pb", bufs=6) as ppool, \
             tc.tile_pool(name="sb", bufs=3) as bpool:
            NT = T // 512  # 4 tq tiles
            # software pipeline: the PE queue is in-order, so scores for
            # chunk idx+DEPTH are emitted before pv/ones of chunk idx; the
            # scores->mask->exp chain (~1.6us) hides behind DEPTH chunks of
            # PE work.  The (h, c) stream is flattened so the pipeline also
            # covers head boundaries.
            DEPTH = 3
            for t in range(NT):
                nch = 4 * (t + 1)
                items = [(h, c) for h in range(HG) for c in range(nch)]
                live = {}

                def front(idx):
                    h, c = items[idx]
                    g = h // NREP
                    o = c * P - t * 512
                    col0 = max(o, 0)
                    ps_sc = psc.tile([P, 512], F32, tag="sc")
                    nc.tensor.matmul(
                        ps_sc[:, col0:512], kT[:, g, c * P:(c + 1) * P],
                        qT[:, h, t * 512 + col0:(t + 1) * 512],
                        start=True, stop=(o < 0))
                    if o >= 0:
                        # after the col0 shift the partial block is always the
                        # i' >= j triangle; accumulate the additive mask with
                        # a 128-col matmul (53ns) right behind the scores
                        nc.tensor.matmul(ps_sc[:, col0:col0 + P], ident,
                                         mask_sb, start=False, stop=True)
                    pt = ppool.tile([P, 512], BF16, tag="pt")
                    nc.scalar.activation(pt[:, col0:512], ps_sc[:, col0:512],
                                         AF.Exp)
                    live[idx] = (pt, col0)

                for i in range(min(DEPTH, len(items))):
                    front(i)
                ys = {}
                for idx, (h, c) in enumerate(items):
                    if idx + DEPTH < len(items):
                        front(idx + DEPTH)
                    g = h // NREP
                    if c == 0:
                        ps_y = psy.tile([P, 512], F32, tag="y")
                        ps_sden = pss.tile([P, 512], F32, tag="sden")
                        ys[h] = (ps_y, ps_sden)
                    ps_y, ps_sden = ys[h]
                    ps_s = ps_sden[0:1, :]
                    pt, col0 = live.pop(idx)
                    st = dict(start=(c == 0), stop=(c == nch - 1))
                    nc.tensor.matmul(ps_y[:, col0:512],
                                     v_sb[:, c, g * P:(g + 1) * P],
                                     pt[:, col0:512], **st)
                    nc.tensor.matmul(ps_s[:, col0:512], ones_col,
                                     pt[:, col0:512], **st)
                    if c == nch - 1:
                        # recip first (frees the single pss buffer fastest),
                        # then copy (frees ps_y); normalize the bf16 slice in
                        # place on the Pool engine once the broadcast lands
                        yslice = yT[:, h, t * 512:(t + 1) * 512]
                        rc = bpool.tile([1, 512], F32, tag="rc")
                        nc.vector.reciprocal(rc, ps_s)
                        nc.vector.tensor_copy(yslice, ps_y)
                        rb = bpool.tile([P, 512], F32, tag="rb")
                        nc.gpsimd.partition_broadcast(rb, rc, channels=P)
                        nc.vector.tensor_mul(yslice, yslice, rb)

        if DEBUG_DUMP:
            with tc.tile_pool(name="dbg", bufs=2) as dpool:
                for h in range(HG):
                    dt_ = dpool.tile([P, T], F32, tag="d")
                    nc.vector.tensor_copy(dt_, qT[:, h, :])
                    nc.sync.dma_start(d_qt[:, h, :], dt_)
                    dt_ = dpool.tile([P, T], F32, tag="d")
                    nc.vector.tensor_copy(dt_, yT[:, h, :])
                    nc.sync.dma_start(d_yt[:, h, :], dt_)
                for g in range(KVG):
                    dt_ = dpool.tile([P, T], F32, tag="d")
                    nc.vector.tensor_copy(dt_, kT[:, g, :])
                    nc.sync.dma_start(d_kt[:, g, :], dt_)
                dt_ = dpool.tile([P, TOKCH * KC], F32, tag="d")
                nc.vector.tensor_copy(dt_.rearrange("p (a b) -> p a b", a=TOKCH), v_sb[:, :, :])
                nc.sync.dma_start(d_v[:, :, :], dt_.rearrange("p (a b) -> p a b", a=TOKCH))

        # ================= phase C: output projection =====================
        if "C" not in PHASES:
            pass
        else:
         with tc.tile_pool(name="po", bufs=2, space="PSUM") as pso, \
             tc.tile_pool(name="so", bufs=3) as opool:
            for t in range(TOKCH):
                for ct in range(C // 512):
                    ps_o = pso.tile([P, 512], F32, tag="o")
                    last = (t == TOKCH - 1 and ct == C // 512 - 1)
                    ob = opool.tile([P, 512], F32, tag="ob")
                    if not last:
                        for hc in range(HG):
                            nc.tensor.matmul(
                                ps_o, yT[:, hc, t * P:(t + 1) * P],
                                wp_ts[ct][:, hc, :],
                                start=(hc == 0), stop=(hc == HG - 1))
                        nc.vector.tensor_copy(ob, ps_o)
                        nc.sync.dma_start(
                            out[t * P:(t + 1) * P, ct * 512:(ct + 1) * 512], ob)
                    else:
                        # final tile in two pipelined halves to shorten the
                        # copy->dma drain tail; halves go out on different
                        # queues so the transfers overlap
                        for q in range(2):
                            cs = slice(q * 256, (q + 1) * 256)
                            for hc in range(HG):
                                nc.tensor.matmul(
                                    ps_o[:, cs], yT[:, hc, t * P:(t + 1) * P],
                                    wp_ts[ct][:, hc, cs],
                                    start=(hc == 0), stop=(hc == HG - 1))
                            (nc.vector.tensor_copy if q == 0
                             else nc.scalar.copy)(ob[:, cs], ps_o[:, cs])
                            (nc.sync if q == 0 else nc.scalar).dma_start(
                                out[t * P:(t + 1) * P,
                                    ct * 512 + q * 256:ct * 512 + (q + 1) * 256],
                                ob[:, cs])
    nc.compile()
    return nc


_NC_CACHE = []


def _get_prog():
    if not _NC_CACHE:
        _NC_CACHE.append(_build())
    return _NC_CACHE[0]


def _make_in_maps(inputs):
    x, cos, sin = inputs["x"], inputs["cos"], inputs["sin"]
    wq, wk, wv, wproj = inputs["wq"], inputs["wk"], inputs["wv"], inputs["wproj"]
    bf = ml_dtypes.bfloat16
    # [p, tc, d] tiling (contiguous 4KB DMA rows)
    cos2 = np.ascontiguousarray(
        cos.reshape(TOKCH, P, D // 2).transpose(1, 0, 2), dtype=np.float32)
    sin2 = np.ascontiguousarray(
        sin.reshape(TOKCH, P, D // 2).transpose(1, 0, 2), dtype=np.float32)
    in_maps = []
    for core in range(8):
        b, g = core // 2, core % 2
        qs = slice(g * QC, (g + 1) * QC)
        ks = slice(g * KC, (g + 1) * KC)
        # x[b].T is [C, T]; tile to [tokch, p(C-chunk), co, tk]
        xtb = (x[b].T.astype(bf)
               .reshape(C // P, P, TOKCH, P)     # [co, p, tc, tk]
               .transpose(2, 1, 0, 3))           # [tc, p, co, tk]
        in_maps.append({
            "xt": np.ascontiguousarray(xtb),
            "wq": np.ascontiguousarray(wq[:, qs]).astype(bf),
            "wkv": np.ascontiguousarray(np.hstack([wk[:, ks], wv[:, ks]])).astype(bf),
            "wp": np.ascontiguousarray(wproj[qs, :]).astype(bf),
            "cosd": cos2,
            "sind": sin2,
        })
    return in_maps


def kernel(x, cos, sin, wq, wk, wv, wproj):
    nc = _get_prog()
    in_maps = _make_in_maps(dict(x=x, cos=cos, sin=sin, wq=wq, wk=wk, wv=wv, wproj=wproj))
    res = run_bass_kernel_spmd(nc, in_maps, core_ids=list(range(8))).results
    outp = np.empty((B, T, C), np.float32)
    for b in range(B):
        outp[b] = res[2 * b]["out"] + res[2 * b + 1]["out"]
    return outp



# revision 91
# speedup vs baseline: 1.2022x; 1.0001x over previous
"""Causal self-attention (GQA + RoPE + QK-norm) Trainium2 Bass kernel.

Sharding: 8 cores = 4 batches x 2 head-groups.  Core c -> batch c//2,
q heads (c%2)*8..+8, kv heads (c%2)*2..+2.  wproj is row-sharded, so each
core emits a partial (T, C) output; the host sums the two partials per batch.

Device-side layout strategy (per core):
  - x is fed pre-transposed (xT, [C, T]) and bf16-cast by the host.
  - QKV projections produce Q,K token-major ([tok, cols]); RoPE + rms-norm
    run token-major (free-axis per-head reductions), then 128x128 PE
    transposes produce qT/kT feature-major for the attention matmuls.
    V is produced token-major, which is exactly the p@v stationary layout.
  - scores are computed transposed (scoresT[tk, tq]) so that after exp the
    p tiles are already the moving operand for the p@v matmul; the softmax
    denominator comes from a ones-column matmul accumulated in PSUM.
  - exp has no max-subtraction: qk-norm bounds |s| <= sqrt(128) ~ 11.32.
  - output projection accumulates over the 8 local heads; partial written
    fp32 to DRAM.
"""

import numpy as np
import ml_dtypes
from contextlib import ExitStack

import concourse.bass as bass
import concourse.mybir as mybir
import concourse.tile as tile
from concourse import bacc
from concourse.bass_utils import run_bass_kernel_spmd
from concourse.masks import make_identity

BF16 = mybir.dt.bfloat16
F32 = mybir.dt.float32
F32R = mybir.dt.float32r
AF = mybir.ActivationFunctionType

B, T, C = 4, 2048, 2048
H, KV, D = 16, 4, 128
HG, KVG = H // 2, KV // 2          # per-core q heads (8), kv heads (2)
QC, KC = HG * D, KVG * D           # 1024, 256
P = 128
TOKCH = T // P                     # 16 token chunks
NREP = H // KV                     # 4
EPS = 1e-5
NEG = -1.0e5                       # additive causal mask (exp -> 0)


DEBUG_DUMP = False
PHASES = ("A", "B", "C")


def _build():
    nc = bacc.Bacc("TRN2", target_bir_lowering=False, debug=False, num_devices=8)
    # x pre-tiled by the host as [tokch, p, co, tk] so every DMA partition row
    # is 4KB contiguous (co*tk*2B) instead of 256B strided
    xt = nc.dram_tensor("xt", [TOKCH, P, C // P, P], BF16, kind="ExternalInput")
    wq = nc.dram_tensor("wq", [C, QC], BF16, kind="ExternalInput")
    wkv = nc.dram_tensor("wkv", [C, 2 * KC], BF16, kind="ExternalInput")
    wp = nc.dram_tensor("wp", [QC, C], BF16, kind="ExternalInput")
    # cos/sin pre-tiled by host as [p, tc, d] (contiguous 4KB rows)
    cosd = nc.dram_tensor("cosd", [P, TOKCH, D // 2], F32, kind="ExternalInput")
    sind = nc.dram_tensor("sind", [P, TOKCH, D // 2], F32, kind="ExternalInput")
    out = nc.dram_tensor("out", [T, C], F32, kind="ExternalOutput")
    if DEBUG_DUMP:
        d_qt = nc.dram_tensor("d_qt", [P, HG, T], F32, kind="ExternalOutput")
        d_kt = nc.dram_tensor("d_kt", [P, KVG, T], F32, kind="ExternalOutput")
        d_v = nc.dram_tensor("d_v", [P, TOKCH, KC], F32, kind="ExternalOutput")
        d_yt = nc.dram_tensor("d_yt", [P, HG, T], F32, kind="ExternalOutput")

    with tile.TileContext(nc) as tc, ExitStack() as ctx:
        singles = ctx.enter_context(tc.tile_pool(name="singles", bufs=1))
        # bufs must cover the V-lag window (xtile(t) is re-read by the lagged
        # V projection at iteration t+VLAG); the pool closes with phase A
        phase_a_pools = ExitStack()
        xpool = phase_a_pools.enter_context(tc.tile_pool(name="xa", bufs=8))

        # ---- prefetch the first x tile before the weight bulk so the PE
        # can start within a few us ----
        # ---- resident tensors ----
        # weight DMAs issued per-co round-robin over both HWDGE queues so
        # early co chunks land in consumption order and issue rate isn't
        # limited by one sequencer (~600ns per dma_start).  The first x
        # chunk + first co weights go out first so the PE starts ASAP.
        wq_sb = singles.tile([P, C // P, QC], BF16)
        wkv_sb = singles.tile([P, C // P, 2 * KC], BF16)
        wqr = wq.rearrange("(co p) q -> p co q", p=P)
        wkvr = wkv.rearrange("(co p) q -> p co q", p=P)
        cos_sb = singles.tile([P, TOKCH, D // 2], F32)
        sin_sb = singles.tile([P, TOKCH, D // 2], F32)
        xtile0 = xpool.tile([P, C // P, P], BF16, tag="xt")
        nc.sync.dma_start(xtile0[:, 0:4, :], xt[0, :, 0:4, :])
        nc.scalar.dma_start(wq_sb[:, 0, :], wqr[:, 0, :])
        nc.sync.dma_start(wkv_sb[:, 0, :], wkvr[:, 0, :])
        nc.scalar.dma_start(wq_sb[:, 1, :], wqr[:, 1, :])
        for g4 in range(1, 4):
            nc.sync.dma_start(xtile0[:, 4 * g4:4 * (g4 + 1), :],
                              xt[0, :, 4 * g4:4 * (g4 + 1), :])
        nc.gpsimd.dma_start(wkv_sb[:, 1, :], wkvr[:, 1, :])
        nc.scalar.dma_start(cos_sb, cosd[:])
        nc.sync.dma_start(sin_sb, sind[:])
        # remaining weights in 2-co pieces: fewer issues (the sequencers'
        # ~600ns per dma_start is the startup bottleneck), still fine-grained
        # enough to land in consumption order
        qs = [nc.sync, nc.scalar]
        for i, co in enumerate(range(2, C // P, 2)):
            eng = qs[i % 2]
            eng.dma_start(wq_sb[:, co:co + 2, :], wqr[:, co:co + 2, :])
            nc.gpsimd.dma_start(wkv_sb[:, co:co + 2, :], wkvr[:, co:co + 2, :])

        ident = singles.tile([P, P], BF16)
        make_identity(nc, ident)
        ones_col = singles.tile([P, 1], BF16)
        nc.vector.memset(ones_col, 1.0)
        zero_col = singles.tile([P, 1], F32)
        nc.vector.memset(zero_col, 0.0)
        eps_col = singles.tile([P, 1], F32)
        nc.vector.memset(eps_col, EPS)
        nc.const_aps.aps[(F32, 0.0)] = zero_col[:]
        nc.const_aps.aps[(F32, EPS)] = eps_col[:]
        # scratch for the dummy exp that prewarms the exp act-table at the
        # A->B phase boundary (overlaps the 1.28us table load)
        warm = singles.tile([1, 1], F32)

        # diagonal-block mask: keep where i >= j (j = tk partition, i = tq
        # free).  bf16 so it can be ADDED into the scores psum by a 128-col
        # matmul (ident.T @ mask) instead of a DVE op in the exp chain.
        mask_sb = singles.tile([P, P], BF16)
        nc.vector.memset(mask_sb, 0.0)
        nc.gpsimd.affine_select(
            out=mask_sb, in_=mask_sb,
            compare_op=mybir.AluOpType.is_ge, fill=NEG,
            base=0, pattern=[[1, P]], channel_multiplier=-1,
        )

        qT = singles.tile([P, HG, T], BF16)      # [d, h, tok]
        kT = singles.tile([P, KVG, T], BF16)
        v_sb = singles.tile([P, TOKCH, KC], BF16)  # [tok%128, chunk, vcol]
        yT = singles.tile([P, HG, T], BF16)

        # ================= phase A: QKV proj + RoPE + qk-norm =============
        if "A" not in PHASES:
            pass
        else:
         with phase_a_pools, \
             tc.tile_pool(name="pa", bufs=2, space="PSUM") as pps, \
             tc.tile_pool(name="pkv", bufs=1, space="PSUM") as pkv, \
             tc.tile_pool(name="sa", bufs=3) as spool:
            # The V projection is split out of the QK pass and lagged by VLAG
            # chunks: the final VLAG V-chunks are pure PE work that runs while
            # the last rope chains (DVE) drain, so phase B starts without
            # waiting on the phase-A tail.
            VLAG = 6
            nco = C // P
            xtiles = {}
            # transposes lag one iteration behind their rope chain so they
            # never sit dep-blocked in the PE's 4-deep wait queue
            pending_tr = []

            def flush_trs():
                while pending_tr:
                    qbf, dstT, h0, nh, tt = pending_tr.pop(0)
                    pst = pkv.tile([P, 4, P], BF16, tag="tr")
                    for i in range(nh):
                        nc.tensor.transpose(pst[:, i, :], qbf[:, i, :], ident)
                    nc.scalar.copy(
                        dstT[:, h0:h0 + nh, tt * P:(tt + 1) * P], pst[:, 0:nh, :])

            def v_chunk(tv):
                xv = xtiles.pop(tv)
                # alternate psum tags so consecutive V chunks don't serialize
                # on one buffer's Act-copy release
                ps_v = pkv.tile([P, KC], F32, tag=("v" if tv % 2 == 0 else "v2"))
                for co in range(nco):
                    nc.tensor.matmul(ps_v, xv[:, co, :],
                                     wkv_sb[:, co, KC:2 * KC],
                                     start=(co == 0), stop=(co == nco - 1))
                # cast straight to resident token-major buffer (Act engine;
                # DVE is the critical engine in this phase)
                nc.scalar.copy(v_sb[:, tv, :], ps_v)

            for t in range(TOKCH):
                if t == 0:
                    xtile = xtile0
                else:
                    xtile = xpool.tile([P, C // P, P], BF16, tag="xt")
                    nc.sync.dma_start(xtile, xt[t])
                xtiles[t] = xtile
                ps_q0 = pps.tile([P, 512], F32, tag="q0")
                ps_q1 = pps.tile([P, 512], F32, tag="q1")
                ps_k = pkv.tile([P, KC], F32, tag="k")
                for co in range(nco):
                    lhsT = xtile[:, co, :]
                    st = dict(start=(co == 0), stop=(co == nco - 1))
                    nc.tensor.matmul(ps_q0, lhsT, wq_sb[:, co, 0:512], **st)
                    nc.tensor.matmul(ps_q1, lhsT, wq_sb[:, co, 512:1024], **st)
                    nc.tensor.matmul(ps_k, lhsT, wkv_sb[:, co, 0:KC], **st)
                if t >= VLAG:
                    v_chunk(t - VLAG)
                # previous iteration's transposes: rope chains long done
                flush_trs()

                # Q/K: fused multi-head rope + rms-norm + cast + transpose
                def rope_norm(ps, nh, dstT, h0, qscale, rsq_dve=False):
                    h2 = D // 2
                    v4 = ps.rearrange("p (h a d) -> p h a d", h=nh, a=2)
                    q1, q2 = v4[:, :, 0, :], v4[:, :, 1, :]
                    r = spool.tile([P, nh, 2, h2], F32, tag=f"rope{nh}")
                    r1, r2 = r[:, :, 0, :], r[:, :, 1, :]
                    s2 = spool.tile([P, nh, h2], F32, tag=f"scr{nh}")
                    cs = cos_sb[:, t, None, :].to_broadcast([P, nh, h2])
                    sn = sin_sb[:, t, None, :].to_broadcast([P, nh, h2])
                    nc.vector.tensor_mul(r1, q1, cs)
                    nc.vector.tensor_mul(s2, q2, sn)
                    nc.vector.tensor_sub(r1, r1, s2)
                    nc.vector.tensor_mul(r2, q1, sn)
                    nc.vector.tensor_mul(s2, q2, cs)
                    nc.vector.tensor_add(r2, r2, s2)
                    rf = r.rearrange("p h a d -> p h (a d)")
                    sq = spool.tile([P, nh, D], F32, tag=f"sq{nh}")
                    ss = spool.tile([P, nh], F32, tag=f"ss{nh}")
                    if rsq_dve:
                        # keep the last iteration's rope entirely off the Act
                        # engine so phase B's first exps aren't queued behind it
                        nc.vector.tensor_mul(sq, rf, rf)
                    else:
                        nc.scalar.activation(sq, rf, AF.Square)
                    nc.vector.tensor_reduce(ss, sq, axis=mybir.AxisListType.X,
                                            op=mybir.AluOpType.add)
                    rq = spool.tile([P, nh], F32, tag=f"rq{nh}")
                    if rsq_dve:
                        # DVE-only fast inverse sqrt (bit trick + 2 Newton
                        # steps, qscale folded into the last).  Used for the
                        # final token chunk so the previous iteration's Sqrt
                        # is the Act engine's last sqrt-set op and the exp
                        # table load hides behind the V tail.
                        ALU = mybir.AluOpType
                        I32 = mybir.dt.int32
                        fx = spool.tile([P, nh], F32, tag=f"fx{nh}")
                        nc.vector.tensor_scalar(fx, ss, 1.0 / D, EPS,
                                                op0=ALU.mult, op1=ALU.add)
                        fj = spool.tile([P, nh], I32, tag=f"fj{nh}")
                        nc.vector.tensor_scalar(fj, fx[:].bitcast(I32), 1, None,
                                                op0=ALU.logical_shift_right)
                        nc.vector.tensor_scalar(fj, fj, -1, 0x5f3759df + 1,
                                                op0=ALU.bitwise_xor, op1=ALU.add)
                        fy = fj[:].bitcast(F32)
                        fa = spool.tile([P, nh], F32, tag=f"fa{nh}")
                        nc.vector.tensor_mul(fa, fy, fy)
                        nc.vector.tensor_mul(fa, fa, fx)
                        nc.vector.tensor_scalar(fa, fa, -0.5, 1.5,
                                                op0=ALU.mult, op1=ALU.add)
                        nc.vector.tensor_mul(rq, fy, fa)
                        nc.vector.tensor_mul(fa, rq, rq)
                        nc.vector.tensor_mul(fa, fa, fx)
                        nc.vector.tensor_scalar(fa, fa, -0.5 * qscale,
                                                1.5 * qscale,
                                                op0=ALU.mult, op1=ALU.add)
                        nc.vector.tensor_mul(rq, rq, fa)
                    else:
                        rt = spool.tile([P, nh], F32, tag=f"rt{nh}")
                        nc.scalar.activation(rt, ss, AF.Sqrt, scale=1.0 / D,
                                             bias=EPS)
                        nc.vector.reciprocal(rq, rt)
                        if qscale != 1.0:
                            nc.vector.tensor_scalar_mul(rq, rq, qscale)
                    qbf = spool.tile([P, nh, D], BF16, tag=f"qbf{nh}")
                    nc.vector.tensor_mul(qbf, rf, rq[:, :, None].to_broadcast([P, nh, D]))
                    pending_tr.append((qbf, dstT, h0, nh, t))

                qsc = 1.0 / float(np.sqrt(D))
                rope_norm(ps_q0, 4, qT, 0, qsc)
                rope_norm(ps_q1, 4, qT, 4, qsc)
                rope_norm(ps_k, KVG, kT, 0, 1.0)
                if t == TOKCH - 1:
                    # prewarm the exp act-table; the 1.28us load runs behind
                    # the V tail
                    nc.scalar.activation(warm, zero_col[0:1, :], AF.Exp)

            # lagged V tail: pure PE work that covers the final rope chains;
            # the last transposes flush once their rope chain has had V cover
            for tv in range(TOKCH - VLAG, TOKCH):
                v_chunk(tv)
                if tv == TOKCH - 2:
                    flush_trs()

        # ================= phase B: attention ============================
        # wp prefetch: issue at phase-B start so the tiles are resident long
        # before phase C begins (phase-A pools have closed, SBUF is free)
        wpool = ctx.enter_context(tc.tile_pool(name="wp", bufs=1))
        wpr = wp.rearrange("(hc p) c -> p hc c", p=P)
        wp_ts = []
        for ct in range(C // 512):
            wp_t = wpool.tile([P, HG, 512], BF16, tag=f"wpt{ct}")
            nc.sync.dma_start(wp_t, wpr[:, :, ct * 512:(ct + 1) * 512])
            wp_ts.append(wp_t)

        if "B" not in PHASES:
            pass
        else:
         with tc.tile_pool(name="psc", bufs=4, space="PSUM") as psc, \
             tc.tile_pool(name="psy", bufs=2, space="PSUM") as psy, \
             tc.tile_pool(name="pss", bufs=2, space="PSUM") as pss, \
             tc.tile_pool(name="pb", bufs=6) as ppool, \
             tc.tile_pool(name="sb", bufs=3) as bpool:
            NT = T // 512  # 4 tq tiles
            # software pipeline: the PE queue is in-order, so scores for
            # chunk idx+DEPTH are emitted before pv/ones of chunk idx; the
            # scores->mask->exp chain (~1.6us) hides behind DEPTH chunks of
            # PE work.  The (h, c) stream is flattened so the pipeline also
            # covers head boundaries.
            DEPTH = 4
            for t in range(NT):
                nch = 4 * (t + 1)
                items = [(h, c) for h in range(HG) for c in range(nch)]
                live = {}

                def front(idx):
                    h, c = items[idx]
                    g = h // NREP
                    o = c * P - t * 512
                    col0 = max(o, 0)
                    ps_sc = psc.tile([P, 512], F32, tag="sc")
                    nc.tensor.matmul(
                        ps_sc[:, col0:512], kT[:, g, c * P:(c + 1) * P],
                        qT[:, h, t * 512 + col0:(t + 1) * 512],
                        start=True, stop=(o < 0))
                    if o >= 0:
                        # after the col0 shift the partial block is always the
                        # i' >= j triangle; accumulate the additive mask with
                        # a 128-col matmul (53ns) right behind the scores
                        nc.tensor.matmul(ps_sc[:, col0:col0 + P], ident,
                                         mask_sb, start=False, stop=True)
                    pt = ppool.tile([P, 512], BF16, tag="pt")
                    nc.scalar.activation(pt[:, col0:512], ps_sc[:, col0:512],
                                         AF.Exp)
                    live[idx] = (pt, col0)

                for i in range(min(DEPTH, len(items))):
                    front(i)
                ys = {}
                for idx, (h, c) in enumerate(items):
                    if idx + DEPTH < len(items):
                        front(idx + DEPTH)
                    g = h // NREP
                    if c == 0:
                        ps_y = psy.tile([P, 512], F32, tag="y")
                        ps_sden = pss.tile([P, 512], F32, tag="sden")
                        ys[h] = (ps_y, ps_sden)
                    ps_y, ps_sden = ys[h]
                    ps_s = ps_sden[0:1, :]
                    pt, col0 = live.pop(idx)
                    st = dict(start=(c == 0), stop=(c == nch - 1))
                    nc.tensor.matmul(ps_y[:, col0:512],
                                     v_sb[:, c, g * P:(g + 1) * P],
                                     pt[:, col0:512], **st)
                    nc.tensor.matmul(ps_s[:, col0:512], ones_col,
                                     pt[:, col0:512], **st)
                    if c == nch - 1:
                        # recip first (frees the single pss buffer fastest),
                        # then copy (frees ps_y); normalize the bf16 slice in
                        # place on the Pool engine once the broadcast lands
                        yslice = yT[:, h, t * 512:(t + 1) * 512]
                        rc = bpool.tile([1, 512], F32, tag="rc")
                        nc.vector.reciprocal(rc, ps_s)
                        nc.vector.tensor_copy(yslice, ps_y)
                        rb = bpool.tile([P, 512], F32, tag="rb")
                        nc.gpsimd.partition_broadcast(rb, rc, channels=P)
                        nc.vector.tensor_mul(yslice, yslice, rb)

        if DEBUG_DUMP:
            with tc.tile_pool(name="dbg", bufs=2) as dpool:
                for h in range(HG):
                    dt_ = dpool.tile([P, T], F32, tag="d")
                    nc.vector.tensor_copy(dt_, qT[:, h, :])
                    nc.sync.dma_start(d_qt[:, h, :], dt_)
                    dt_ = dpool.tile([P, T], F32, tag="d")
                    nc.vector.tensor_copy(dt_, yT[:, h, :])
                    nc.sync.dma_start(d_yt[:, h, :], dt_)
                for g in range(KVG):
                    dt_ = dpool.tile([P, T], F32, tag="d")
                    nc.vector.tensor_copy(dt_, kT[:, g, :])
                    nc.sync.dma_start(d_kt[:, g, :], dt_)
                dt_ = dpool.tile([P, TOKCH * KC], F32, tag="d")
                nc.vector.tensor_copy(dt_.rearrange("p (a b) -> p a b", a=TOKCH), v_sb[:, :, :])
                nc.sync.dma_start(d_v[:, :, :], dt_.rearrange("p (a b) -> p a b", a=TOKCH))

        # ================= phase C: output projection =====================
        if "C" not in PHASES:
            pass
        else:
         with tc.tile_pool(name="po", bufs=2, space="PSUM") as pso, \
             tc.tile_pool(name="so", bufs=3) as opool:
            for t in range(TOKCH):
                for ct in range(C // 512):
                    ps_o = pso.tile([P, 512], F32, tag="o")
                    last = (t == TOKCH - 1 and ct == C // 512 - 1)
                    ob = opool.tile([P, 512], F32, tag="ob")
                    if not last:
                        for hc in range(HG):
                            nc.tensor.matmul(
                                ps_o, yT[:, hc, t * P:(t + 1) * P],
                                wp_ts[ct][:, hc, :],
                                start=(hc == 0), stop=(hc == HG - 1))
                        nc.vector.tensor_copy(ob, ps_o)
                        nc.sync.dma_start(
                            out[t * P:(t + 1) * P, ct * 512:(ct + 1) * 512], ob)
                    else:
                        # final tile in two pipelined halves to shorten the
                        # copy->dma drain tail; halves go out on different
                        # queues so the transfers overlap
                        for q in range(2):
                            cs = slice(q * 256, (q + 1) * 256)
                            for hc in range(HG):
                                nc.tensor.matmul(
                                    ps_o[:, cs], yT[:, hc, t * P:(t + 1) * P],
                                    wp_ts[ct][:, hc, cs],
                                    start=(hc == 0), stop=(hc == HG - 1))
                            (nc.vector.tensor_copy if q == 0
                             else nc.scalar.copy)(ob[:, cs], ps_o[:, cs])
                            (nc.sync if q == 0 else nc.scalar).dma_start(
                                out[t * P:(t + 1) * P,
                                    ct * 512 + q * 256:ct * 512 + (q + 1) * 256],
                                ob[:, cs])
    nc.compile()
    return nc


_NC_CACHE = []


def _get_prog():
    if not _NC_CACHE:
        _NC_CACHE.append(_build())
    return _NC_CACHE[0]


def _make_in_maps(inputs):
    x, cos, sin = inputs["x"], inputs["cos"], inputs["sin"]
    wq, wk, wv, wproj = inputs["wq"], inputs["wk"], inputs["wv"], inputs["wproj"]
    bf = ml_dtypes.bfloat16
    # [p, tc, d] tiling (contiguous 4KB DMA rows)
    cos2 = np.ascontiguousarray(
        cos.reshape(TOKCH, P, D // 2).transpose(1, 0, 2), dtype=np.float32)
    sin2 = np.ascontiguousarray(
        sin.reshape(TOKCH, P, D // 2).transpose(1, 0, 2), dtype=np.float32)
    in_maps = []
    for core in range(8):
        b, g = core // 2, core % 2
        qs = slice(g * QC, (g + 1) * QC)
        ks = slice(g * KC, (g + 1) * KC)
        # x[b].T is [C, T]; tile to [tokch, p(C-chunk), co, tk]
        xtb = (x[b].T.astype(bf)
               .reshape(C // P, P, TOKCH, P)     # [co, p, tc, tk]
               .transpose(2, 1, 0, 3))           # [tc, p, co, tk]
        in_maps.append({
            "xt": np.ascontiguousarray(xtb),
            "wq": np.ascontiguousarray(wq[:, qs]).astype(bf),
            "wkv": np.ascontiguousarray(np.hstack([wk[:, ks], wv[:, ks]])).astype(bf),
            "wp": np.ascontiguousarray(wproj[qs, :]).astype(bf),
            "cosd": cos2,
            "sind": sin2,
        })
    return in_maps


def kernel(x, cos, sin, wq, wk, wv, wproj):
    nc = _get_prog()
    in_maps = _make_in_maps(dict(x=x, cos=cos, sin=sin, wq=wq, wk=wk, wv=wv, wproj=wproj))
    res = run_bass_kernel_spmd(nc, in_maps, core_ids=list(range(8))).results
    outp = np.empty((B, T, C), np.float32)
    for b in range(B):
        outp[b] = res[2 * b]["out"] + res[2 * b + 1]["out"]
    return outp



# revision 97
# speedup vs baseline: 1.2022x; 1.0000x over previous
"""Causal self-attention (GQA + RoPE + QK-norm) Trainium2 Bass kernel.

Sharding: 8 cores = 4 batches x 2 head-groups.  Core c -> batch c//2,
q heads (c%2)*8..+8, kv heads (c%2)*2..+2.  wproj is row-sharded, so each
core emits a partial (T, C) output; the host sums the two partials per batch.

Device-side layout strategy (per core):
  - x is fed pre-transposed (xT, [C, T]) and bf16-cast by the host.
  - QKV projections produce Q,K token-major ([tok, cols]); RoPE + rms-norm
    run token-major (free-axis per-head reductions), then 128x128 PE
    transposes produce qT/kT feature-major for the attention matmuls.
    V is produced token-major, which is exactly the p@v stationary layout.
  - scores are computed transposed (scoresT[tk, tq]) so that after exp the
    p tiles are already the moving operand for the p@v matmul; the softmax
    denominator comes from a ones-column matmul accumulated in PSUM.
  - exp has no max-subtraction: qk-norm bounds |s| <= sqrt(128) ~ 11.32.
  - output projection accumulates over the 8 local heads; partial written
    fp32 to DRAM.
"""

import numpy as np
import ml_dtypes
from contextlib import ExitStack

import concourse.bass as bass
import concourse.mybir as mybir
import concourse.tile as tile
from concourse import bacc
from concourse.bass_utils import run_bass_kernel_spmd
from concourse.masks import make_identity

BF16 = mybir.dt.bfloat16
F32 = mybir.dt.float32
F32R = mybir.dt.float32r
AF = mybir.ActivationFunctionType

B, T, C = 4, 2048, 2048
H, KV, D = 16, 4, 128
HG, KVG = H // 2, KV // 2          # per-core q heads (8), kv heads (2)
QC, KC = HG * D, KVG * D           # 1024, 256
P = 128
TOKCH = T // P                     # 16 token chunks
NREP = H // KV                     # 4
EPS = 1e-5
NEG = -1.0e5                       # additive causal mask (exp -> 0)


DEBUG_DUMP = False
PHASES = ("A", "B", "C")


def _build():
    nc = bacc.Bacc("TRN2", target_bir_lowering=False, debug=False, num_devices=8)
    # x pre-tiled by the host as [tokch, p, co, tk] so every DMA partition row
    # is 4KB contiguous (co*tk*2B) instead of 256B strided
    xt = nc.dram_tensor("xt", [TOKCH, P, C // P, P], BF16, kind="ExternalInput")
    wq = nc.dram_tensor("wq", [C, QC], BF16, kind="ExternalInput")
    wkv = nc.dram_tensor("wkv", [C, 2 * KC], BF16, kind="ExternalInput")
    wp = nc.dram_tensor("wp", [QC, C], BF16, kind="ExternalInput")
    # cos/sin pre-tiled by host as [p, tc, d] (contiguous 4KB rows)
    cosd = nc.dram_tensor("cosd", [P, TOKCH, D // 2], F32, kind="ExternalInput")
    sind = nc.dram_tensor("sind", [P, TOKCH, D // 2], F32, kind="ExternalInput")
    out = nc.dram_tensor("out", [T, C], F32, kind="ExternalOutput")
    if DEBUG_DUMP:
        d_qt = nc.dram_tensor("d_qt", [P, HG, T], F32, kind="ExternalOutput")
        d_kt = nc.dram_tensor("d_kt", [P, KVG, T], F32, kind="ExternalOutput")
        d_v = nc.dram_tensor("d_v", [P, TOKCH, KC], F32, kind="ExternalOutput")
        d_yt = nc.dram_tensor("d_yt", [P, HG, T], F32, kind="ExternalOutput")

    with tile.TileContext(nc) as tc, ExitStack() as ctx:
        singles = ctx.enter_context(tc.tile_pool(name="singles", bufs=1))
        # bufs must cover the V-lag window (xtile(t) is re-read by the lagged
        # V projection at iteration t+VLAG); the pool closes with phase A
        phase_a_pools = ExitStack()
        xpool = phase_a_pools.enter_context(tc.tile_pool(name="xa", bufs=8))

        # ---- prefetch the first x tile before the weight bulk so the PE
        # can start within a few us ----
        # ---- resident tensors ----
        # weight DMAs issued per-co round-robin over both HWDGE queues so
        # early co chunks land in consumption order and issue rate isn't
        # limited by one sequencer (~600ns per dma_start).  The first x
        # chunk + first co weights go out first so the PE starts ASAP.
        wq_sb = singles.tile([P, C // P, QC], BF16)
        wkv_sb = singles.tile([P, C // P, 2 * KC], BF16)
        wqr = wq.rearrange("(co p) q -> p co q", p=P)
        wkvr = wkv.rearrange("(co p) q -> p co q", p=P)
        cos_sb = singles.tile([P, TOKCH, D // 2], F32)
        sin_sb = singles.tile([P, TOKCH, D // 2], F32)
        # DMAs ordered by first consumption: iteration 0 runs three column
        # passes (q0 cols 0:512, q1 cols 512:1024, k) so its first rope --
        # the start of the 160us serialized DVE chain that bounds phase A --
        # only needs the q0 half of wq plus xtile0.
        xtile0 = xpool.tile([P, C // P, P], BF16, tag="xt")
        nc.sync.dma_start(xtile0[:, 0:4, :], xt[0, :, 0:4, :])
        qs = [nc.sync, nc.scalar]
        # k half first: only 0.5MB + the first x chunks gate the k pass, so
        # the serialized DVE rope chain (phase A's bound) starts ~3.5us in
        for i, co in enumerate(range(0, C // P, 4)):
            qs[i % 2].dma_start(wkv_sb[:, co:co + 4, 0:KC],
                                wkvr[:, co:co + 4, 0:KC])
        for g4 in range(1, 4):
            nc.sync.dma_start(xtile0[:, 4 * g4:4 * (g4 + 1), :],
                              xt[0, :, 4 * g4:4 * (g4 + 1), :])
        nc.scalar.dma_start(cos_sb, cosd[:])
        nc.scalar.dma_start(sin_sb, sind[:])
        for i, co in enumerate(range(0, C // P, 2)):
            qs[i % 2].dma_start(wq_sb[:, co:co + 2, 0:512],
                                wqr[:, co:co + 2, 0:512])
        for i, co in enumerate(range(0, C // P, 2)):
            qs[i % 2].dma_start(wq_sb[:, co:co + 2, 512:1024],
                                wqr[:, co:co + 2, 512:1024])
        # V weights stream last (first consumed at iteration VLAG, ~70us in)
        for co in range(0, C // P, 4):
            nc.gpsimd.dma_start(wkv_sb[:, co:co + 4, KC:2 * KC],
                                wkvr[:, co:co + 4, KC:2 * KC])

        ident = singles.tile([P, P], BF16)
        make_identity(nc, ident)
        ones_col = singles.tile([P, 1], BF16)
        nc.vector.memset(ones_col, 1.0)
        zero_col = singles.tile([P, 1], F32)
        nc.vector.memset(zero_col, 0.0)
        eps_col = singles.tile([P, 1], F32)
        nc.vector.memset(eps_col, EPS)
        nc.const_aps.aps[(F32, 0.0)] = zero_col[:]
        nc.const_aps.aps[(F32, EPS)] = eps_col[:]
        # scratch for the dummy exp that prewarms the exp act-table at the
        # A->B phase boundary (overlaps the 1.28us table load)
        warm = singles.tile([1, 1], F32)

        # diagonal-block mask: keep where i >= j (j = tk partition, i = tq
        # free).  bf16 so it can be ADDED into the scores psum by a 128-col
        # matmul (ident.T @ mask) instead of a DVE op in the exp chain.
        mask_sb = singles.tile([P, P], BF16)
        nc.vector.memset(mask_sb, 0.0)
        nc.gpsimd.affine_select(
            out=mask_sb, in_=mask_sb,
            compare_op=mybir.AluOpType.is_ge, fill=NEG,
            base=0, pattern=[[1, P]], channel_multiplier=-1,
        )

        qT = singles.tile([P, HG, T], BF16)      # [d, h, tok]
        kT = singles.tile([P, KVG, T], BF16)
        v_sb = singles.tile([P, TOKCH, KC], BF16)  # [tok%128, chunk, vcol]
        yT = singles.tile([P, HG, T], BF16)

        # ================= phase A: QKV proj + RoPE + qk-norm =============
        if "A" not in PHASES:
            pass
        else:
         with phase_a_pools, \
             tc.tile_pool(name="pa", bufs=2, space="PSUM") as pps, \
             tc.tile_pool(name="pkv", bufs=1, space="PSUM") as pkv, \
             tc.tile_pool(name="sa", bufs=3) as spool:
            # The V projection is split out of the QK pass and lagged by VLAG
            # chunks: the final VLAG V-chunks are pure PE work that runs while
            # the last rope chains (DVE) drain, so phase B starts without
            # waiting on the phase-A tail.
            VLAG = 6
            nco = C // P
            xtiles = {}
            # transposes lag one iteration behind their rope chain so they
            # never sit dep-blocked in the PE's 4-deep wait queue
            pending_tr = []

            def flush_trs():
                while pending_tr:
                    qbf, dstT, h0, nh, tt = pending_tr.pop(0)
                    pst = pkv.tile([P, 4, P], BF16, tag="tr")
                    for i in range(nh):
                        nc.tensor.transpose(pst[:, i, :], qbf[:, i, :], ident)
                    nc.scalar.copy(
                        dstT[:, h0:h0 + nh, tt * P:(tt + 1) * P], pst[:, 0:nh, :])

            def v_chunk(tv):
                xv = xtiles.pop(tv)
                # alternate psum tags so consecutive V chunks don't serialize
                # on one buffer's Act-copy release
                ps_v = pkv.tile([P, KC], F32, tag=("v" if tv % 2 == 0 else "v2"))
                for co in range(nco):
                    nc.tensor.matmul(ps_v, xv[:, co, :],
                                     wkv_sb[:, co, KC:2 * KC],
                                     start=(co == 0), stop=(co == nco - 1))
                # cast straight to resident token-major buffer (Act engine;
                # DVE is the critical engine in this phase)
                nc.scalar.copy(v_sb[:, tv, :], ps_v)

            for t in range(TOKCH):
                if t == 0:
                    xtile = xtile0
                else:
                    xtile = xpool.tile([P, C // P, P], BF16, tag="xt")
                    nc.sync.dma_start(xtile, xt[t])
                xtiles[t] = xtile
                ps_q0 = pps.tile([P, 512], F32, tag="q0")
                ps_q1 = pps.tile([P, 512], F32, tag="q1")
                ps_k = pkv.tile([P, KC], F32, tag="k")

                # Q/K: fused multi-head rope + rms-norm + cast + transpose
                def rope_norm(ps, nh, dstT, h0, qscale, rsq_dve=False):
                    h2 = D // 2
                    v4 = ps.rearrange("p (h a d) -> p h a d", h=nh, a=2)
                    q1, q2 = v4[:, :, 0, :], v4[:, :, 1, :]
                    r = spool.tile([P, nh, 2, h2], F32, tag=f"rope{nh}")
                    r1, r2 = r[:, :, 0, :], r[:, :, 1, :]
                    s2 = spool.tile([P, nh, h2], F32, tag=f"scr{nh}")
                    cs = cos_sb[:, t, None, :].to_broadcast([P, nh, h2])
                    sn = sin_sb[:, t, None, :].to_broadcast([P, nh, h2])
                    nc.vector.tensor_mul(r1, q1, cs)
                    nc.vector.tensor_mul(s2, q2, sn)
                    nc.vector.tensor_sub(r1, r1, s2)
                    nc.vector.tensor_mul(r2, q1, sn)
                    nc.vector.tensor_mul(s2, q2, cs)
                    nc.vector.tensor_add(r2, r2, s2)
                    rf = r.rearrange("p h a d -> p h (a d)")
                    sq = spool.tile([P, nh, D], F32, tag=f"sq{nh}")
                    ss = spool.tile([P, nh], F32, tag=f"ss{nh}")
                    if rsq_dve:
                        # keep the last iteration's rope entirely off the Act
                        # engine so phase B's first exps aren't queued behind it
                        nc.vector.tensor_mul(sq, rf, rf)
                    else:
                        nc.scalar.activation(sq, rf, AF.Square)
                    nc.vector.tensor_reduce(ss, sq, axis=mybir.AxisListType.X,
                                            op=mybir.AluOpType.add)
                    rq = spool.tile([P, nh], F32, tag=f"rq{nh}")
                    if rsq_dve:
                        # DVE-only fast inverse sqrt (bit trick + 2 Newton
                        # steps, qscale folded into the last).  Used for the
                        # final token chunk so the previous iteration's Sqrt
                        # is the Act engine's last sqrt-set op and the exp
                        # table load hides behind the V tail.
                        ALU = mybir.AluOpType
                        I32 = mybir.dt.int32
                        fx = spool.tile([P, nh], F32, tag=f"fx{nh}")
                        nc.vector.tensor_scalar(fx, ss, 1.0 / D, EPS,
                                                op0=ALU.mult, op1=ALU.add)
                        fj = spool.tile([P, nh], I32, tag=f"fj{nh}")
                        nc.vector.tensor_scalar(fj, fx[:].bitcast(I32), 1, None,
                                                op0=ALU.logical_shift_right)
                        nc.vector.tensor_scalar(fj, fj, -1, 0x5f3759df + 1,
                                                op0=ALU.bitwise_xor, op1=ALU.add)
                        fy = fj[:].bitcast(F32)
                        fa = spool.tile([P, nh], F32, tag=f"fa{nh}")
                        nc.vector.tensor_mul(fa, fy, fy)
                        nc.vector.tensor_mul(fa, fa, fx)
                        nc.vector.tensor_scalar(fa, fa, -0.5, 1.5,
                                                op0=ALU.mult, op1=ALU.add)
                        nc.vector.tensor_mul(rq, fy, fa)
                        nc.vector.tensor_mul(fa, rq, rq)
                        nc.vector.tensor_mul(fa, fa, fx)
                        nc.vector.tensor_scalar(fa, fa, -0.5 * qscale,
                                                1.5 * qscale,
                                                op0=ALU.mult, op1=ALU.add)
                        nc.vector.tensor_mul(rq, rq, fa)
                    else:
                        rt = spool.tile([P, nh], F32, tag=f"rt{nh}")
                        nc.scalar.activation(rt, ss, AF.Sqrt, scale=1.0 / D,
                                             bias=EPS)
                        nc.vector.reciprocal(rq, rt)
                        if qscale != 1.0:
                            nc.vector.tensor_scalar_mul(rq, rq, qscale)
                    qbf = spool.tile([P, nh, D], BF16, tag=f"qbf{nh}")
                    nc.vector.tensor_mul(qbf, rf, rq[:, :, None].to_broadcast([P, nh, D]))
                    pending_tr.append((qbf, dstT, h0, nh, t))

                qsc = 1.0 / float(np.sqrt(D))
                for co in range(nco):
                    lhsT = xtile[:, co, :]
                    st = dict(start=(co == 0), stop=(co == nco - 1))
                    nc.tensor.matmul(ps_q0, lhsT, wq_sb[:, co, 0:512], **st)
                    nc.tensor.matmul(ps_q1, lhsT, wq_sb[:, co, 512:1024], **st)
                    nc.tensor.matmul(ps_k, lhsT, wkv_sb[:, co, 0:KC], **st)
                rope_norm(ps_q0, 4, qT, 0, qsc)
                rope_norm(ps_q1, 4, qT, 4, qsc)
                rope_norm(ps_k, KVG, kT, 0, 1.0)
                if t >= VLAG:
                    v_chunk(t - VLAG)
                # previous iteration's transposes: rope chains long done
                flush_trs()
                if t == TOKCH - 1:
                    # prewarm the exp act-table; the 1.28us load runs behind
                    # the V tail
                    nc.scalar.activation(warm, zero_col[0:1, :], AF.Exp)

            # lagged V tail: pure PE work that covers the final rope chains;
            # the last transposes flush once their rope chain has had V cover
            for tv in range(TOKCH - VLAG, TOKCH):
                v_chunk(tv)
                if tv == TOKCH - 2:
                    flush_trs()

        # ================= phase B: attention ============================
        # wp prefetch: issue at phase-B start so the tiles are resident long
        # before phase C begins (phase-A pools have closed, SBUF is free)
        wpool = ctx.enter_context(tc.tile_pool(name="wp", bufs=1))
        wpr = wp.rearrange("(hc p) c -> p hc c", p=P)
        wp_ts = []
        for ct in range(C // 512):
            wp_t = wpool.tile([P, HG, 512], BF16, tag=f"wpt{ct}")
            nc.sync.dma_start(wp_t, wpr[:, :, ct * 512:(ct + 1) * 512])
            wp_ts.append(wp_t)

        if "B" not in PHASES:
            pass
        else:
         with tc.tile_pool(name="psc", bufs=4, space="PSUM") as psc, \
             tc.tile_pool(name="psy", bufs=2, space="PSUM") as psy, \
             tc.tile_pool(name="pss", bufs=2, space="PSUM") as pss, \
             tc.tile_pool(name="pb", bufs=6) as ppool, \
             tc.tile_pool(name="sb", bufs=3) as bpool:
            NT = T // 512  # 4 tq tiles
            # software pipeline: the PE queue is in-order, so scores for
            # chunk idx+DEPTH are emitted before pv/ones of chunk idx; the
            # scores->mask->exp chain (~1.6us) hides behind DEPTH chunks of
            # PE work.  The (h, c) stream is flattened so the pipeline also
            # covers head boundaries.
            DEPTH = 4
            for t in range(NT):
                nch = 4 * (t + 1)
                items = [(h, c) for h in range(HG) for c in range(nch)]
                live = {}

                def front(idx):
                    h, c = items[idx]
                    g = h // NREP
                    o = c * P - t * 512
                    col0 = max(o, 0)
                    ps_sc = psc.tile([P, 512], F32, tag="sc")
                    nc.tensor.matmul(
                        ps_sc[:, col0:512], kT[:, g, c * P:(c + 1) * P],
                        qT[:, h, t * 512 + col0:(t + 1) * 512],
                        start=True, stop=(o < 0))
                    if o >= 0:
                        # after the col0 shift the partial block is always the
                        # i' >= j triangle; accumulate the additive mask with
                        # a 128-col matmul (53ns) right behind the scores
                        nc.tensor.matmul(ps_sc[:, col0:col0 + P], ident,
                                         mask_sb, start=False, stop=True)
                    pt = ppool.tile([P, 512], BF16, tag="pt")
                    nc.scalar.activation(pt[:, col0:512], ps_sc[:, col0:512],
                                         AF.Exp)
                    live[idx] = (pt, col0)

                for i in range(min(DEPTH, len(items))):
                    front(i)
                ys = {}
                for idx, (h, c) in enumerate(items):
                    if idx + DEPTH < len(items):
                        front(idx + DEPTH)
                    g = h // NREP
                    if c == 0:
                        ps_y = psy.tile([P, 512], F32, tag="y")
                        ps_sden = pss.tile([P, 512], F32, tag="sden")
                        ys[h] = (ps_y, ps_sden)
                    ps_y, ps_sden = ys[h]
                    ps_s = ps_sden[0:1, :]
                    pt, col0 = live.pop(idx)
                    st = dict(start=(c == 0), stop=(c == nch - 1))
                    nc.tensor.matmul(ps_y[:, col0:512],
                                     v_sb[:, c, g * P:(g + 1) * P],
                                     pt[:, col0:512], **st)
                    nc.tensor.matmul(ps_s[:, col0:512], ones_col,
                                     pt[:, col0:512], **st)
                    if c == nch - 1:
                        # recip first (frees the single pss buffer fastest),
                        # then copy (frees ps_y); normalize the bf16 slice in
                        # place on the Pool engine once the broadcast lands
                        yslice = yT[:, h, t * 512:(t + 1) * 512]
                        rc = bpool.tile([1, 512], F32, tag="rc")
                        nc.vector.reciprocal(rc, ps_s)
                        nc.vector.tensor_copy(yslice, ps_y)
                        rb = bpool.tile([P, 512], F32, tag="rb")
                        nc.gpsimd.partition_broadcast(rb, rc, channels=P)
                        nc.vector.tensor_mul(yslice, yslice, rb)

        if DEBUG_DUMP:
            with tc.tile_pool(name="dbg", bufs=2) as dpool:
                for h in range(HG):
                    dt_ = dpool.tile([P, T], F32, tag="d")
                    nc.vector.tensor_copy(dt_, qT[:, h, :])
                    nc.sync.dma_start(d_qt[:, h, :], dt_)
                    dt_ = dpool.tile([P, T], F32, tag="d")
                    nc.vector.tensor_copy(dt_, yT[:, h, :])
                    nc.sync.dma_start(d_yt[:, h, :], dt_)
                for g in range(KVG):
                    dt_ = dpool.tile([P, T], F32, tag="d")
                    nc.vector.tensor_copy(dt_, kT[:, g, :])
                    nc.sync.dma_start(d_kt[:, g, :], dt_)
                dt_ = dpool.tile([P, TOKCH * KC], F32, tag="d")
                nc.vector.tensor_copy(dt_.rearrange("p (a b) -> p a b", a=TOKCH), v_sb[:, :, :])
                nc.sync.dma_start(d_v[:, :, :], dt_.rearrange("p (a b) -> p a b", a=TOKCH))

        # ================= phase C: output projection =====================
        if "C" not in PHASES:
            pass
        else:
         with tc.tile_pool(name="po", bufs=2, space="PSUM") as pso, \
             tc.tile_pool(name="so", bufs=3) as opool:
            for t in range(TOKCH):
                for ct in range(C // 512):
                    ps_o = pso.tile([P, 512], F32, tag="o")
                    last = (t == TOKCH - 1 and ct == C // 512 - 1)
                    ob = opool.tile([P, 512], F32, tag="ob")
                    if not last:
                        for hc in range(HG):
                            nc.tensor.matmul(
                                ps_o, yT[:, hc, t * P:(t + 1) * P],
                                wp_ts[ct][:, hc, :],
                                start=(hc == 0), stop=(hc == HG - 1))
                        nc.vector.tensor_copy(ob, ps_o)
                        nc.sync.dma_start(
                            out[t * P:(t + 1) * P, ct * 512:(ct + 1) * 512], ob)
                    else:
                        # final tile in two pipelined halves to shorten the
                        # copy->dma drain tail; halves go out on different
                        # queues so the transfers overlap
                        for q in range(2):
                            cs = slice(q * 256, (q + 1) * 256)
                            for hc in range(HG):
                                nc.tensor.matmul(
                                    ps_o[:, cs], yT[:, hc, t * P:(t + 1) * P],
                                    wp_ts[ct][:, hc, cs],
                                    start=(hc == 0), stop=(hc == HG - 1))
                            (nc.vector.tensor_copy if q == 0
                             else nc.scalar.copy)(ob[:, cs], ps_o[:, cs])
                            (nc.sync if q == 0 else nc.scalar).dma_start(
                                out[t * P:(t + 1) * P,
                                    ct * 512 + q * 256:ct * 512 + (q + 1) * 256],
                                ob[:, cs])
    nc.compile()
    return nc


_NC_CACHE = []


def _get_prog():
    if not _NC_CACHE:
        _NC_CACHE.append(_build())
    return _NC_CACHE[0]


def _make_in_maps(inputs):
    x, cos, sin = inputs["x"], inputs["cos"], inputs["sin"]
    wq, wk, wv, wproj = inputs["wq"], inputs["wk"], inputs["wv"], inputs["wproj"]
    bf = ml_dtypes.bfloat16
    # [p, tc, d] tiling (contiguous 4KB DMA rows)
    cos2 = np.ascontiguousarray(
        cos.reshape(TOKCH, P, D // 2).transpose(1, 0, 2), dtype=np.float32)
    sin2 = np.ascontiguousarray(
        sin.reshape(TOKCH, P, D // 2).transpose(1, 0, 2), dtype=np.float32)
    in_maps = []
    for core in range(8):
        b, g = core // 2, core % 2
        qs = slice(g * QC, (g + 1) * QC)
        ks = slice(g * KC, (g + 1) * KC)
        # x[b].T is [C, T]; tile to [tokch, p(C-chunk), co, tk]
        xtb = (x[b].T.astype(bf)
               .reshape(C // P, P, TOKCH, P)     # [co, p, tc, tk]
               .transpose(2, 1, 0, 3))           # [tc, p, co, tk]
        in_maps.append({
            "xt": np.ascontiguousarray(xtb),
            "wq": np.ascontiguousarray(wq[:, qs]).astype(bf),
            "wkv": np.ascontiguousarray(np.hstack([wk[:, ks], wv[:, ks]])).astype(bf),
            "wp": np.ascontiguousarray(wproj[qs, :]).astype(bf),
            "cosd": cos2,
            "sind": sin2,
        })
    return in_maps


def kernel(x, cos, sin, wq, wk, wv, wproj):
    nc = _get_prog()
    in_maps = _make_in_maps(dict(x=x, cos=cos, sin=sin, wq=wq, wk=wk, wv=wv, wproj=wproj))
    res = run_bass_kernel_spmd(nc, in_maps, core_ids=list(range(8))).results
    outp = np.empty((B, T, C), np.float32)
    for b in range(B):
        outp[b] = res[2 * b]["out"] + res[2 * b + 1]["out"]
    return outp



# revision 102
# speedup vs baseline: 1.2038x; 1.0013x over previous
"""Causal self-attention (GQA + RoPE + QK-norm) Trainium2 Bass kernel.

Sharding: 8 cores = 4 batches x 2 head-groups.  Core c -> batch c//2,
q heads (c%2)*8..+8, kv heads (c%2)*2..+2.  wproj is row-sharded, so each
core emits a partial (T, C) output; the host sums the two partials per batch.

Device-side layout strategy (per core):
  - x is fed pre-transposed (xT, [C, T]) and bf16-cast by the host.
  - QKV projections produce Q,K token-major ([tok, cols]); RoPE + rms-norm
    run token-major (free-axis per-head reductions), then 128x128 PE
    transposes produce qT/kT feature-major for the attention matmuls.
    V is produced token-major, which is exactly the p@v stationary layout.
  - scores are computed transposed (scoresT[tk, tq]) so that after exp the
    p tiles are already the moving operand for the p@v matmul; the softmax
    denominator comes from a ones-column matmul accumulated in PSUM.
  - exp has no max-subtraction: qk-norm bounds |s| <= sqrt(128) ~ 11.32.
  - output projection accumulates over the 8 local heads; partial written
    fp32 to DRAM.
"""

import numpy as np
import ml_dtypes
from contextlib import ExitStack

import concourse.bass as bass
import concourse.mybir as mybir
import concourse.tile as tile
from concourse import bacc
from concourse.bass_utils import run_bass_kernel_spmd
from concourse.masks import make_identity

BF16 = mybir.dt.bfloat16
F32 = mybir.dt.float32
F32R = mybir.dt.float32r
AF = mybir.ActivationFunctionType

B, T, C = 4, 2048, 2048
H, KV, D = 16, 4, 128
HG, KVG = H // 2, KV // 2          # per-core q heads (8), kv heads (2)
QC, KC = HG * D, KVG * D           # 1024, 256
P = 128
TOKCH = T // P                     # 16 token chunks
NREP = H // KV                     # 4
EPS = 1e-5
NEG = -1.0e5                       # additive causal mask (exp -> 0)


DEBUG_DUMP = False
PHASES = ("A", "B", "C")


def _build():
    nc = bacc.Bacc("TRN2", target_bir_lowering=False, debug=False, num_devices=8)
    # x pre-tiled by the host as [tokch, p, co, tk] so every DMA partition row
    # is 4KB contiguous (co*tk*2B) instead of 256B strided
    xt = nc.dram_tensor("xt", [TOKCH, P, C // P, P], BF16, kind="ExternalInput")
    wq = nc.dram_tensor("wq", [C, QC], BF16, kind="ExternalInput")
    wkv = nc.dram_tensor("wkv", [C, 2 * KC], BF16, kind="ExternalInput")
    wp = nc.dram_tensor("wp", [QC, C], BF16, kind="ExternalInput")
    # cos/sin pre-tiled by host as [p, tc, d] (contiguous 4KB rows)
    cosd = nc.dram_tensor("cosd", [P, TOKCH, D // 2], F32, kind="ExternalInput")
    sind = nc.dram_tensor("sind", [P, TOKCH, D // 2], F32, kind="ExternalInput")
    out = nc.dram_tensor("out", [T, C], F32, kind="ExternalOutput")
    if DEBUG_DUMP:
        d_qt = nc.dram_tensor("d_qt", [P, HG, T], F32, kind="ExternalOutput")
        d_kt = nc.dram_tensor("d_kt", [P, KVG, T], F32, kind="ExternalOutput")
        d_v = nc.dram_tensor("d_v", [P, TOKCH, KC], F32, kind="ExternalOutput")
        d_yt = nc.dram_tensor("d_yt", [P, HG, T], F32, kind="ExternalOutput")

    with tile.TileContext(nc) as tc, ExitStack() as ctx:
        singles = ctx.enter_context(tc.tile_pool(name="singles", bufs=1))
        # bufs must cover the V-lag window (xtile(t) is re-read by the lagged
        # V projection at iteration t+VLAG); the pool closes with phase A
        phase_a_pools = ExitStack()
        xpool = phase_a_pools.enter_context(tc.tile_pool(name="xa", bufs=8))

        # ---- prefetch the first x tile before the weight bulk so the PE
        # can start within a few us ----
        # ---- resident tensors ----
        # weight DMAs issued per-co round-robin over both HWDGE queues so
        # early co chunks land in consumption order and issue rate isn't
        # limited by one sequencer (~600ns per dma_start).  The first x
        # chunk + first co weights go out first so the PE starts ASAP.
        wq_sb = singles.tile([P, C // P, QC], BF16)
        wkv_sb = singles.tile([P, C // P, 2 * KC], BF16)
        wqr = wq.rearrange("(co p) q -> p co q", p=P)
        wkvr = wkv.rearrange("(co p) q -> p co q", p=P)
        cos_sb = singles.tile([P, TOKCH, D // 2], F32)
        sin_sb = singles.tile([P, TOKCH, D // 2], F32)
        # DMAs ordered by first consumption: iteration 0 runs three column
        # passes (q0 cols 0:512, q1 cols 512:1024, k) so its first rope --
        # the start of the 160us serialized DVE chain that bounds phase A --
        # only needs the q0 half of wq plus xtile0.
        xtile0 = xpool.tile([P, C // P, P], BF16, tag="xt")
        nc.sync.dma_start(xtile0[:, 0:4, :], xt[0, :, 0:4, :])
        qs = [nc.sync, nc.scalar]
        # first 2 co's full weight needs, then the k half (it plus the first
        # x chunks gates the k rope, the start of the serialized DVE chain)
        nc.scalar.dma_start(wq_sb[:, 0:2, 0:512], wqr[:, 0:2, 0:512])
        nc.sync.dma_start(wq_sb[:, 0:2, 512:1024], wqr[:, 0:2, 512:1024])
        for i, co in enumerate(range(0, C // P, 4)):
            qs[i % 2].dma_start(wkv_sb[:, co:co + 4, 0:KC],
                                wkvr[:, co:co + 4, 0:KC])
        for g4 in range(1, 4):
            nc.sync.dma_start(xtile0[:, 4 * g4:4 * (g4 + 1), :],
                              xt[0, :, 4 * g4:4 * (g4 + 1), :])
        nc.scalar.dma_start(cos_sb, cosd[:])
        nc.scalar.dma_start(sin_sb, sind[:])
        for i, co in enumerate(range(2, C // P, 2)):
            qs[i % 2].dma_start(wq_sb[:, co:co + 2, 0:512],
                                wqr[:, co:co + 2, 0:512])
        for i, co in enumerate(range(2, C // P, 2)):
            qs[i % 2].dma_start(wq_sb[:, co:co + 2, 512:1024],
                                wqr[:, co:co + 2, 512:1024])
        # V weights stream last (first consumed at iteration VLAG, ~70us in)
        for co in range(0, C // P, 4):
            nc.gpsimd.dma_start(wkv_sb[:, co:co + 4, KC:2 * KC],
                                wkvr[:, co:co + 4, KC:2 * KC])

        ident = singles.tile([P, P], BF16)
        make_identity(nc, ident)
        ones_col = singles.tile([P, 1], BF16)
        nc.vector.memset(ones_col, 1.0)
        zero_col = singles.tile([P, 1], F32)
        nc.vector.memset(zero_col, 0.0)
        eps_col = singles.tile([P, 1], F32)
        nc.vector.memset(eps_col, EPS)
        nc.const_aps.aps[(F32, 0.0)] = zero_col[:]
        nc.const_aps.aps[(F32, EPS)] = eps_col[:]
        # scratch for the dummy exp that prewarms the exp act-table at the
        # A->B phase boundary (overlaps the 1.28us table load)
        warm = singles.tile([1, 1], F32)

        # diagonal-block mask: keep where i >= j (j = tk partition, i = tq
        # free).  bf16 so it can be ADDED into the scores psum by a 128-col
        # matmul (ident.T @ mask) instead of a DVE op in the exp chain.
        mask_sb = singles.tile([P, P], BF16)
        nc.vector.memset(mask_sb, 0.0)
        nc.gpsimd.affine_select(
            out=mask_sb, in_=mask_sb,
            compare_op=mybir.AluOpType.is_ge, fill=NEG,
            base=0, pattern=[[1, P]], channel_multiplier=-1,
        )

        qT = singles.tile([P, HG, T], BF16)      # [d, h, tok]
        kT = singles.tile([P, KVG, T], BF16)
        v_sb = singles.tile([P, TOKCH, KC], BF16)  # [tok%128, chunk, vcol]
        yT = singles.tile([P, HG, T], BF16)

        # ================= phase A: QKV proj + RoPE + qk-norm =============
        if "A" not in PHASES:
            pass
        else:
         with phase_a_pools, \
             tc.tile_pool(name="pa", bufs=2, space="PSUM") as pps, \
             tc.tile_pool(name="pkv", bufs=1, space="PSUM") as pkv, \
             tc.tile_pool(name="sa", bufs=3) as spool:
            # The V projection is split out of the QK pass and lagged by VLAG
            # chunks: the final VLAG V-chunks are pure PE work that runs while
            # the last rope chains (DVE) drain, so phase B starts without
            # waiting on the phase-A tail.
            VLAG = 6
            nco = C // P
            xtiles = {}
            # transposes lag one iteration behind their rope chain so they
            # never sit dep-blocked in the PE's 4-deep wait queue
            pending_tr = []

            def flush_trs():
                while pending_tr:
                    qbf, dstT, h0, nh, tt = pending_tr.pop(0)
                    pst = pkv.tile([P, 4, P], BF16, tag="tr")
                    for i in range(nh):
                        nc.tensor.transpose(pst[:, i, :], qbf[:, i, :], ident)
                    nc.scalar.copy(
                        dstT[:, h0:h0 + nh, tt * P:(tt + 1) * P], pst[:, 0:nh, :])

            def v_chunk(tv):
                xv = xtiles.pop(tv)
                # alternate psum tags so consecutive V chunks don't serialize
                # on one buffer's Act-copy release
                ps_v = pkv.tile([P, KC], F32, tag=("v" if tv % 2 == 0 else "v2"))
                for co in range(nco):
                    nc.tensor.matmul(ps_v, xv[:, co, :],
                                     wkv_sb[:, co, KC:2 * KC],
                                     start=(co == 0), stop=(co == nco - 1))
                # cast straight to resident token-major buffer (Act engine;
                # DVE is the critical engine in this phase)
                nc.scalar.copy(v_sb[:, tv, :], ps_v)

            for t in range(TOKCH):
                if t == 0:
                    xtile = xtile0
                else:
                    xtile = xpool.tile([P, C // P, P], BF16, tag="xt")
                    nc.sync.dma_start(xtile, xt[t])
                xtiles[t] = xtile
                ps_q0 = pps.tile([P, 512], F32, tag="q0")
                ps_q1 = pps.tile([P, 512], F32, tag="q1")
                ps_k = pkv.tile([P, KC], F32, tag="k")

                # Q/K: fused multi-head rope + rms-norm + cast + transpose
                def rope_norm(ps, nh, dstT, h0, qscale, rsq_dve=False):
                    h2 = D // 2
                    v4 = ps.rearrange("p (h a d) -> p h a d", h=nh, a=2)
                    q1, q2 = v4[:, :, 0, :], v4[:, :, 1, :]
                    r = spool.tile([P, nh, 2, h2], F32, tag=f"rope{nh}")
                    r1, r2 = r[:, :, 0, :], r[:, :, 1, :]
                    s2 = spool.tile([P, nh, h2], F32, tag=f"scr{nh}")
                    cs = cos_sb[:, t, None, :].to_broadcast([P, nh, h2])
                    sn = sin_sb[:, t, None, :].to_broadcast([P, nh, h2])
                    nc.vector.tensor_mul(r1, q1, cs)
                    nc.vector.tensor_mul(s2, q2, sn)
                    nc.vector.tensor_sub(r1, r1, s2)
                    nc.vector.tensor_mul(r2, q1, sn)
                    nc.vector.tensor_mul(s2, q2, cs)
                    nc.vector.tensor_add(r2, r2, s2)
                    rf = r.rearrange("p h a d -> p h (a d)")
                    sq = spool.tile([P, nh, D], F32, tag=f"sq{nh}")
                    ss = spool.tile([P, nh], F32, tag=f"ss{nh}")
                    if rsq_dve:
                        # keep the last iteration's rope entirely off the Act
                        # engine so phase B's first exps aren't queued behind it
                        nc.vector.tensor_mul(sq, rf, rf)
                    else:
                        nc.scalar.activation(sq, rf, AF.Square)
                    nc.vector.tensor_reduce(ss, sq, axis=mybir.AxisListType.X,
                                            op=mybir.AluOpType.add)
                    rq = spool.tile([P, nh], F32, tag=f"rq{nh}")
                    if rsq_dve:
                        # DVE-only fast inverse sqrt (bit trick + 2 Newton
                        # steps, qscale folded into the last).  Used for the
                        # final token chunk so the previous iteration's Sqrt
                        # is the Act engine's last sqrt-set op and the exp
                        # table load hides behind the V tail.
                        ALU = mybir.AluOpType
                        I32 = mybir.dt.int32
                        fx = spool.tile([P, nh], F32, tag=f"fx{nh}")
                        nc.vector.tensor_scalar(fx, ss, 1.0 / D, EPS,
                                                op0=ALU.mult, op1=ALU.add)
                        fj = spool.tile([P, nh], I32, tag=f"fj{nh}")
                        nc.vector.tensor_scalar(fj, fx[:].bitcast(I32), 1, None,
                                                op0=ALU.logical_shift_right)
                        nc.vector.tensor_scalar(fj, fj, -1, 0x5f3759df + 1,
                                                op0=ALU.bitwise_xor, op1=ALU.add)
                        fy = fj[:].bitcast(F32)
                        fa = spool.tile([P, nh], F32, tag=f"fa{nh}")
                        nc.vector.tensor_mul(fa, fy, fy)
                        nc.vector.tensor_mul(fa, fa, fx)
                        nc.vector.tensor_scalar(fa, fa, -0.5, 1.5,
                                                op0=ALU.mult, op1=ALU.add)
                        nc.vector.tensor_mul(rq, fy, fa)
                        nc.vector.tensor_mul(fa, rq, rq)
                        nc.vector.tensor_mul(fa, fa, fx)
                        nc.vector.tensor_scalar(fa, fa, -0.5 * qscale,
                                                1.5 * qscale,
                                                op0=ALU.mult, op1=ALU.add)
                        nc.vector.tensor_mul(rq, rq, fa)
                    else:
                        rt = spool.tile([P, nh], F32, tag=f"rt{nh}")
                        nc.scalar.activation(rt, ss, AF.Sqrt, scale=1.0 / D,
                                             bias=EPS)
                        nc.vector.reciprocal(rq, rt)
                        if qscale != 1.0:
                            nc.vector.tensor_scalar_mul(rq, rq, qscale)
                    qbf = spool.tile([P, nh, D], BF16, tag=f"qbf{nh}")
                    nc.vector.tensor_mul(qbf, rf, rq[:, :, None].to_broadcast([P, nh, D]))
                    pending_tr.append((qbf, dstT, h0, nh, t))

                qsc = 1.0 / float(np.sqrt(D))
                for co in range(nco):
                    lhsT = xtile[:, co, :]
                    st = dict(start=(co == 0), stop=(co == nco - 1))
                    nc.tensor.matmul(ps_q0, lhsT, wq_sb[:, co, 0:512], **st)
                    nc.tensor.matmul(ps_q1, lhsT, wq_sb[:, co, 512:1024], **st)
                    nc.tensor.matmul(ps_k, lhsT, wkv_sb[:, co, 0:KC], **st)
                rope_norm(ps_q0, 4, qT, 0, qsc)
                rope_norm(ps_q1, 4, qT, 4, qsc)
                rope_norm(ps_k, KVG, kT, 0, 1.0)
                if t >= VLAG:
                    v_chunk(t - VLAG)
                # previous iteration's transposes: rope chains long done
                flush_trs()
                if t == TOKCH - 1:
                    # prewarm the exp act-table; the 1.28us load runs behind
                    # the V tail
                    nc.scalar.activation(warm, zero_col[0:1, :], AF.Exp)

            # lagged V tail: pure PE work that covers the final rope chains;
            # the last transposes flush once their rope chain has had V cover
            for tv in range(TOKCH - VLAG, TOKCH):
                v_chunk(tv)
                if tv == TOKCH - 2:
                    flush_trs()

        # ================= phase B: attention ============================
        # wp prefetch: issue at phase-B start so the tiles are resident long
        # before phase C begins (phase-A pools have closed, SBUF is free)
        wpool = ctx.enter_context(tc.tile_pool(name="wp", bufs=1))
        wpr = wp.rearrange("(hc p) c -> p hc c", p=P)
        wp_ts = []
        for ct in range(C // 512):
            wp_t = wpool.tile([P, HG, 512], BF16, tag=f"wpt{ct}")
            nc.sync.dma_start(wp_t, wpr[:, :, ct * 512:(ct + 1) * 512])
            wp_ts.append(wp_t)

        if "B" not in PHASES:
            pass
        else:
         with tc.tile_pool(name="psc", bufs=4, space="PSUM") as psc, \
             tc.tile_pool(name="psy", bufs=2, space="PSUM") as psy, \
             tc.tile_pool(name="pss", bufs=2, space="PSUM") as pss, \
             tc.tile_pool(name="pb", bufs=6) as ppool, \
             tc.tile_pool(name="sb", bufs=3) as bpool:
            NT = T // 512  # 4 tq tiles
            # software pipeline: the PE queue is in-order, so scores for
            # chunk idx+DEPTH are emitted before pv/ones of chunk idx; the
            # scores->mask->exp chain (~1.6us) hides behind DEPTH chunks of
            # PE work.  The (h, c) stream is flattened so the pipeline also
            # covers head boundaries.
            DEPTH = 4
            for t in range(NT):
                nch = 4 * (t + 1)
                items = [(h, c) for h in range(HG) for c in range(nch)]
                live = {}

                def front(idx):
                    h, c = items[idx]
                    g = h // NREP
                    o = c * P - t * 512
                    col0 = max(o, 0)
                    ps_sc = psc.tile([P, 512], F32, tag="sc")
                    nc.tensor.matmul(
                        ps_sc[:, col0:512], kT[:, g, c * P:(c + 1) * P],
                        qT[:, h, t * 512 + col0:(t + 1) * 512],
                        start=True, stop=(o < 0))
                    if o >= 0:
                        # after the col0 shift the partial block is always the
                        # i' >= j triangle; accumulate the additive mask with
                        # a 128-col matmul (53ns) right behind the scores
                        nc.tensor.matmul(ps_sc[:, col0:col0 + P], ident,
                                         mask_sb, start=False, stop=True)
                    pt = ppool.tile([P, 512], BF16, tag="pt")
                    nc.scalar.activation(pt[:, col0:512], ps_sc[:, col0:512],
                                         AF.Exp)
                    live[idx] = (pt, col0)

                for i in range(min(DEPTH, len(items))):
                    front(i)
                ys = {}
                for idx, (h, c) in enumerate(items):
                    if idx + DEPTH < len(items):
                        front(idx + DEPTH)
                    g = h // NREP
                    if c == 0:
                        ps_y = psy.tile([P, 512], F32, tag="y")
                        ps_sden = pss.tile([P, 512], F32, tag="sden")
                        ys[h] = (ps_y, ps_sden)
                    ps_y, ps_sden = ys[h]
                    ps_s = ps_sden[0:1, :]
                    pt, col0 = live.pop(idx)
                    st = dict(start=(c == 0), stop=(c == nch - 1))
                    nc.tensor.matmul(ps_y[:, col0:512],
                                     v_sb[:, c, g * P:(g + 1) * P],
                                     pt[:, col0:512], **st)
                    nc.tensor.matmul(ps_s[:, col0:512], ones_col,
                                     pt[:, col0:512], **st)
                    if c == nch - 1:
                        # recip first (frees the single pss buffer fastest),
                        # then copy (frees ps_y); normalize the bf16 slice in
                        # place on the Pool engine once the broadcast lands
                        yslice = yT[:, h, t * 512:(t + 1) * 512]
                        rc = bpool.tile([1, 512], F32, tag="rc")
                        nc.vector.reciprocal(rc, ps_s)
                        nc.vector.tensor_copy(yslice, ps_y)
                        rb = bpool.tile([P, 512], F32, tag="rb")
                        nc.gpsimd.partition_broadcast(rb, rc, channels=P)
                        nc.vector.tensor_mul(yslice, yslice, rb)

        if DEBUG_DUMP:
            with tc.tile_pool(name="dbg", bufs=2) as dpool:
                for h in range(HG):
                    dt_ = dpool.tile([P, T], F32, tag="d")
                    nc.vector.tensor_copy(dt_, qT[:, h, :])
                    nc.sync.dma_start(d_qt[:, h, :], dt_)
                    dt_ = dpool.tile([P, T], F32, tag="d")
                    nc.vector.tensor_copy(dt_, yT[:, h, :])
                    nc.sync.dma_start(d_yt[:, h, :], dt_)
                for g in range(KVG):
                    dt_ = dpool.tile([P, T], F32, tag="d")
                    nc.vector.tensor_copy(dt_, kT[:, g, :])
                    nc.sync.dma_start(d_kt[:, g, :], dt_)
                dt_ = dpool.tile([P, TOKCH * KC], F32, tag="d")
                nc.vector.tensor_copy(dt_.rearrange("p (a b) -> p a b", a=TOKCH), v_sb[:, :, :])
                nc.sync.dma_start(d_v[:, :, :], dt_.rearrange("p (a b) -> p a b", a=TOKCH))

        # ================= phase C: output projection =====================
        if "C" not in PHASES:
            pass
        else:
         with tc.tile_pool(name="po", bufs=2, space="PSUM") as pso, \
             tc.tile_pool(name="so", bufs=3) as opool:
            for t in range(TOKCH):
                for ct in range(C // 512):
                    ps_o = pso.tile([P, 512], F32, tag="o")
                    last = (t == TOKCH - 1 and ct == C // 512 - 1)
                    ob = opool.tile([P, 512], F32, tag="ob")
                    if not last:
                        for hc in range(HG):
                            nc.tensor.matmul(
                                ps_o, yT[:, hc, t * P:(t + 1) * P],
                                wp_ts[ct][:, hc, :],
                                start=(hc == 0), stop=(hc == HG - 1))
                        nc.vector.tensor_copy(ob, ps_o)
                        nc.sync.dma_start(
                            out[t * P:(t + 1) * P, ct * 512:(ct + 1) * 512], ob)
                    else:
                        # final tile in two pipelined halves to shorten the
                        # copy->dma drain tail; halves go out on different
                        # queues so the transfers overlap
                        for q in range(2):
                            cs = slice(q * 256, (q + 1) * 256)
                            for hc in range(HG):
                                nc.tensor.matmul(
                                    ps_o[:, cs], yT[:, hc, t * P:(t + 1) * P],
                                    wp_ts[ct][:, hc, cs],
                                    start=(hc == 0), stop=(hc == HG - 1))
                            (nc.vector.tensor_copy if q == 0
                             else nc.scalar.copy)(ob[:, cs], ps_o[:, cs])
                            (nc.sync if q == 0 else nc.scalar).dma_start(
                                out[t * P:(t + 1) * P,
                                    ct * 512 + q * 256:ct * 512 + (q + 1) * 256],
                                ob[:, cs])
    nc.compile()
    return nc


_NC_CACHE = []


def _get_prog():
    if not _NC_CACHE:
        _NC_CACHE.append(_build())
    return _NC_CACHE[0]


def _make_in_maps(inputs):
    x, cos, sin = inputs["x"], inputs["cos"], inputs["sin"]
    wq, wk, wv, wproj = inputs["wq"], inputs["wk"], inputs["wv"], inputs["wproj"]
    bf = ml_dtypes.bfloat16
    # [p, tc, d] tiling (contiguous 4KB DMA rows)
    cos2 = np.ascontiguousarray(
        cos.reshape(TOKCH, P, D // 2).transpose(1, 0, 2), dtype=np.float32)
    sin2 = np.ascontiguousarray(
        sin.reshape(TOKCH, P, D // 2).transpose(1, 0, 2), dtype=np.float32)
    in_maps = []
    for core in range(8):
        b, g = core // 2, core % 2
        qs = slice(g * QC, (g + 1) * QC)
        ks = slice(g * KC, (g + 1) * KC)
        # x[b].T is [C, T]; tile to [tokch, p(C-chunk), co, tk]
        xtb = (x[b].T.astype(bf)
               .reshape(C // P, P, TOKCH, P)     # [co, p, tc, tk]
               .transpose(2, 1, 0, 3))           # [tc, p, co, tk]
        in_maps.append({
            "xt": np.ascontiguousarray(xtb),
            "wq": np.ascontiguousarray(wq[:, qs]).astype(bf),
            "wkv": np.ascontiguousarray(np.hstack([wk[:, ks], wv[:, ks]])).astype(bf),
            "wp": np.ascontiguousarray(wproj[qs, :]).astype(bf),
            "cosd": cos2,
            "sind": sin2,
        })
    return in_maps


def kernel(x, cos, sin, wq, wk, wv, wproj):
    nc = _get_prog()
    in_maps = _make_in_maps(dict(x=x, cos=cos, sin=sin, wq=wq, wk=wk, wv=wv, wproj=wproj))
    res = run_bass_kernel_spmd(nc, in_maps, core_ids=list(range(8))).results
    outp = np.empty((B, T, C), np.float32)
    for b in range(B):
        outp[b] = res[2 * b]["out"] + res[2 * b + 1]["out"]
    return outp



# revision 106
# speedup vs baseline: 1.2062x; 1.0021x over previous
"""Causal self-attention (GQA + RoPE + QK-norm) Trainium2 Bass kernel.

Sharding: 8 cores = 4 batches x 2 head-groups.  Core c -> batch c//2,
q heads (c%2)*8..+8, kv heads (c%2)*2..+2.  wproj is row-sharded, so each
core emits a partial (T, C) output; the host sums the two partials per batch.

Device-side layout strategy (per core):
  - x is fed pre-transposed (xT, [C, T]) and bf16-cast by the host.
  - QKV projections produce Q,K token-major ([tok, cols]); RoPE + rms-norm
    run token-major (free-axis per-head reductions), then 128x128 PE
    transposes produce qT/kT feature-major for the attention matmuls.
    V is produced token-major, which is exactly the p@v stationary layout.
  - scores are computed transposed (scoresT[tk, tq]) so that after exp the
    p tiles are already the moving operand for the p@v matmul; the softmax
    denominator comes from a ones-column matmul accumulated in PSUM.
  - exp has no max-subtraction: qk-norm bounds |s| <= sqrt(128) ~ 11.32.
  - output projection accumulates over the 8 local heads; partial written
    fp32 to DRAM.
"""

import numpy as np
import ml_dtypes
from contextlib import ExitStack

import concourse.bass as bass
import concourse.mybir as mybir
import concourse.tile as tile
from concourse import bacc
from concourse.bass_utils import run_bass_kernel_spmd
from concourse.masks import make_identity

BF16 = mybir.dt.bfloat16
F32 = mybir.dt.float32
F32R = mybir.dt.float32r
AF = mybir.ActivationFunctionType

B, T, C = 4, 2048, 2048
H, KV, D = 16, 4, 128
HG, KVG = H // 2, KV // 2          # per-core q heads (8), kv heads (2)
QC, KC = HG * D, KVG * D           # 1024, 256
P = 128
TOKCH = T // P                     # 16 token chunks
NREP = H // KV                     # 4
EPS = 1e-5
NEG = -1.0e5                       # additive causal mask (exp -> 0)


DEBUG_DUMP = False
PHASES = ("A", "B", "C")


def _build():
    nc = bacc.Bacc("TRN2", target_bir_lowering=False, debug=False, num_devices=8)
    # x pre-tiled by the host as [tokch, p, co, tk] so every DMA partition row
    # is 4KB contiguous (co*tk*2B) instead of 256B strided
    xt = nc.dram_tensor("xt", [TOKCH, P, C // P, P], BF16, kind="ExternalInput")
    wq = nc.dram_tensor("wq", [C, QC], BF16, kind="ExternalInput")
    wkv = nc.dram_tensor("wkv", [C, 2 * KC], BF16, kind="ExternalInput")
    wp = nc.dram_tensor("wp", [QC, C], BF16, kind="ExternalInput")
    # cos/sin pre-tiled by host as [p, tc, d] (contiguous 4KB rows)
    cosd = nc.dram_tensor("cosd", [P, TOKCH, D // 2], F32, kind="ExternalInput")
    sind = nc.dram_tensor("sind", [P, TOKCH, D // 2], F32, kind="ExternalInput")
    out = nc.dram_tensor("out", [T, C], F32, kind="ExternalOutput")
    if DEBUG_DUMP:
        d_qt = nc.dram_tensor("d_qt", [P, HG, T], F32, kind="ExternalOutput")
        d_kt = nc.dram_tensor("d_kt", [P, KVG, T], F32, kind="ExternalOutput")
        d_v = nc.dram_tensor("d_v", [P, TOKCH, KC], F32, kind="ExternalOutput")
        d_yt = nc.dram_tensor("d_yt", [P, HG, T], F32, kind="ExternalOutput")

    with tile.TileContext(nc) as tc, ExitStack() as ctx:
        singles = ctx.enter_context(tc.tile_pool(name="singles", bufs=1))
        # bufs must cover the V-lag window (xtile(t) is re-read by the lagged
        # V projection at iteration t+VLAG); the pool closes with phase A
        phase_a_pools = ExitStack()
        xpool = phase_a_pools.enter_context(tc.tile_pool(name="xa", bufs=8))

        # ---- prefetch the first x tile before the weight bulk so the PE
        # can start within a few us ----
        # ---- resident tensors ----
        # weight DMAs issued per-co round-robin over both HWDGE queues so
        # early co chunks land in consumption order and issue rate isn't
        # limited by one sequencer (~600ns per dma_start).  The first x
        # chunk + first co weights go out first so the PE starts ASAP.
        wq_sb = singles.tile([P, C // P, QC], BF16)
        wkv_sb = singles.tile([P, C // P, 2 * KC], BF16)
        wqr = wq.rearrange("(co p) q -> p co q", p=P)
        wkvr = wkv.rearrange("(co p) q -> p co q", p=P)
        cos_sb = singles.tile([P, TOKCH, D // 2], F32)
        sin_sb = singles.tile([P, TOKCH, D // 2], F32)
        # DMAs ordered by first consumption: iteration 0 runs three column
        # passes (q0 cols 0:512, q1 cols 512:1024, k) so its first rope --
        # the start of the 160us serialized DVE chain that bounds phase A --
        # only needs the q0 half of wq plus xtile0.
        xtile0 = xpool.tile([P, C // P, P], BF16, tag="xt")
        nc.sync.dma_start(xtile0[:, 0:4, :], xt[0, :, 0:4, :])
        qs = [nc.sync, nc.scalar]
        # first 2 co's full weight needs, then the k half (it plus the first
        # x chunks gates the k rope, the start of the serialized DVE chain)
        nc.scalar.dma_start(wq_sb[:, 0:2, 0:512], wqr[:, 0:2, 0:512])
        nc.sync.dma_start(wq_sb[:, 0:2, 512:1024], wqr[:, 0:2, 512:1024])
        for i, co in enumerate(range(0, C // P, 4)):
            qs[i % 2].dma_start(wkv_sb[:, co:co + 4, 0:KC],
                                wkvr[:, co:co + 4, 0:KC])
        for g4 in range(1, 4):
            nc.sync.dma_start(xtile0[:, 4 * g4:4 * (g4 + 1), :],
                              xt[0, :, 4 * g4:4 * (g4 + 1), :])
        nc.scalar.dma_start(cos_sb, cosd[:])
        nc.scalar.dma_start(sin_sb, sind[:])
        for i, co in enumerate(range(2, C // P, 2)):
            qs[i % 2].dma_start(wq_sb[:, co:co + 2, 0:512],
                                wqr[:, co:co + 2, 0:512])
        for i, co in enumerate(range(2, C // P, 2)):
            qs[i % 2].dma_start(wq_sb[:, co:co + 2, 512:1024],
                                wqr[:, co:co + 2, 512:1024])
        # V weights stream last (first consumed at iteration VLAG, ~70us in)
        for co in range(0, C // P, 4):
            nc.gpsimd.dma_start(wkv_sb[:, co:co + 4, KC:2 * KC],
                                wkvr[:, co:co + 4, KC:2 * KC])

        ident = singles.tile([P, P], BF16)
        make_identity(nc, ident)
        ones_col = singles.tile([P, 1], BF16)
        nc.vector.memset(ones_col, 1.0)
        zero_col = singles.tile([P, 1], F32)
        nc.vector.memset(zero_col, 0.0)
        eps_col = singles.tile([P, 1], F32)
        nc.vector.memset(eps_col, EPS)
        nc.const_aps.aps[(F32, 0.0)] = zero_col[:]
        nc.const_aps.aps[(F32, EPS)] = eps_col[:]
        # scratch for the dummy exp that prewarms the exp act-table at the
        # A->B phase boundary (overlaps the 1.28us table load)
        warm = singles.tile([1, 1], F32)
        # k-psum evacuation buffers (parity-alternated): the single-buffered
        # k psum is freed by a quick Act copy instead of being held through
        # the whole rope-k chain, so the next iteration's k matmuls never WAR
        evk_a = singles.tile([P, KC], F32)
        evk_b = singles.tile([P, KC], F32)

        # diagonal-block mask: keep where i >= j (j = tk partition, i = tq
        # free).  bf16 so it can be ADDED into the scores psum by a 128-col
        # matmul (ident.T @ mask) instead of a DVE op in the exp chain.
        mask_sb = singles.tile([P, P], BF16)
        nc.vector.memset(mask_sb, 0.0)
        nc.gpsimd.affine_select(
            out=mask_sb, in_=mask_sb,
            compare_op=mybir.AluOpType.is_ge, fill=NEG,
            base=0, pattern=[[1, P]], channel_multiplier=-1,
        )

        qT = singles.tile([P, HG, T], BF16)      # [d, h, tok]
        kT = singles.tile([P, KVG, T], BF16)
        v_sb = singles.tile([P, TOKCH, KC], BF16)  # [tok%128, chunk, vcol]
        yT = singles.tile([P, HG, T], BF16)

        # ================= phase A: QKV proj + RoPE + qk-norm =============
        if "A" not in PHASES:
            pass
        else:
         with phase_a_pools, \
             tc.tile_pool(name="pa", bufs=2, space="PSUM") as pps, \
             tc.tile_pool(name="pkv", bufs=1, space="PSUM") as pkv, \
             tc.tile_pool(name="sa", bufs=3) as spool:
            # The V projection is split out of the QK pass and lagged by VLAG
            # chunks: the final VLAG V-chunks are pure PE work that runs while
            # the last rope chains (DVE) drain, so phase B starts without
            # waiting on the phase-A tail.
            VLAG = 6
            nco = C // P
            xtiles = {}
            # transposes lag one iteration behind their rope chain so they
            # never sit dep-blocked in the PE's 4-deep wait queue
            pending_tr = []

            def flush_trs():
                while pending_tr:
                    qbf, dstT, h0, nh, tt = pending_tr.pop(0)
                    pst = pkv.tile([P, 4, P], BF16, tag="tr")
                    for i in range(nh):
                        nc.tensor.transpose(pst[:, i, :], qbf[:, i, :], ident)
                    nc.scalar.copy(
                        dstT[:, h0:h0 + nh, tt * P:(tt + 1) * P], pst[:, 0:nh, :])

            def v_chunk(tv):
                xv = xtiles.pop(tv)
                # alternate psum tags so consecutive V chunks don't serialize
                # on one buffer's Act-copy release
                ps_v = pkv.tile([P, KC], F32, tag=("v" if tv % 2 == 0 else "v2"))
                for co in range(nco):
                    nc.tensor.matmul(ps_v, xv[:, co, :],
                                     wkv_sb[:, co, KC:2 * KC],
                                     start=(co == 0), stop=(co == nco - 1))
                # cast straight to resident token-major buffer (Act engine;
                # DVE is the critical engine in this phase)
                nc.scalar.copy(v_sb[:, tv, :], ps_v)

            for t in range(TOKCH):
                if t == 0:
                    xtile = xtile0
                else:
                    xtile = xpool.tile([P, C // P, P], BF16, tag="xt")
                    nc.sync.dma_start(xtile, xt[t])
                xtiles[t] = xtile
                ps_q0 = pps.tile([P, 512], F32, tag="q0")
                ps_q1 = pps.tile([P, 512], F32, tag="q1")
                ps_k = pkv.tile([P, KC], F32, tag="k")

                # Q/K: fused multi-head rope + rms-norm + cast + transpose
                def rope_norm(ps, nh, dstT, h0, qscale, rsq_dve=False):
                    h2 = D // 2
                    v4 = ps.rearrange("p (h a d) -> p h a d", h=nh, a=2)
                    q1, q2 = v4[:, :, 0, :], v4[:, :, 1, :]
                    r = spool.tile([P, nh, 2, h2], F32, tag=f"rope{nh}")
                    r1, r2 = r[:, :, 0, :], r[:, :, 1, :]
                    s2 = spool.tile([P, nh, h2], F32, tag=f"scr{nh}")
                    cs = cos_sb[:, t, None, :].to_broadcast([P, nh, h2])
                    sn = sin_sb[:, t, None, :].to_broadcast([P, nh, h2])
                    nc.vector.tensor_mul(r1, q1, cs)
                    nc.vector.tensor_mul(s2, q2, sn)
                    nc.vector.tensor_sub(r1, r1, s2)
                    nc.vector.tensor_mul(r2, q1, sn)
                    nc.vector.tensor_mul(s2, q2, cs)
                    nc.vector.tensor_add(r2, r2, s2)
                    rf = r.rearrange("p h a d -> p h (a d)")
                    sq = spool.tile([P, nh, D], F32, tag=f"sq{nh}")
                    ss = spool.tile([P, nh], F32, tag=f"ss{nh}")
                    if rsq_dve:
                        # keep the last iteration's rope entirely off the Act
                        # engine so phase B's first exps aren't queued behind it
                        nc.vector.tensor_mul(sq, rf, rf)
                    else:
                        nc.scalar.activation(sq, rf, AF.Square)
                    nc.vector.tensor_reduce(ss, sq, axis=mybir.AxisListType.X,
                                            op=mybir.AluOpType.add)
                    rq = spool.tile([P, nh], F32, tag=f"rq{nh}")
                    if rsq_dve:
                        # DVE-only fast inverse sqrt (bit trick + 2 Newton
                        # steps, qscale folded into the last).  Used for the
                        # final token chunk so the previous iteration's Sqrt
                        # is the Act engine's last sqrt-set op and the exp
                        # table load hides behind the V tail.
                        ALU = mybir.AluOpType
                        I32 = mybir.dt.int32
                        fx = spool.tile([P, nh], F32, tag=f"fx{nh}")
                        nc.vector.tensor_scalar(fx, ss, 1.0 / D, EPS,
                                                op0=ALU.mult, op1=ALU.add)
                        fj = spool.tile([P, nh], I32, tag=f"fj{nh}")
                        nc.vector.tensor_scalar(fj, fx[:].bitcast(I32), 1, None,
                                                op0=ALU.logical_shift_right)
                        nc.vector.tensor_scalar(fj, fj, -1, 0x5f3759df + 1,
                                                op0=ALU.bitwise_xor, op1=ALU.add)
                        fy = fj[:].bitcast(F32)
                        fa = spool.tile([P, nh], F32, tag=f"fa{nh}")
                        nc.vector.tensor_mul(fa, fy, fy)
                        nc.vector.tensor_mul(fa, fa, fx)
                        nc.vector.tensor_scalar(fa, fa, -0.5, 1.5,
                                                op0=ALU.mult, op1=ALU.add)
                        nc.vector.tensor_mul(rq, fy, fa)
                        nc.vector.tensor_mul(fa, rq, rq)
                        nc.vector.tensor_mul(fa, fa, fx)
                        nc.vector.tensor_scalar(fa, fa, -0.5 * qscale,
                                                1.5 * qscale,
                                                op0=ALU.mult, op1=ALU.add)
                        nc.vector.tensor_mul(rq, rq, fa)
                    else:
                        rt = spool.tile([P, nh], F32, tag=f"rt{nh}")
                        nc.scalar.activation(rt, ss, AF.Sqrt, scale=1.0 / D,
                                             bias=EPS)
                        nc.vector.reciprocal(rq, rt)
                        if qscale != 1.0:
                            nc.vector.tensor_scalar_mul(rq, rq, qscale)
                    qbf = spool.tile([P, nh, D], BF16, tag=f"qbf{nh}")
                    nc.vector.tensor_mul(qbf, rf, rq[:, :, None].to_broadcast([P, nh, D]))
                    pending_tr.append((qbf, dstT, h0, nh, t))

                qsc = 1.0 / float(np.sqrt(D))
                for co in range(nco):
                    lhsT = xtile[:, co, :]
                    st = dict(start=(co == 0), stop=(co == nco - 1))
                    nc.tensor.matmul(ps_q0, lhsT, wq_sb[:, co, 0:512], **st)
                    nc.tensor.matmul(ps_q1, lhsT, wq_sb[:, co, 512:1024], **st)
                    nc.tensor.matmul(ps_k, lhsT, wkv_sb[:, co, 0:KC], **st)
                evk = evk_a if t % 2 == 0 else evk_b
                nc.scalar.copy(evk, ps_k)
                rope_norm(ps_q0, 4, qT, 0, qsc)
                rope_norm(ps_q1, 4, qT, 4, qsc)
                rope_norm(evk[:], KVG, kT, 0, 1.0)
                if t >= VLAG:
                    v_chunk(t - VLAG)
                # previous iteration's transposes: rope chains long done
                flush_trs()
                if t == TOKCH - 1:
                    # prewarm the exp act-table; the 1.28us load runs behind
                    # the V tail
                    nc.scalar.activation(warm, zero_col[0:1, :], AF.Exp)

            # lagged V tail: pure PE work that covers the final rope chains;
            # the last transposes flush once their rope chain has had V cover
            for tv in range(TOKCH - VLAG, TOKCH):
                v_chunk(tv)
                if tv == TOKCH - 2:
                    flush_trs()

        # ================= phase B: attention ============================
        # wp prefetch: issue at phase-B start so the tiles are resident long
        # before phase C begins (phase-A pools have closed, SBUF is free)
        wpool = ctx.enter_context(tc.tile_pool(name="wp", bufs=1))
        wpr = wp.rearrange("(hc p) c -> p hc c", p=P)
        wp_ts = []
        for ct in range(C // 512):
            wp_t = wpool.tile([P, HG, 512], BF16, tag=f"wpt{ct}")
            nc.sync.dma_start(wp_t, wpr[:, :, ct * 512:(ct + 1) * 512])
            wp_ts.append(wp_t)

        if "B" not in PHASES:
            pass
        else:
         with tc.tile_pool(name="psc", bufs=4, space="PSUM") as psc, \
             tc.tile_pool(name="psy", bufs=2, space="PSUM") as psy, \
             tc.tile_pool(name="pss", bufs=2, space="PSUM") as pss, \
             tc.tile_pool(name="pb", bufs=6) as ppool, \
             tc.tile_pool(name="sb", bufs=3) as bpool:
            NT = T // 512  # 4 tq tiles
            # software pipeline: the PE queue is in-order, so scores for
            # chunk idx+DEPTH are emitted before pv/ones of chunk idx; the
            # scores->mask->exp chain (~1.6us) hides behind DEPTH chunks of
            # PE work.  The (h, c) stream is flattened so the pipeline also
            # covers head boundaries.
            DEPTH = 4
            for t in range(NT):
                nch = 4 * (t + 1)
                items = [(h, c) for h in range(HG) for c in range(nch)]
                live = {}

                def front(idx):
                    h, c = items[idx]
                    g = h // NREP
                    o = c * P - t * 512
                    col0 = max(o, 0)
                    ps_sc = psc.tile([P, 512], F32, tag="sc")
                    nc.tensor.matmul(
                        ps_sc[:, col0:512], kT[:, g, c * P:(c + 1) * P],
                        qT[:, h, t * 512 + col0:(t + 1) * 512],
                        start=True, stop=(o < 0))
                    if o >= 0:
                        # after the col0 shift the partial block is always the
                        # i' >= j triangle; accumulate the additive mask with
                        # a 128-col matmul (53ns) right behind the scores
                        nc.tensor.matmul(ps_sc[:, col0:col0 + P], ident,
                                         mask_sb, start=False, stop=True)
                    pt = ppool.tile([P, 512], BF16, tag="pt")
                    nc.scalar.activation(pt[:, col0:512], ps_sc[:, col0:512],
                                         AF.Exp)
                    live[idx] = (pt, col0)

                for i in range(min(DEPTH, len(items))):
                    front(i)
                ys = {}
                for idx, (h, c) in enumerate(items):
                    if idx + DEPTH < len(items):
                        front(idx + DEPTH)
                    g = h // NREP
                    if c == 0:
                        ps_y = psy.tile([P, 512], F32, tag="y")
                        ps_sden = pss.tile([P, 512], F32, tag="sden")
                        ys[h] = (ps_y, ps_sden)
                    ps_y, ps_sden = ys[h]
                    ps_s = ps_sden[0:1, :]
                    pt, col0 = live.pop(idx)
                    st = dict(start=(c == 0), stop=(c == nch - 1))
                    nc.tensor.matmul(ps_y[:, col0:512],
                                     v_sb[:, c, g * P:(g + 1) * P],
                                     pt[:, col0:512], **st)
                    nc.tensor.matmul(ps_s[:, col0:512], ones_col,
                                     pt[:, col0:512], **st)
                    if c == nch - 1:
                        # recip first (frees the single pss buffer fastest),
                        # then copy (frees ps_y); normalize the bf16 slice in
                        # place on the Pool engine once the broadcast lands
                        yslice = yT[:, h, t * 512:(t + 1) * 512]
                        rc = bpool.tile([1, 512], F32, tag="rc")
                        nc.vector.reciprocal(rc, ps_s)
                        nc.vector.tensor_copy(yslice, ps_y)
                        rb = bpool.tile([P, 512], F32, tag="rb")
                        nc.gpsimd.partition_broadcast(rb, rc, channels=P)
                        nc.vector.tensor_mul(yslice, yslice, rb)

        if DEBUG_DUMP:
            with tc.tile_pool(name="dbg", bufs=2) as dpool:
                for h in range(HG):
                    dt_ = dpool.tile([P, T], F32, tag="d")
                    nc.vector.tensor_copy(dt_, qT[:, h, :])
                    nc.sync.dma_start(d_qt[:, h, :], dt_)
                    dt_ = dpool.tile([P, T], F32, tag="d")
                    nc.vector.tensor_copy(dt_, yT[:, h, :])
                    nc.sync.dma_start(d_yt[:, h, :], dt_)
                for g in range(KVG):
                    dt_ = dpool.tile([P, T], F32, tag="d")
                    nc.vector.tensor_copy(dt_, kT[:, g, :])
                    nc.sync.dma_start(d_kt[:, g, :], dt_)
                dt_ = dpool.tile([P, TOKCH * KC], F32, tag="d")
                nc.vector.tensor_copy(dt_.rearrange("p (a b) -> p a b", a=TOKCH), v_sb[:, :, :])
                nc.sync.dma_start(d_v[:, :, :], dt_.rearrange("p (a b) -> p a b", a=TOKCH))

        # ================= phase C: output projection =====================
        if "C" not in PHASES:
            pass
        else:
         with tc.tile_pool(name="po", bufs=2, space="PSUM") as pso, \
             tc.tile_pool(name="so", bufs=3) as opool:
            for t in range(TOKCH):
                for ct in range(C // 512):
                    ps_o = pso.tile([P, 512], F32, tag="o")
                    last = (t == TOKCH - 1 and ct == C // 512 - 1)
                    ob = opool.tile([P, 512], F32, tag="ob")
                    if not last:
                        for hc in range(HG):
                            nc.tensor.matmul(
                                ps_o, yT[:, hc, t * P:(t + 1) * P],
                                wp_ts[ct][:, hc, :],
                                start=(hc == 0), stop=(hc == HG - 1))
                        nc.vector.tensor_copy(ob, ps_o)
                        nc.sync.dma_start(
                            out[t * P:(t + 1) * P, ct * 512:(ct + 1) * 512], ob)
                    else:
                        # final tile in two pipelined halves to shorten the
                        # copy->dma drain tail; halves go out on different
                        # queues so the transfers overlap
                        for q in range(2):
                            cs = slice(q * 256, (q + 1) * 256)
                            for hc in range(HG):
                                nc.tensor.matmul(
                                    ps_o[:, cs], yT[:, hc, t * P:(t + 1) * P],
                                    wp_ts[ct][:, hc, cs],
                                    start=(hc == 0), stop=(hc == HG - 1))
                            (nc.vector.tensor_copy if q == 0
                             else nc.scalar.copy)(ob[:, cs], ps_o[:, cs])
                            (nc.sync if q == 0 else nc.scalar).dma_start(
                                out[t * P:(t + 1) * P,
                                    ct * 512 + q * 256:ct * 512 + (q + 1) * 256],
                                ob[:, cs])
    nc.compile()
    return nc


_NC_CACHE = []


def _get_prog():
    if not _NC_CACHE:
        _NC_CACHE.append(_build())
    return _NC_CACHE[0]


def _make_in_maps(inputs):
    x, cos, sin = inputs["x"], inputs["cos"], inputs["sin"]
    wq, wk, wv, wproj = inputs["wq"], inputs["wk"], inputs["wv"], inputs["wproj"]
    bf = ml_dtypes.bfloat16
    # [p, tc, d] tiling (contiguous 4KB DMA rows)
    cos2 = np.ascontiguousarray(
        cos.reshape(TOKCH, P, D // 2).transpose(1, 0, 2), dtype=np.float32)
    sin2 = np.ascontiguousarray(
        sin.reshape(TOKCH, P, D // 2).transpose(1, 0, 2), dtype=np.float32)
    in_maps = []
    for core in range(8):
        b, g = core // 2, core % 2
        qs = slice(g * QC, (g + 1) * QC)
        ks = slice(g * KC, (g + 1) * KC)
        # x[b].T is [C, T]; tile to [tokch, p(C-chunk), co, tk]
        xtb = (x[b].T.astype(bf)
               .reshape(C // P, P, TOKCH, P)     # [co, p, tc, tk]
               .transpose(2, 1, 0, 3))           # [tc, p, co, tk]
        in_maps.append({
            "xt": np.ascontiguousarray(xtb),
            "wq": np.ascontiguousarray(wq[:, qs]).astype(bf),
            "wkv": np.ascontiguousarray(np.hstack([wk[:, ks], wv[:, ks]])).astype(bf),
            "wp": np.ascontiguousarray(wproj[qs, :]).astype(bf),
            "cosd": cos2,
            "sind": sin2,
        })
    return in_maps


def kernel(x, cos, sin, wq, wk, wv, wproj):
    nc = _get_prog()
    in_maps = _make_in_maps(dict(x=x, cos=cos, sin=sin, wq=wq, wk=wk, wv=wv, wproj=wproj))
    res = run_bass_kernel_spmd(nc, in_maps, core_ids=list(range(8))).results
    outp = np.empty((B, T, C), np.float32)
    for b in range(B):
        outp[b] = res[2 * b]["out"] + res[2 * b + 1]["out"]
    return outp

